# revision 7
# baseline (speedup 1.0000x reference)
"""Trainium2 Bass kernel v3 for nn_BoxEncoder (B=128, T=200, NC=3, NB=2, D=512, DH=256).

Data-parallel over batch: 16 batch items per core x 8 cores; partition
p = bt*8 + q.  Per partition: 75 dist tokens (+1 pad) and 150 box tokens
(+2 pad), processed as 4-slot transpose chunks.

v3 vs v2 (~2x): the whole kernel is built to keep the PE p-state warm
(cost model: matmuls run 2x faster once the PE has been ~continuously
busy for 3us; any long stall drops it back):
 - z is computed TRANSPOSED (weights-stationary): zT = W1bandedT @ cta_s,
   so gelu(zT) directly yields hT = the lhsT of the W2 matmuls.  The
   dma_start_transpose of h (SP-queue serial 1.24us each + 900ns sem) is
   gone - that chain caused the recurring 3-5us PE stalls in v2.
 - LN rstd is folded into the geometry features BEFORE the z matmul
   (f0..f9 *= rstd per token), so gelu needs no per-partition scale and
   batches [128,512] over 4 slots per call.  cx,cy are duplicated into
   f18,f19 (unscaled) for the center_w rows of the extras matmul.
 - all matmuls are full-K (no tile_position): banded *weights* (zero rows
   outside the slot's 32-band) instead of banded matmuls; the extras and
   z matmuls share one LDWEIGHTS of the feature chunk.
 - variance via a 12-slot x 10-feature gram pack (13 transposes instead
   of 38) and a single batched sqrt+reciprocal.
 - PE pre-warm: a dozen junk matmuls issued at t~1us keep the PE busy
   during the DVE feature-prep phase so the clock is warm when real
   matmuls start.
 - staging copies round-robin DVE/ACT/GPSIMD; out DMAs on the otherwise
   idle SP queue; one ACT table preload for Gelu right after the rstd
   sqrt so no table load lands inside the steady-state loop.
"""

import numpy as np
import ml_dtypes

B, T, NCAM, NB, D, DH = 128, 200, 3, 2, 512, 256
IW, IH = 640.0, 400.0
NCORES = 8
BPC = B // NCORES
JB = 150                  # real box slots per partition
JBP = 156                 # padded (38 chunks use 152; gram packs use 156)
JD = 75                   # real dist slots
JDP = 76                  # padded (19 chunks)
NCH = 38                  # box chunks
NDC = 19                  # dist chunks
NOCT = 19                 # box octs (2 chunks = 8 slots each; last has 6)
NGP = 13                  # gram packs (12 slots x 10 feats)

_CACHE = {}


def _build_nc():
    from contextlib import ExitStack
    import concourse.bacc as bacc
    import concourse.mybir as mybir
    import concourse.tile as tile

    f32 = mybir.dt.float32
    bf16 = mybir.dt.bfloat16
    A = mybir.AluOpType
    AF = mybir.ActivationFunctionType

    # bpk bf16 column offsets
    C_W1B = 0                       # 8 x 128 (band b: hi, lo)
    C_W2HI = C_W1B + 8 * 128
    C_W2LO = C_W2HI + 512
    C_W2XB = C_W2LO + 512           # 12 x 512 (cam c, band b)
    C_G = C_W2XB + 12 * 512
    C_ID = C_G + 128
    NBF = C_ID + 128
    C_P1 = C_W2XB            # part 1 = w1b, w2hi, w2lo (cols 0..C_W2XB)


    nc = bacc.Bacc("TRN2", target_bir_lowering=False, debug=False,
                   num_devices=NCORES)
    fpk = nc.declare_dram_parameter("fpk", [128, 900 + 128], f32, isOutput=False)
    bpk = nc.declare_dram_parameter("bpk", [128, NBF], bf16, isOutput=False)
    out_d = nc.declare_dram_parameter("out", [BPC, 1800, D], bf16, isOutput=True)

    with ExitStack() as ctx:
        tc = ctx.enter_context(tile.TileContext(nc))
        cp = ctx.enter_context(tc.tile_pool(name="const", bufs=1))
        sc = ctx.enter_context(tc.tile_pool(name="scratch", bufs=1))
        # PSUM pools (8 banks): zp 4x[128,512]=4 + opa 3x[128,512]=3 +
        # tp 2x[128,128]=0.5
        zp = ctx.enter_context(tc.tile_pool(name="zp", bufs=4, space="PSUM"))
        opa = ctx.enter_context(tc.tile_pool(name="opa", bufs=2, space="PSUM"))
        gpck = ctx.enter_context(tc.tile_pool(name="gpck", bufs=2))
        octp = ctx.enter_context(tc.tile_pool(name="octp", bufs=4))
        cdp = ctx.enter_context(tc.tile_pool(name="cdp", bufs=1))
        htp = ctx.enter_context(tc.tile_pool(name="htp", bufs=8))
        bstg = ctx.enter_context(tc.tile_pool(name="bstage", bufs=3))
        dstg = ctx.enter_context(tc.tile_pool(name="dstage", bufs=3))

        fpack = cp.tile([128, 900 + 128], f32)
        nc.sync.dma_start(fpack[:], fpk[:])
        bpack = cp.tile([128, NBF], bf16)
        # split the weight load: small part (warm-up, gram, z) first so the
        # big w2xb block (12KB/partition) doesn't gate the early phases
        nc.sync.dma_start(bpack[:, C_G:NBF], bpk[:, C_G:NBF])
        nc.sync.dma_start(bpack[:, 0:C_P1], bpk[:, 0:C_P1])
        nc.sync.dma_start(bpack[:, C_P1:C_G], bpk[:, C_P1:C_G])

        raw = fpack[:, 0:900]
        idf = fpack[:, 900:1028]
        w1b = [(bpack[:, C_W1B + (2 * b) * 128: C_W1B + (2 * b + 1) * 128],
                bpack[:, C_W1B + (2 * b + 1) * 128: C_W1B + (2 * b + 2) * 128])
               for b in range(4)]
        w2hi = bpack[:, C_W2HI:C_W2HI + 512]
        w2lo = bpack[:, C_W2LO:C_W2LO + 512]
        w2xb = [[bpack[:, C_W2XB + (c * 4 + b) * 512: C_W2XB + (c * 4 + b + 1) * 512]
                 for b in range(4)] for c in range(3)]
        Gblk = bpack[:, C_G:C_G + 128]
        idb = bpack[:, C_ID:C_ID + 128]

        # ---------------- PE pre-warm: junk matmuls over bpack ----------
        for _ in range(13):
            wps = opa.tile([128, 2 * D], f32, tag="oa", name="oa")
            nc.tensor.matmul(wps[:, 0:512], bpack[:, 0:128], bpack[:, 0:512],
                             start=True, stop=True)

        # ---------------- P1: feature planes ----------------
        TFB = cp.tile([128, JBP * 32], bf16)
        TFD = cp.tile([128, JDP * 32], bf16)
        TFb = TFB.rearrange("p (j f) -> p j f", f=32)
        TFd = TFD.rearrange("p (j f) -> p j f", f=32)
        # zeros: box f16,17 + f20..31 + pad slots; dist all but f16,f17
        # zeros via ACT (uint32-bitcast x0 is NaN-safe); tiny pads + the 1.0
        # fill on DVE.  Pad slots' f0..9 must precede the gram prepack.
        nc.vector.memset(TFb[:, JB:JBP, 0:16], 0.0)
        nc.vector.memset(TFb[:, JB:JBP, 18:20], 0.0)
        nc.scalar.memzero(TFb[:, :, 16:18])
        nc.scalar.memzero(TFb[:, :, 20:32])
        nc.scalar.memzero(TFd[:, :, 0:16])
        nc.scalar.memzero(TFd[:, :, 18:32])
        nc.vector.memset(TFd[:, :, 17], 1.0)
        nc.vector.memset(TFd[:, JD:JDP, 16], 0.0)

        TFr = TFb[:, 0:JB, :]
        raw6 = raw.rearrange("p (b s) -> p b s", s=6)
        rawp = raw.rearrange("p (m g s) -> p m g s", g=2, s=6)
        TFbp = TFB[:, 0:JB * 32].rearrange("p (m g f) -> p m g f", g=2, f=32)

        sPres = sc.tile([128, JB], f32)
        sKey = sc.tile([128, JB], f32)
        sSwap = sc.tile([128, JD], f32)
        sD = sc.tile([128, JD], f32)
        sSD = sc.tile([128, JD], f32)
        sw = [sc.tile([128, JB], f32, tag=f"swp{i}", name=f"swp{i}")
              for i in range(6)]
        sT0 = sc.tile([128, JB], f32)
        sT1 = sc.tile([128, JB], f32)

        nc.vector.tensor_tensor(sT0[:], raw6[:, :, 0], raw6[:, :, 1], A.add)
        nc.vector.tensor_tensor(sT1[:], raw6[:, :, 2], raw6[:, :, 3], A.add)
        nc.vector.tensor_tensor(sT0[:], sT0[:], sT1[:], A.add)
        nc.vector.tensor_scalar(sPres[:], sT0[:], 0.0, None, A.not_equal)
        nc.vector.scalar_tensor_tensor(sKey[:], sPres[:], -1000.0,
                                       raw6[:, :, 4], A.mult, A.add)
        sKeyp = sKey.rearrange("p (m g) -> p m g", g=2)
        nc.vector.tensor_tensor(sSwap[:], sKeyp[:, :, 1], sKeyp[:, :, 0], A.is_lt)

        for i in range(6):
            ve, vo = rawp[:, :, 0, i], rawp[:, :, 1, i]
            dst = sw[i].rearrange("p (m g) -> p m g", g=2)
            nc.vector.tensor_tensor(sD[:], vo, ve, A.subtract)
            nc.vector.tensor_tensor(sSD[:], sD[:], sSwap[:], A.mult)
            nc.vector.tensor_tensor(dst[:, :, 0], ve, sSD[:], A.add)
            nc.vector.tensor_tensor(dst[:, :, 1], vo, sSD[:], A.subtract)
        sPresP = sPres.rearrange("p (m g) -> p m g", g=2)
        nc.vector.tensor_tensor(sD[:], sPresP[:, :, 1], sPresP[:, :, 0], A.subtract)
        nc.vector.tensor_tensor(sSD[:], sD[:], sSwap[:], A.mult)
        nc.vector.tensor_tensor(TFbp[:, :, 0, 14], sPresP[:, :, 0], sSD[:], A.add)
        nc.vector.tensor_tensor(TFbp[:, :, 1, 14], sPresP[:, :, 1], sSD[:], A.subtract)

        sX1, sY1, sX2, sY2, sCat, sConf = sw
        # all derived geometry in f32 scratch (bf16-rounded coords would
        # catastrophically cancel in w/h near zero -> aspect blows up)
        sW32 = sc.tile([128, JB], f32)
        sH32 = sc.tile([128, JB], f32)
        sCx = sc.tile([128, JB], f32)
        sCy = sc.tile([128, JB], f32)
        nc.vector.tensor_scalar(TFr[:, :, 0], sX1[:], 1.0 / IW, None, A.mult)
        nc.vector.tensor_scalar(TFr[:, :, 1], sY1[:], 1.0 / IH, None, A.mult)
        nc.vector.tensor_scalar(TFr[:, :, 2], sX2[:], 1.0 / IW, None, A.mult)
        nc.vector.tensor_scalar(TFr[:, :, 3], sY2[:], 1.0 / IH, None, A.mult)
        nc.vector.tensor_tensor(sW32[:], sX2[:], sX1[:], A.subtract)
        nc.vector.tensor_tensor(sH32[:], sY2[:], sY1[:], A.subtract)
        nc.vector.tensor_tensor(sCx[:], sX1[:], sX2[:], A.add)
        nc.vector.tensor_tensor(sCy[:], sY1[:], sY2[:], A.add)
        nc.vector.tensor_scalar(TFr[:, :, 4], sW32[:], 1.0 / IW, None, A.mult)
        nc.vector.tensor_scalar(TFr[:, :, 5], sH32[:], 1.0 / IH, None, A.mult)
        nc.vector.tensor_scalar(TFr[:, :, 6], sCx[:], 1.0 / IW, None, A.mult)
        nc.vector.tensor_scalar(TFr[:, :, 7], sCy[:], 1.0 / IH, None, A.mult)
        nc.vector.tensor_scalar(TFr[:, :, 18], sCx[:], 1.0 / IW, None, A.mult)
        nc.vector.tensor_scalar(TFr[:, :, 19], sCy[:], 1.0 / IH, None, A.mult)
        sT2 = sc.tile([128, JB], f32)
        nc.vector.tensor_tensor(sT2[:], sW32[:], sH32[:], A.mult)
        nc.vector.tensor_scalar(TFr[:, :, 8], sT2[:], 1.0 / (IW * IH), None, A.mult)
        sHp = sT0
        nc.vector.tensor_scalar(sHp[:], sH32[:], 1e-6 * IH, 1.0 / IH, A.add, A.mult)
        sR = sT1
        nc.vector.reciprocal(sR[:], sHp[:])
        nc.vector.tensor_scalar(sT2[:], sW32[:], 1.0 / IW, None, A.mult)
        nc.vector.tensor_tensor(TFr[:, :, 9], sT2[:], sR[:], A.mult)
        for k in range(3):
            nc.vector.scalar_tensor_tensor(TFr[:, :, 10 + k], sCat[:], float(k),
                                           TFr[:, :, 14], A.is_equal, A.mult)
        nc.vector.tensor_tensor(TFr[:, :, 13], sConf[:], TFr[:, :, 14], A.mult)
        nc.vector.tensor_scalar(TFr[:, :, 15], TFr[:, :, 14], -1.0, 1.0,
                                A.mult, A.add)
        # dist features: f16 = 0.5*sqrt(dx2^2+dy2^2)/IW-scaled, f17 = 1
        sDx = sc.tile([128, JD], f32)
        sDy = sc.tile([128, JD], f32)
        sCxp = sCx.rearrange("p (m g) -> p m g", g=2)
        sCyp = sCy.rearrange("p (m g) -> p m g", g=2)
        nc.vector.tensor_tensor(sDx[:], sCxp[:, :, 0], sCxp[:, :, 1], A.subtract)
        nc.vector.tensor_tensor(sDy[:], sCyp[:, :, 0], sCyp[:, :, 1], A.subtract)
        nc.vector.tensor_scalar(sDx[:], sDx[:], 1.0 / IW, None, A.mult)
        nc.vector.tensor_scalar(sDy[:], sDy[:], 1.0 / IH, None, A.mult)
        nc.vector.tensor_tensor(sDx[:], sDx[:], sDx[:], A.mult)
        nc.vector.tensor_tensor(sDy[:], sDy[:], sDy[:], A.mult)
        nc.vector.tensor_tensor(sDx[:], sDx[:], sDy[:], A.add)
        nc.scalar.activation(TFd[:, 0:JD, 16], sDx[:], AF.Sqrt, scale=0.25)

        # ---------------- P2: gram variance ----------------
        v = sc.tile([128, 160], f32)
        copy_rr = [nc.vector.tensor_copy, nc.scalar.copy]

        # prepack geom features f0..9 of all 156 slots contiguously; each
        # 128-col transpose window overlaps 8 cols into the next pack, which
        # land on zero rows of Gblk (harmless).
        gprep = sc.tile([128, 13 * 120 + 8], bf16)
        nc.vector.memset(gprep[:, 13 * 120:], 0.0)
        nc.vector.tensor_copy(
            gprep[:, 0:1560].rearrange("p (j f) -> p j f", f=10),
            TFb[:, 0:156, 0:10])

        for gi in range(NGP):
            s0 = 12 * gi
            src = TFb[:, s0:s0 + 12, 0:10]
            pst = zp.tile([128, 1024], bf16, tag="z", name="z")
            ps = pst[:, 0:128]
            nc.tensor.transpose(ps[:], gprep[:, 120 * gi:120 * gi + 128], idb)
            pk = gpck.tile([128, 128], bf16, tag="gp", name="gp")
            nc.scalar.copy(pk[:], ps[:])
            yt = zp.tile([128, 512], f32, tag="z", name="z")
            y = yt[:, 0:128]
            nc.tensor.matmul(y, pk[:], Gblk, start=True, stop=True)
            tmp = sc.tile([128, 120], f32, tag="gtmp", name="gtmp")
            nc.vector.tensor_tensor(tmp[:], src, y[:, 0:120], A.mult)
            nc.vector.tensor_reduce(v[:, s0:s0 + 12],
                                    tmp.rearrange("p (j f) -> p j f", f=10),
                                    mybir.AxisListType.X, A.add)

        # ---------------- P2b: dist transposes ----------------
        cta_d = cp.tile([128, NDC * 128], bf16)
        for dc in range(NDC):
            pst = zp.tile([128, 1024], bf16, tag="z", name="z")
            ps = pst[:, 0:128]
            nc.tensor.transpose(ps[:], TFd[:, 4 * dc:4 * dc + 4, :], idb)
            copy_rr[dc % 2](cta_d[:, dc * 128:(dc + 1) * 128], ps[:])

        # ---------------- P3: rstd + feature scale + gelu preload --------
        eps = sc.tile([128, 1], f32)
        nc.vector.memset(eps[:], 1e-5)
        sd = sc.tile([128, 156], f32)
        rstd = sc.tile([128, 156], f32)
        nc.scalar.activation(sd[:], v[:, 0:156], AF.Sqrt,
                             bias=eps[:], scale=1.0 / DH)
        nc.vector.reciprocal(rstd[:], sd[:])
        # preload the Gelu ACT table off the critical path
        gjunk = sc.tile([128, 8], bf16)
        nc.scalar.activation(gjunk[:], sd[:, 0:8], AF.Gelu)
        # scale geometry features f0..9 by rstd (per token)
        for f in range(10):
            nc.vector.tensor_tensor(TFb[:, :, f], TFb[:, :, f], rstd[:], A.mult)

        # ---------------- P2c: dist W2 + staging ----------------
        dist_stage = {"tile": None, "fill": 0, "base": 0}
        vd = out_d[:, 0:600, :].rearrange("b (q r) d -> b q r d", q=8)

        def stage_dist(kd, pair, width):
            if dist_stage["tile"] is None:
                dist_stage["tile"] = dstg.tile([128, 6 * D], bf16, tag="dstage",
                                               name="dstage")
                dist_stage["fill"] = 0
                dist_stage["base"] = kd
            fill = dist_stage["fill"]
            copy_rr[fill % 2](dist_stage["tile"][:, fill * D:fill * D + width],
                              pair[:, 0:width])
            dist_stage["fill"] = fill + width // D
            if dist_stage["fill"] == 6 or kd + width // D - 1 == JD - 1:
                b0 = dist_stage["base"]
                gsz = dist_stage["fill"]
                nc.sync.dma_start(vd[:, :, b0:b0 + gsz, :],
                                  dist_stage["tile"][:, 0:gsz * D])
                dist_stage["tile"] = None

        for dc in range(NDC):
            for bp in range(2):
                kd = 4 * dc + 2 * bp
                if kd >= JD:
                    continue
                pair = opa.tile([128, 2 * D], f32, tag="oa", name="oa")
                nc.tensor.matmul(pair[:, 0:D], cta_d[:, dc * 128:(dc + 1) * 128],
                                 w2xb[0][2 * bp], start=True, stop=True)
                nwide = 2 * D if kd + 1 < JD else D
                if nwide == 2 * D:
                    nc.tensor.matmul(pair[:, D:2 * D],
                                     cta_d[:, dc * 128:(dc + 1) * 128],
                                     w2xb[0][2 * bp + 1], start=True, stop=True)
                stage_dist(kd, pair, nwide)

        # ---------------- P4/P5: box pipeline ----------------
        vb = out_d[:, 600:1800, :].rearrange("b (q r) d -> b q r d", q=8)
        oct_tiles = {}     # o -> sbuf [128, 256] bf16 (chunks 2o | 2o+1)
        ht_tiles = {}      # o -> list of 4 sbuf [128, 512] bf16 (per band)
        box_stage = {"tile": None, "fill": 0, "base": 0}
        ccnt = {"i": 0}

        def emit_pass2_half(o, ci):
            if ci == 0:
                oct_tiles[o] = octp.tile([128, 256], bf16, tag="oct", name="oct")
            t = oct_tiles[o]
            c = 2 * o + ci
            pst = zp.tile([128, 1024], bf16, tag="z", name="z")
            ps = pst[:, 0:128]
            nc.tensor.transpose(ps[:], TFb[:, 4 * c:4 * c + 4, :], idb)
            copy_rr[c % 2](t[:, ci * 128:(ci + 1) * 128], ps[:])

        def emit_z(o):
            rhs = oct_tiles[o]
            hts = []
            for b in range(4):
                zb = zp.tile([128, 512], f32, tag="z", name="z")
                nc.tensor.matmul(zb[:, 0:256], w1b[b][0], rhs[:],
                                 start=True, stop=True)
                nc.tensor.matmul(zb[:, 256:512], w1b[b][1], rhs[:],
                                 start=True, stop=True)
                ht = htp.tile([128, 512], bf16, tag="ht", name="ht")
                nc.scalar.activation(ht[:], zb[:], AF.Gelu)
                hts.append(ht)
            ht_tiles[o] = hts

        def flush_box(last_k):
            b0 = box_stage["base"]
            gsz = box_stage["fill"]
            nc.sync.dma_start(vb[:, :, b0:b0 + gsz, :],
                              box_stage["tile"][:, 0:gsz * D])
            box_stage["tile"] = None

        def emit_w2(o):
            hts = ht_tiles.pop(o)
            cchunk = oct_tiles[o]
            for ci in range(2):
                c = 2 * o + ci
                pair = None
                for b in range(4):
                    s = 4 * c + b
                    if s >= JB:
                        continue
                    ht = hts[b]
                    cam = (s % 6) // 2
                    if b % 2 == 0:
                        pair = opa.tile([128, 2 * D], f32, tag="oa", name="oa")
                    ot = pair[:, (b % 2) * D:(b % 2 + 1) * D]
                    nc.tensor.matmul(ot, ht[:, ci * 128:ci * 128 + 128],
                                     w2hi, start=True, stop=False)
                    nc.tensor.matmul(ot, ht[:, 256 + ci * 128:256 + ci * 128 + 128],
                                     w2lo, start=False, stop=False)
                    nc.tensor.matmul(ot, cchunk[:, ci * 128:(ci + 1) * 128],
                                     w2xb[cam][b], start=False, stop=True)
                    if box_stage["tile"] is None:
                        box_stage["tile"] = bstg.tile([128, 8 * D], bf16,
                                                      tag="bstage", name="bstage")
                        box_stage["fill"] = 0
                        box_stage["base"] = s - s % 8
                    if b % 2 == 1:
                        off = (s % 8 - 1) * D
                        copy_rr[ccnt["i"] % 2](
                            box_stage["tile"][:, off:off + 2 * D],
                            pair[:, 0:2 * D])
                        ccnt["i"] += 1
                        box_stage["fill"] += 2
                    if box_stage["fill"] == 8 or s == JB - 1:
                        flush_box(s)
            oct_tiles.pop(o)

        for step in range(NOCT + 2):
            if step < NOCT:
                emit_pass2_half(step, 0)
            if 1 <= step <= NOCT:
                emit_z(step - 1)
            if step < NOCT:
                emit_pass2_half(step, 1)
            if step >= 2:
                emit_w2(step - 2)

    nc.compile()
    return nc


def _prep_inputs(inputs):
    f32 = np.float32
    bf = ml_dtypes.bfloat16
    scale = float(np.asarray(inputs["scale"]))

    W1p = np.zeros((32, DH), f32)
    W1p[0:10] = np.asarray(inputs["geom_w1"], f32)
    W1p[6] *= 0.5
    W1p[7] *= 0.5
    W1p -= W1p.mean(axis=1, keepdims=True)

    w1b_cols = []
    for b in range(4):
        hi = np.zeros((128, 128), f32)
        hi[32 * b:32 * b + 32] = W1p[:, :128]
        lo = np.zeros((128, 128), f32)
        lo[32 * b:32 * b + 32] = W1p[:, 128:]
        w1b_cols += [hi, lo]

    W2s = scale * np.asarray(inputs["geom_w2"], f32)
    w2hi, w2lo = W2s[:128], W2s[128:]

    cat_t = np.asarray(inputs["cat_table"], f32)
    cam_t = np.asarray(inputs["cam_table"], f32)
    bias_row = (np.asarray(inputs["geom_b2"], f32)
                + np.asarray(inputs["conf_b"], f32)
                + np.asarray(inputs["center_b"], f32))
    w2xb_cols = []
    for c in range(3):
        W2X = np.zeros((32, D), f32)
        W2X[10:13] = scale * cat_t
        W2X[13] = scale * np.asarray(inputs["conf_w"], f32)[0]
        W2X[14] = scale * (bias_row + cam_t[c])
        W2X[15] = np.asarray(inputs["missing_emb"], f32)[0]
        W2X[16] = np.asarray(inputs["dist_w"], f32)[0]
        W2X[17] = np.asarray(inputs["dist_b"], f32)
        W2X[18] = scale * np.asarray(inputs["center_w"], f32)[0] * 0.5
        W2X[19] = scale * np.asarray(inputs["center_w"], f32)[1] * 0.5
        for b in range(4):
            t = np.zeros((128, D), f32)
            t[32 * b:32 * b + 32] = W2X
            w2xb_cols.append(t)

    G10 = (W1p[0:10] @ W1p[0:10].T).astype(f32)
    Gblk = np.zeros((128, 128), f32)
    for s in range(12):
        Gblk[10 * s:10 * s + 10, 10 * s:10 * s + 10] = G10

    idf32 = np.eye(128, dtype=f32)
    bpk = np.concatenate(w1b_cols + [w2hi, w2lo] + w2xb_cols + [Gblk, idf32],
                         axis=1).astype(bf)

    box = np.asarray(inputs["box_data"], f32)
    fpks = []
    for c in range(NCORES):
        rawc = box[c * BPC:(c + 1) * BPC].reshape(BPC, T * 6, 6)
        rawc = rawc.reshape(BPC, 8, JB, 6).reshape(128, 900)
        fpks.append(np.ascontiguousarray(
            np.concatenate([rawc, idf32], axis=1), dtype=f32))
    return fpks, bpk


def _fast_path_ok(inputs):
    try:
        shapes = {
            "box_data": (B, T, 6, 6), "cat_table": (3, D), "geom_w1": (10, DH),
            "geom_b1": (DH,), "ln_g": (DH,), "ln_b": (DH,), "geom_w2": (DH, D),
            "geom_b2": (D,), "conf_w": (1, D), "conf_b": (D,),
            "center_w": (2, D), "center_b": (D,), "missing_emb": (1, D),
            "dist_w": (1, D), "dist_b": (D,), "cam_table": (NCAM, D),
        }
        for k, s in shapes.items():
            if tuple(np.asarray(inputs[k]).shape) != s:
                return False
        if not np.all(np.asarray(inputs["geom_b1"]) == 0):
            return False
        if not np.all(np.asarray(inputs["ln_g"]) == 1):
            return False
        if not np.all(np.asarray(inputs["ln_b"]) == 0):
            return False
        return True
    except Exception:
        return False


def _numpy_fallback(inputs):
    import math
    f32 = np.float32
    inp = {k: np.asarray(v) for k, v in inputs.items()}
    coords = inp["box_data"][..., :4].astype(f32)
    category = inp["box_data"][..., 4].astype(np.int32)
    conf = inp["box_data"][..., 5].astype(f32)
    norm = np.array([IW, IH, IW, IH], f32)
    cn = (coords / norm).reshape(B, T, NCAM, NB, 4)
    category = category.reshape(B, T, NCAM, NB)
    conf = conf.reshape(B, T, NCAM, NB, 1)
    presence = (cn.sum(-1) != 0).astype(f32)
    sort_key = category.astype(f32) + (1.0 - presence) * 1000.0
    idx = np.argsort(sort_key, axis=-1, kind="stable")
    cn = np.take_along_axis(cn, idx[..., None], axis=-2)
    category = np.take_along_axis(category, idx, axis=-1)
    conf = np.take_along_axis(conf, idx[..., None], axis=-2)
    presence = (cn.sum(-1) != 0).astype(f32)[..., None]
    x1, y1, x2, y2 = cn[..., 0], cn[..., 1], cn[..., 2], cn[..., 3]
    w, h = x2 - x1, y2 - y1
    cx, cy = (x1 + x2) * 0.5, (y1 + y2) * 0.5
    area, aspect = w * h, w / (h + 1e-6)
    dx, dy = cx[..., 0] - cx[..., 1], cy[..., 0] - cy[..., 1]
    dist = np.sqrt(dx * dx + dy * dy)[..., None]
    dist_tok = dist @ inp["dist_w"].astype(f32) + inp["dist_b"].astype(f32)
    geom = np.stack([x1, y1, x2, y2, w, h, cx, cy, area, aspect], axis=-1)
    z = geom @ inp["geom_w1"].astype(f32) + inp["geom_b1"].astype(f32)
    mu = z.mean(-1, keepdims=True)
    var = ((z - mu) ** 2).mean(-1, keepdims=True)
    xh = (z - mu) / np.sqrt(var + 1e-5) * inp["ln_g"].astype(f32) + inp["ln_b"].astype(f32)
    try:
        from scipy.special import erf as _erf
        g = xh * 0.5 * (1.0 + _erf(xh / np.sqrt(2.0)))
    except Exception:
        verf = np.vectorize(math.erf)
        g = xh * 0.5 * (1.0 + verf(xh / np.sqrt(2.0)))
    geom_p = g @ inp["geom_w2"].astype(f32) + inp["geom_b2"].astype(f32)
    cat_emb = inp["cat_table"].astype(f32)[category]
    conf_p = conf @ inp["conf_w"].astype(f32) + inp["conf_b"].astype(f32)
    center_p = np.stack([cx, cy], axis=-1) @ inp["center_w"].astype(f32) + inp["center_b"].astype(f32)
    cam_emb = inp["cam_table"].astype(f32).reshape(1, 1, NCAM, 1, D)
    tok = (geom_p + cat_emb + conf_p + center_p + cam_emb) * float(inp["scale"])
    tok = np.where(presence == 0, inp["missing_emb"].astype(f32)[0], tok)
    out = np.concatenate([dist_tok.reshape(B, T * NCAM, D),
                          tok.reshape(B, T * NCAM * NB, D)], axis=1)
    return out.astype(np.float32)


def _run(inputs, trace=False, tmpdir=None):
    from concourse.bass_utils import run_bass_kernel_spmd

    if "nc" not in _CACHE:
        _CACHE["nc"] = _build_nc()
    nc = _CACHE["nc"]

    fpks, bpk = _prep_inputs(inputs)
    in_maps = [{"fpk": fpks[c], "bpk": bpk} for c in range(NCORES)]
    res = run_bass_kernel_spmd(nc, in_maps, list(range(NCORES)),
                               trace=trace, tmpdir=tmpdir)
    out = np.concatenate([np.asarray(res.results[c]["out"])
                          for c in range(NCORES)], axis=0)
    return out.astype(np.float32), res


def kernel(**inputs):
    if not _fast_path_ok(inputs):
        return _numpy_fallback(inputs)
    out, _ = _run(inputs)
    return out


if __name__ == "__main__":
    import reference as ref
    inputs = {k: np.asarray(v) for k, v in ref.setup_inputs().items()}
    got = kernel(**inputs)
    exp = np.load("/tmp/expected.npy")
    d = got - exp
    print("rel fro:", np.linalg.norm(d) / np.linalg.norm(exp))
    print("absmax rel:", np.abs(d).max() / np.abs(exp).max())


# revision 11
# speedup vs baseline: 1.1109x; 1.1109x over previous
"""Trainium2 Bass kernel v3 for nn_BoxEncoder (B=128, T=200, NC=3, NB=2, D=512, DH=256).

Data-parallel over batch: 16 batch items per core x 8 cores; partition
p = bt*8 + q.  Per partition: 75 dist tokens (+1 pad) and 150 box tokens
(+2 pad), processed as 4-slot transpose chunks.

v3 vs v2 (~2x): the whole kernel is built to keep the PE p-state warm
(cost model: matmuls run 2x faster once the PE has been ~continuously
busy for 3us; any long stall drops it back):
 - z is computed TRANSPOSED (weights-stationary): zT = W1bandedT @ cta_s,
   so gelu(zT) directly yields hT = the lhsT of the W2 matmuls.  The
   dma_start_transpose of h (SP-queue serial 1.24us each + 900ns sem) is
   gone - that chain caused the recurring 3-5us PE stalls in v2.
 - LN rstd is folded into the geometry features BEFORE the z matmul
   (f0..f9 *= rstd per token), so gelu needs no per-partition scale and
   batches [128,512] over 4 slots per call.  cx,cy are duplicated into
   f18,f19 (unscaled) for the center_w rows of the extras matmul.
 - all matmuls are full-K (no tile_position): banded *weights* (zero rows
   outside the slot's 32-band) instead of banded matmuls; the extras and
   z matmuls share one LDWEIGHTS of the feature chunk.
 - variance via a 12-slot x 10-feature gram pack (13 transposes instead
   of 38) and a single batched sqrt+reciprocal.
 - PE pre-warm: a dozen junk matmuls issued at t~1us keep the PE busy
   during the DVE feature-prep phase so the clock is warm when real
   matmuls start.
 - staging copies round-robin DVE/ACT/GPSIMD; out DMAs on the otherwise
   idle SP queue; one ACT table preload for Gelu right after the rstd
   sqrt so no table load lands inside the steady-state loop.
"""

import numpy as np
import ml_dtypes

B, T, NCAM, NB, D, DH = 128, 200, 3, 2, 512, 256
IW, IH = 640.0, 400.0
NCORES = 8
BPC = B // NCORES
JB = 150                  # real box slots per partition
JBP = 156                 # padded (38 chunks use 152; gram packs use 156)
JD = 75                   # real dist slots
JDP = 76                  # padded (19 chunks)
NCH = 38                  # box chunks
NDC = 19                  # dist chunks
NOCT = 19                 # box octs (2 chunks = 8 slots each; last has 6)
NGP = 13                  # gram packs (12 slots x 10 feats)

_CACHE = {}


def _build_nc():
    from contextlib import ExitStack
    import concourse.bacc as bacc
    import concourse.mybir as mybir
    import concourse.tile as tile

    f32 = mybir.dt.float32
    bf16 = mybir.dt.bfloat16
    A = mybir.AluOpType
    AF = mybir.ActivationFunctionType

    # bpk bf16 column offsets
    C_W1B = 0                       # 8 x 128 (band b: hi, lo)
    C_W2HI = C_W1B + 8 * 128
    C_W2LO = C_W2HI + 512
    C_W2XB = C_W2LO + 512           # 12 x 512 (cam c, band b)
    C_G = C_W2XB + 12 * 512
    C_ID = C_G + 128
    NBF = C_ID + 128
    C_P1 = C_W2XB            # part 1 = w1b, w2hi, w2lo (cols 0..C_W2XB)


    nc = bacc.Bacc("TRN2", target_bir_lowering=False, debug=False,
                   num_devices=NCORES)
    fpk = nc.declare_dram_parameter("fpk", [128, 900 + 128], f32, isOutput=False)
    bpk = nc.declare_dram_parameter("bpk", [128, NBF], bf16, isOutput=False)
    out_d = nc.declare_dram_parameter("out", [BPC, 1800, D], bf16, isOutput=True)

    with ExitStack() as ctx:
        tc = ctx.enter_context(tile.TileContext(nc))
        cp = ctx.enter_context(tc.tile_pool(name="const", bufs=1))
        sc = ctx.enter_context(tc.tile_pool(name="scratch", bufs=1))
        # PSUM pools (8 banks): zp 4x[128,512]=4 + opa 3x[128,512]=3 +
        # tp 2x[128,128]=0.5
        zp = ctx.enter_context(tc.tile_pool(name="zp", bufs=4, space="PSUM"))
        opa = ctx.enter_context(tc.tile_pool(name="opa", bufs=4, space="PSUM"))
        gpck = ctx.enter_context(tc.tile_pool(name="gpck", bufs=2))
        octp = ctx.enter_context(tc.tile_pool(name="octp", bufs=4))
        cdp = ctx.enter_context(tc.tile_pool(name="cdp", bufs=1))
        htp = ctx.enter_context(tc.tile_pool(name="htp", bufs=8))
        bstg = ctx.enter_context(tc.tile_pool(name="bstage", bufs=3))
        dstg = ctx.enter_context(tc.tile_pool(name="dstage", bufs=3))

        fpack = cp.tile([128, 900 + 128], f32)
        nc.sync.dma_start(fpack[:], fpk[:])
        bpack = cp.tile([128, NBF], bf16)
        # split the weight load: small part (warm-up, gram, z) first so the
        # big w2xb block (12KB/partition) doesn't gate the early phases
        nc.sync.dma_start(bpack[:, C_G:NBF], bpk[:, C_G:NBF])
        nc.sync.dma_start(bpack[:, 0:C_P1], bpk[:, 0:C_P1])
        nc.sync.dma_start(bpack[:, C_P1:C_G], bpk[:, C_P1:C_G])

        raw = fpack[:, 0:900]
        idf = fpack[:, 900:1028]
        w1b = [(bpack[:, C_W1B + (2 * b) * 128: C_W1B + (2 * b + 1) * 128],
                bpack[:, C_W1B + (2 * b + 1) * 128: C_W1B + (2 * b + 2) * 128])
               for b in range(4)]
        w2hi = bpack[:, C_W2HI:C_W2HI + 512]
        w2lo = bpack[:, C_W2LO:C_W2LO + 512]
        w2xb = [[bpack[:, C_W2XB + (c * 4 + b) * 512: C_W2XB + (c * 4 + b + 1) * 512]
                 for b in range(4)] for c in range(3)]
        Gblk = bpack[:, C_G:C_G + 128]
        idb = bpack[:, C_ID:C_ID + 128]

        # ---------------- PE pre-warm: junk matmuls on memset tiles -----
        # (independent of the input DMAs so the PE busies from ~0.5us)
        junkw = cp.tile([128, 512], bf16)
        nc.vector.memset(junkw[:], 0.25)
        for _ in range(70):
            wps = opa.tile([128, D], f32, tag="oa", name="oa")
            nc.tensor.matmul(wps[:], junkw[:, 0:128], junkw[:],
                             start=True, stop=True)

        # ---------------- P1: feature planes ----------------
        TFB = cp.tile([128, JBP * 32], bf16)
        TFD = cp.tile([128, JDP * 32], bf16)
        TFb = TFB.rearrange("p (j f) -> p j f", f=32)
        TFd = TFD.rearrange("p (j f) -> p j f", f=32)
        # zeros: box f16,17 + f20..31 + pad slots; dist all but f16,f17
        # zeros via ACT (uint32-bitcast x0 is NaN-safe); tiny pads + the 1.0
        # fill on DVE.  Pad slots' f0..9 must precede the gram prepack.
        nc.vector.memset(TFb[:, JB:JBP, 0:16], 0.0)
        nc.vector.memset(TFb[:, JB:JBP, 18:20], 0.0)
        nc.scalar.memzero(TFb[:, :, 16:18])
        nc.scalar.memzero(TFb[:, :, 20:32])
        nc.scalar.memzero(TFd[:, :, 0:16])
        nc.scalar.memzero(TFd[:, :, 18:32])
        nc.vector.memset(TFd[:, :, 17], 1.0)
        nc.vector.memset(TFd[:, JD:JDP, 16], 0.0)

        TFr = TFb[:, 0:JB, :]
        raw6 = raw.rearrange("p (b s) -> p b s", s=6)
        rawp = raw.rearrange("p (m g s) -> p m g s", g=2, s=6)
        TFbp = TFB[:, 0:JB * 32].rearrange("p (m g f) -> p m g f", g=2, f=32)

        sPres = sc.tile([128, JB], f32)
        sKey = sc.tile([128, JB], f32)
        sSwap = sc.tile([128, JD], f32)
        sD = sc.tile([128, JD], f32)
        sSD = sc.tile([128, JD], f32)
        sT0 = sc.tile([128, JB], f32)
        sT1 = sc.tile([128, JB], f32)

        nc.vector.tensor_tensor(sT0[:], raw6[:, :, 0], raw6[:, :, 1], A.add)
        nc.vector.tensor_tensor(sT1[:], raw6[:, :, 2], raw6[:, :, 3], A.add)
        nc.vector.tensor_tensor(sT0[:], sT0[:], sT1[:], A.add)
        nc.vector.tensor_scalar(sPres[:], sT0[:], 0.0, None, A.not_equal)
        nc.vector.scalar_tensor_tensor(sKey[:], sPres[:], -1000.0,
                                       raw6[:, :, 4], A.mult, A.add)
        sKeyp = sKey.rearrange("p (m g) -> p m g", g=2)
        nc.vector.tensor_tensor(sSwap[:], sKeyp[:, :, 1], sKeyp[:, :, 0], A.is_lt)

        # block compare-and-swap: all 6 raw components in 4 DVE ops
        sRaw = sc.tile([128, JD, 2, 6], f32)
        sD6 = sc.tile([128, JD, 6], f32)
        swb = sSwap[:].unsqueeze(-1).broadcast_to([128, JD, 6])
        nc.vector.tensor_tensor(sD6[:], rawp[:, :, 1, :], rawp[:, :, 0, :],
                                A.subtract)
        nc.vector.tensor_tensor(sD6[:], sD6[:], swb, A.mult)
        nc.vector.tensor_tensor(sRaw[:, :, 0, :], rawp[:, :, 0, :], sD6[:], A.add)
        nc.vector.tensor_tensor(sRaw[:, :, 1, :], rawp[:, :, 1, :], sD6[:],
                                A.subtract)
        sPresP = sPres.rearrange("p (m g) -> p m g", g=2)
        nc.vector.tensor_tensor(sD[:], sPresP[:, :, 1], sPresP[:, :, 0], A.subtract)
        nc.vector.tensor_tensor(sSD[:], sD[:], sSwap[:], A.mult)
        nc.vector.tensor_tensor(TFbp[:, :, 0, 14], sPresP[:, :, 0], sSD[:], A.add)
        nc.vector.tensor_tensor(TFbp[:, :, 1, 14], sPresP[:, :, 1], sSD[:], A.subtract)

        sRw = sRaw.rearrange("p m g s -> p (m g) s")
        sX1, sY1, sX2, sY2 = (sRw[:, :, i] for i in range(4))
        sCat, sConf = sRw[:, :, 4], sRw[:, :, 5]
        # all derived geometry in f32 scratch (bf16-rounded coords would
        # catastrophically cancel in w/h near zero -> aspect blows up)
        sW32 = sc.tile([128, JB], f32)
        sH32 = sc.tile([128, JB], f32)
        sCx = sc.tile([128, JB], f32)
        sCy = sc.tile([128, JB], f32)
        nc.scalar.mul(TFr[:, :, 0], sX1, 1.0 / IW)
        nc.scalar.mul(TFr[:, :, 1], sY1, 1.0 / IH)
        nc.scalar.mul(TFr[:, :, 2], sX2, 1.0 / IW)
        nc.scalar.mul(TFr[:, :, 3], sY2, 1.0 / IH)
        nc.vector.tensor_tensor(sW32[:], sX2, sX1, A.subtract)
        nc.vector.tensor_tensor(sH32[:], sY2, sY1, A.subtract)
        nc.vector.tensor_tensor(sCx[:], sX1, sX2, A.add)
        nc.vector.tensor_tensor(sCy[:], sY1, sY2, A.add)
        nc.scalar.mul(TFr[:, :, 4], sW32[:], 1.0 / IW)
        nc.scalar.mul(TFr[:, :, 5], sH32[:], 1.0 / IH)
        nc.scalar.mul(TFr[:, :, 6], sCx[:], 1.0 / IW)
        nc.scalar.mul(TFr[:, :, 7], sCy[:], 1.0 / IH)
        nc.scalar.mul(TFr[:, :, 18], sCx[:], 1.0 / IW)
        nc.scalar.mul(TFr[:, :, 19], sCy[:], 1.0 / IH)
        sT2 = sc.tile([128, JB], f32)
        nc.vector.tensor_tensor(sT2[:], sW32[:], sH32[:], A.mult)
        nc.scalar.mul(TFr[:, :, 8], sT2[:], 1.0 / (IW * IH))
        sHp = sT0
        nc.vector.tensor_scalar(sHp[:], sH32[:], 1e-6 * IH, 1.0 / IH, A.add, A.mult)
        sR = sT1
        nc.vector.reciprocal(sR[:], sHp[:])
        nc.vector.tensor_scalar(sT2[:], sW32[:], 1.0 / IW, None, A.mult)
        nc.vector.tensor_tensor(TFr[:, :, 9], sT2[:], sR[:], A.mult)
        for k in range(3):
            nc.vector.scalar_tensor_tensor(TFr[:, :, 10 + k], sCat, float(k),
                                           TFr[:, :, 14], A.is_equal, A.mult)
        nc.vector.tensor_tensor(TFr[:, :, 13], sConf, TFr[:, :, 14], A.mult)
        nc.scalar.activation(TFr[:, :, 15], TFr[:, :, 14],
                             AF.Identity, bias=1.0, scale=-1.0)
        # dist features: f16 = 0.5*sqrt(dx2^2+dy2^2)/IW-scaled, f17 = 1
        sDx = sc.tile([128, JD], f32)
        sDy = sc.tile([128, JD], f32)
        sCxp = sCx.rearrange("p (m g) -> p m g", g=2)
        sCyp = sCy.rearrange("p (m g) -> p m g", g=2)
        nc.vector.tensor_tensor(sDx[:], sCxp[:, :, 0], sCxp[:, :, 1], A.subtract)
        nc.vector.tensor_tensor(sDy[:], sCyp[:, :, 0], sCyp[:, :, 1], A.subtract)
        nc.vector.tensor_scalar(sDx[:], sDx[:], 1.0 / IW, None, A.mult)
        nc.vector.tensor_scalar(sDy[:], sDy[:], 1.0 / IH, None, A.mult)
        nc.vector.tensor_tensor(sDx[:], sDx[:], sDx[:], A.mult)
        nc.vector.tensor_tensor(sDy[:], sDy[:], sDy[:], A.mult)
        nc.vector.tensor_tensor(sDx[:], sDx[:], sDy[:], A.add)
        nc.scalar.activation(TFd[:, 0:JD, 16], sDx[:], AF.Sqrt, scale=0.25)

        # ---------------- P2: gram variance ----------------
        v = sc.tile([128, 160], f32)
        copy_rr = [nc.vector.tensor_copy, nc.scalar.copy]

        # prepack geom features f0..9 of all 156 slots contiguously; each
        # 128-col transpose window overlaps 8 cols into the next pack, which
        # land on zero rows of Gblk (harmless).
        gprep = sc.tile([128, 13 * 120 + 8], bf16)
        nc.vector.memset(gprep[:, 13 * 120:], 0.0)
        nc.vector.tensor_copy(
            gprep[:, 0:1560].rearrange("p (j f) -> p j f", f=10),
            TFb[:, 0:156, 0:10])

        for gi in range(NGP):
            s0 = 12 * gi
            src = TFb[:, s0:s0 + 12, 0:10]
            pst = zp.tile([128, 1024], bf16, tag="z", name="z")
            ps = pst[:, 0:128]
            nc.tensor.transpose(ps[:], gprep[:, 120 * gi:120 * gi + 128], idb)
            pk = gpck.tile([128, 128], bf16, tag="gp", name="gp")
            nc.scalar.copy(pk[:], ps[:])
            yt = zp.tile([128, 512], f32, tag="z", name="z")
            y = yt[:, 0:128]
            nc.tensor.matmul(y, pk[:], Gblk, start=True, stop=True)
            tmp = sc.tile([128, 120], f32, tag="gtmp", name="gtmp")
            nc.vector.tensor_tensor(tmp[:], src, y[:, 0:120], A.mult)
            nc.vector.tensor_reduce(v[:, s0:s0 + 12],
                                    tmp.rearrange("p (j f) -> p j f", f=10),
                                    mybir.AxisListType.X, A.add)

        # ---------------- P2b: dist transposes ----------------
        cta_d = cp.tile([128, NDC * 128], bf16)
        for dc in range(NDC):
            pst = zp.tile([128, 1024], bf16, tag="z", name="z")
            ps = pst[:, 0:128]
            nc.tensor.transpose(ps[:], TFd[:, 4 * dc:4 * dc + 4, :], idb)
            copy_rr[dc % 2](cta_d[:, dc * 128:(dc + 1) * 128], ps[:])

        # ---------------- P3: rstd + feature scale + gelu preload --------
        eps = sc.tile([128, 1], f32)
        nc.vector.memset(eps[:], 1e-5)
        sd = sc.tile([128, 156], f32)
        rstd = sc.tile([128, 156], f32)
        nc.scalar.activation(sd[:], v[:, 0:156], AF.Sqrt,
                             bias=eps[:], scale=1.0 / DH)
        nc.vector.reciprocal(rstd[:], sd[:])
        # preload the Gelu ACT table off the critical path
        gjunk = sc.tile([128, 8], bf16)
        nc.scalar.activation(gjunk[:], sd[:, 0:8], AF.Gelu)
        # scale geometry features f0..9 by rstd (per token)
        for f in range(10):
            nc.vector.tensor_tensor(TFb[:, :, f], TFb[:, :, f], rstd[:], A.mult)

        # ---------------- P2c: dist W2 + staging ----------------
        dist_stage = {"tile": None, "fill": 0, "base": 0}
        vd = out_d[:, 0:600, :].rearrange("b (q r) d -> b q r d", q=8)

        def stage_dist(kd, o):
            if dist_stage["tile"] is None:
                dist_stage["tile"] = dstg.tile([128, 6 * D], bf16, tag="dstage",
                                               name="dstage")
                dist_stage["fill"] = 0
                dist_stage["base"] = kd
            fill = dist_stage["fill"]
            copy_rr[kd % 2](dist_stage["tile"][:, fill * D:(fill + 1) * D], o[:])
            dist_stage["fill"] = fill + 1
            if dist_stage["fill"] == 6 or kd == JD - 1:
                b0 = dist_stage["base"]
                gsz = dist_stage["fill"]
                nc.sync.dma_start(vd[:, :, b0:b0 + gsz, :],
                                  dist_stage["tile"][:, 0:gsz * D])
                dist_stage["tile"] = None

        for dc in range(NDC):
            for b in range(4):
                kd = 4 * dc + b
                if kd >= JD:
                    continue
                o = opa.tile([128, D], f32, tag="oa", name="oa")
                nc.tensor.matmul(o[:], cta_d[:, dc * 128:(dc + 1) * 128],
                                 w2xb[0][b], start=True, stop=True)
                stage_dist(kd, o)

        # mid fillers: keep the PE busy across the rstd chain window
        for _ in range(18):
            wps = opa.tile([128, D], f32, tag="oa", name="oa")
            nc.tensor.matmul(wps[:], junkw[:, 0:128], junkw[:],
                             start=True, stop=True)

        # ---------------- P4/P5: box pipeline ----------------
        vb = out_d[:, 600:1800, :].rearrange("b (q r) d -> b q r d", q=8)
        oct_tiles = {}     # o -> sbuf [128, 256] bf16 (chunks 2o | 2o+1)
        ht_tiles = {}      # o -> list of 4 sbuf [128, 512] bf16 (per band)
        box_stage = {"tile": None, "fill": 0, "base": 0}
        ccnt = {"i": 0}

        def emit_pass2_half(o, ci):
            if ci == 0:
                oct_tiles[o] = octp.tile([128, 256], bf16, tag="oct", name="oct")
            t = oct_tiles[o]
            c = 2 * o + ci
            pst = zp.tile([128, 1024], bf16, tag="z", name="z")
            ps = pst[:, 0:128]
            nc.tensor.transpose(ps[:], TFb[:, 4 * c:4 * c + 4, :], idb)
            copy_rr[c % 2](t[:, ci * 128:(ci + 1) * 128], ps[:])

        z_banks = {}

        def emit_z_mm(o):
            rhs = oct_tiles[o]
            zbs = []
            for b in range(4):
                zb = zp.tile([128, 512], f32, tag="z", name="z")
                nc.tensor.matmul(zb[:, 0:256], w1b[b][0], rhs[:],
                                 start=True, stop=True)
                nc.tensor.matmul(zb[:, 256:512], w1b[b][1], rhs[:],
                                 start=True, stop=True)
                zbs.append(zb)
            z_banks[o] = zbs

        def emit_gelu(o):
            zbs = z_banks.pop(o)
            hts = []
            for b in range(4):
                ht = htp.tile([128, 512], bf16, tag="ht", name="ht")
                nc.scalar.activation(ht[:], zbs[b][:], AF.Gelu)
                hts.append(ht)
            ht_tiles[o] = hts

        def flush_box(last_k):
            b0 = box_stage["base"]
            gsz = box_stage["fill"]
            nc.sync.dma_start(vb[:, :, b0:b0 + gsz, :],
                              box_stage["tile"][:, 0:gsz * D])
            box_stage["tile"] = None

        def emit_w2(o):
            hts = ht_tiles.pop(o)
            cchunk = oct_tiles[o]
            for ci in range(2):
                c = 2 * o + ci
                for b in range(4):
                    s = 4 * c + b
                    if s >= JB:
                        continue
                    ht = hts[b]
                    cam = (s % 6) // 2
                    ot = opa.tile([128, D], f32, tag="oa", name="oa")
                    nc.tensor.matmul(ot[:], ht[:, ci * 128:ci * 128 + 128],
                                     w2hi, start=True, stop=False)
                    nc.tensor.matmul(ot[:], ht[:, 256 + ci * 128:256 + ci * 128 + 128],
                                     w2lo, start=False, stop=False)
                    nc.tensor.matmul(ot[:], cchunk[:, ci * 128:(ci + 1) * 128],
                                     w2xb[cam][b], start=False, stop=True)
                    if box_stage["tile"] is None:
                        box_stage["tile"] = bstg.tile([128, 8 * D], bf16,
                                                      tag="bstage", name="bstage")
                        box_stage["fill"] = 0
                        box_stage["base"] = s - s % 8
                    csel = 0 if (ccnt["i"] % 8) in (0, 2, 3, 5, 6) else 1
                    copy_rr[csel](
                        box_stage["tile"][:, (s % 8) * D:(s % 8 + 1) * D], ot[:])
                    ccnt["i"] += 1
                    box_stage["fill"] += 1
                    if box_stage["fill"] == 8 or s == JB - 1:
                        flush_box(s)
            oct_tiles.pop(o)

        for step in range(NOCT + 2):
            if step < NOCT:
                emit_pass2_half(step, 0)
            if 1 <= step <= NOCT:
                emit_z_mm(step - 1)
            if step < NOCT:
                emit_pass2_half(step, 1)
            if step >= 2:
                emit_w2(step - 2)
            if 1 <= step <= NOCT:
                emit_gelu(step - 1)

    nc.compile()
    return nc


def _prep_inputs(inputs):
    f32 = np.float32
    bf = ml_dtypes.bfloat16
    scale = float(np.asarray(inputs["scale"]))

    W1p = np.zeros((32, DH), f32)
    W1p[0:10] = np.asarray(inputs["geom_w1"], f32)
    W1p[6] *= 0.5
    W1p[7] *= 0.5
    W1p -= W1p.mean(axis=1, keepdims=True)

    w1b_cols = []
    for b in range(4):
        hi = np.zeros((128, 128), f32)
        hi[32 * b:32 * b + 32] = W1p[:, :128]
        lo = np.zeros((128, 128), f32)
        lo[32 * b:32 * b + 32] = W1p[:, 128:]
        w1b_cols += [hi, lo]

    W2s = scale * np.asarray(inputs["geom_w2"], f32)
    w2hi, w2lo = W2s[:128], W2s[128:]

    cat_t = np.asarray(inputs["cat_table"], f32)
    cam_t = np.asarray(inputs["cam_table"], f32)
    bias_row = (np.asarray(inputs["geom_b2"], f32)
                + np.asarray(inputs["conf_b"], f32)
                + np.asarray(inputs["center_b"], f32))
    w2xb_cols = []
    for c in range(3):
        W2X = np.zeros((32, D), f32)
        W2X[10:13] = scale * cat_t
        W2X[13] = scale * np.asarray(inputs["conf_w"], f32)[0]
        W2X[14] = scale * (bias_row + cam_t[c])
        W2X[15] = np.asarray(inputs["missing_emb"], f32)[0]
        W2X[16] = np.asarray(inputs["dist_w"], f32)[0]
        W2X[17] = np.asarray(inputs["dist_b"], f32)
        W2X[18] = scale * np.asarray(inputs["center_w"], f32)[0] * 0.5
        W2X[19] = scale * np.asarray(inputs["center_w"], f32)[1] * 0.5
        for b in range(4):
            t = np.zeros((128, D), f32)
            t[32 * b:32 * b + 32] = W2X
            w2xb_cols.append(t)

    G10 = (W1p[0:10] @ W1p[0:10].T).astype(f32)
    Gblk = np.zeros((128, 128), f32)
    for s in range(12):
        Gblk[10 * s:10 * s + 10, 10 * s:10 * s + 10] = G10

    idf32 = np.eye(128, dtype=f32)
    bpk = np.concatenate(w1b_cols + [w2hi, w2lo] + w2xb_cols + [Gblk, idf32],
                         axis=1).astype(bf)

    box = np.asarray(inputs["box_data"], f32)
    fpks = []
    for c in range(NCORES):
        rawc = box[c * BPC:(c + 1) * BPC].reshape(BPC, T * 6, 6)
        rawc = rawc.reshape(BPC, 8, JB, 6).reshape(128, 900)
        fpks.append(np.ascontiguousarray(
            np.concatenate([rawc, idf32], axis=1), dtype=f32))
    return fpks, bpk


def _fast_path_ok(inputs):
    try:
        shapes = {
            "box_data": (B, T, 6, 6), "cat_table": (3, D), "geom_w1": (10, DH),
            "geom_b1": (DH,), "ln_g": (DH,), "ln_b": (DH,), "geom_w2": (DH, D),
            "geom_b2": (D,), "conf_w": (1, D), "conf_b": (D,),
            "center_w": (2, D), "center_b": (D,), "missing_emb": (1, D),
            "dist_w": (1, D), "dist_b": (D,), "cam_table": (NCAM, D),
        }
        for k, s in shapes.items():
            if tuple(np.asarray(inputs[k]).shape) != s:
                return False
        if not np.all(np.asarray(inputs["geom_b1"]) == 0):
            return False
        if not np.all(np.asarray(inputs["ln_g"]) == 1):
            return False
        if not np.all(np.asarray(inputs["ln_b"]) == 0):
            return False
        return True
    except Exception:
        return False


def _numpy_fallback(inputs):
    import math
    f32 = np.float32
    inp = {k: np.asarray(v) for k, v in inputs.items()}
    coords = inp["box_data"][..., :4].astype(f32)
    category = inp["box_data"][..., 4].astype(np.int32)
    conf = inp["box_data"][..., 5].astype(f32)
    norm = np.array([IW, IH, IW, IH], f32)
    cn = (coords / norm).reshape(B, T, NCAM, NB, 4)
    category = category.reshape(B, T, NCAM, NB)
    conf = conf.reshape(B, T, NCAM, NB, 1)
    presence = (cn.sum(-1) != 0).astype(f32)
    sort_key = category.astype(f32) + (1.0 - presence) * 1000.0
    idx = np.argsort(sort_key, axis=-1, kind="stable")
    cn = np.take_along_axis(cn, idx[..., None], axis=-2)
    category = np.take_along_axis(category, idx, axis=-1)
    conf = np.take_along_axis(conf, idx[..., None], axis=-2)
    presence = (cn.sum(-1) != 0).astype(f32)[..., None]
    x1, y1, x2, y2 = cn[..., 0], cn[..., 1], cn[..., 2], cn[..., 3]
    w, h = x2 - x1, y2 - y1
    cx, cy = (x1 + x2) * 0.5, (y1 + y2) * 0.5
    area, aspect = w * h, w / (h + 1e-6)
    dx, dy = cx[..., 0] - cx[..., 1], cy[..., 0] - cy[..., 1]
    dist = np.sqrt(dx * dx + dy * dy)[..., None]
    dist_tok = dist @ inp["dist_w"].astype(f32) + inp["dist_b"].astype(f32)
    geom = np.stack([x1, y1, x2, y2, w, h, cx, cy, area, aspect], axis=-1)
    z = geom @ inp["geom_w1"].astype(f32) + inp["geom_b1"].astype(f32)
    mu = z.mean(-1, keepdims=True)
    var = ((z - mu) ** 2).mean(-1, keepdims=True)
    xh = (z - mu) / np.sqrt(var + 1e-5) * inp["ln_g"].astype(f32) + inp["ln_b"].astype(f32)
    try:
        from scipy.special import erf as _erf
        g = xh * 0.5 * (1.0 + _erf(xh / np.sqrt(2.0)))
    except Exception:
        verf = np.vectorize(math.erf)
        g = xh * 0.5 * (1.0 + verf(xh / np.sqrt(2.0)))
    geom_p = g @ inp["geom_w2"].astype(f32) + inp["geom_b2"].astype(f32)
    cat_emb = inp["cat_table"].astype(f32)[category]
    conf_p = conf @ inp["conf_w"].astype(f32) + inp["conf_b"].astype(f32)
    center_p = np.stack([cx, cy], axis=-1) @ inp["center_w"].astype(f32) + inp["center_b"].astype(f32)
    cam_emb = inp["cam_table"].astype(f32).reshape(1, 1, NCAM, 1, D)
    tok = (geom_p + cat_emb + conf_p + center_p + cam_emb) * float(inp["scale"])
    tok = np.where(presence == 0, inp["missing_emb"].astype(f32)[0], tok)
    out = np.concatenate([dist_tok.reshape(B, T * NCAM, D),
                          tok.reshape(B, T * NCAM * NB, D)], axis=1)
    return out.astype(np.float32)


def _run(inputs, trace=False, tmpdir=None):
    from concourse.bass_utils import run_bass_kernel_spmd

    if "nc" not in _CACHE:
        _CACHE["nc"] = _build_nc()
    nc = _CACHE["nc"]

    fpks, bpk = _prep_inputs(inputs)
    in_maps = [{"fpk": fpks[c], "bpk": bpk} for c in range(NCORES)]
    res = run_bass_kernel_spmd(nc, in_maps, list(range(NCORES)),
                               trace=trace, tmpdir=tmpdir)
    out = np.concatenate([np.asarray(res.results[c]["out"])
                          for c in range(NCORES)], axis=0)
    return out.astype(np.float32), res


def kernel(**inputs):
    if not _fast_path_ok(inputs):
        return _numpy_fallback(inputs)
    out, _ = _run(inputs)
    return out


if __name__ == "__main__":
    import reference as ref
    inputs = {k: np.asarray(v) for k, v in ref.setup_inputs().items()}
    got = kernel(**inputs)
    exp = np.load("/tmp/expected.npy")
    d = got - exp
    print("rel fro:", np.linalg.norm(d) / np.linalg.norm(exp))
    print("absmax rel:", np.abs(d).max() / np.abs(exp).max())


# revision 12
# speedup vs baseline: 1.1902x; 1.0714x over previous
"""Trainium2 Bass kernel v3 for nn_BoxEncoder (B=128, T=200, NC=3, NB=2, D=512, DH=256).

Data-parallel over batch: 16 batch items per core x 8 cores; partition
p = bt*8 + q.  Per partition: 75 dist tokens (+1 pad) and 150 box tokens
(+2 pad), processed as 4-slot transpose chunks.

v3 vs v2 (~2x): the whole kernel is built to keep the PE p-state warm
(cost model: matmuls run 2x faster once the PE has been ~continuously
busy for 3us; any long stall drops it back):
 - z is computed TRANSPOSED (weights-stationary): zT = W1bandedT @ cta_s,
   so gelu(zT) directly yields hT = the lhsT of the W2 matmuls.  The
   dma_start_transpose of h (SP-queue serial 1.24us each + 900ns sem) is
   gone - that chain caused the recurring 3-5us PE stalls in v2.
 - LN rstd is folded into the geometry features BEFORE the z matmul
   (f0..f9 *= rstd per token), so gelu needs no per-partition scale and
   batches [128,512] over 4 slots per call.  cx,cy are duplicated into
   f18,f19 (unscaled) for the center_w rows of the extras matmul.
 - all matmuls are full-K (no tile_position): banded *weights* (zero rows
   outside the slot's 32-band) instead of banded matmuls; the extras and
   z matmuls share one LDWEIGHTS of the feature chunk.
 - variance via a 12-slot x 10-feature gram pack (13 transposes instead
   of 38) and a single batched sqrt+reciprocal.
 - PE pre-warm: a dozen junk matmuls issued at t~1us keep the PE busy
   during the DVE feature-prep phase so the clock is warm when real
   matmuls start.
 - staging copies round-robin DVE/ACT/GPSIMD; out DMAs on the otherwise
   idle SP queue; one ACT table preload for Gelu right after the rstd
   sqrt so no table load lands inside the steady-state loop.
"""

import numpy as np
import ml_dtypes

B, T, NCAM, NB, D, DH = 128, 200, 3, 2, 512, 256
IW, IH = 640.0, 400.0
NCORES = 8
BPC = B // NCORES
JB = 150                  # real box slots per partition
JBP = 156                 # padded (38 chunks use 152; gram packs use 156)
JD = 75                   # real dist slots
JDP = 76                  # padded (19 chunks)
NCH = 38                  # box chunks
NDC = 19                  # dist chunks
NOCT = 19                 # box octs (2 chunks = 8 slots each; last has 6)
NGP = 13                  # gram packs (12 slots x 10 feats)

_CACHE = {}


def _build_nc():
    from contextlib import ExitStack
    import concourse.bacc as bacc
    import concourse.mybir as mybir
    import concourse.tile as tile

    f32 = mybir.dt.float32
    bf16 = mybir.dt.bfloat16
    A = mybir.AluOpType
    AF = mybir.ActivationFunctionType

    # bpk bf16 column offsets
    C_W1B = 0                       # 8 x 128 (band b: hi, lo)
    C_W2HI = C_W1B + 8 * 128
    C_W2LO = C_W2HI + 512
    C_W2XB = C_W2LO + 512           # 12 x 512 (cam c, band b)
    C_G = C_W2XB + 12 * 512
    C_ID = C_G + 128
    NBF = C_ID + 128
    C_P1 = C_W2XB            # part 1 = w1b, w2hi, w2lo (cols 0..C_W2XB)


    nc = bacc.Bacc("TRN2", target_bir_lowering=False, debug=False,
                   num_devices=NCORES)
    fpk = nc.declare_dram_parameter("fpk", [128, 900 + 128], f32, isOutput=False)
    bpk = nc.declare_dram_parameter("bpk", [128, NBF], bf16, isOutput=False)
    out_d = nc.declare_dram_parameter("out", [BPC, 1800, D], bf16, isOutput=True)

    with ExitStack() as ctx:
        tc = ctx.enter_context(tile.TileContext(nc))
        cp = ctx.enter_context(tc.tile_pool(name="const", bufs=1))
        sc = ctx.enter_context(tc.tile_pool(name="scratch", bufs=1))
        # PSUM pools (8 banks): zp 4x[128,512]=4 + opa 3x[128,512]=3 +
        # tp 2x[128,128]=0.5
        zp = ctx.enter_context(tc.tile_pool(name="zp", bufs=4, space="PSUM"))
        opa = ctx.enter_context(tc.tile_pool(name="opa", bufs=4, space="PSUM"))
        gpck = ctx.enter_context(tc.tile_pool(name="gpck", bufs=2))
        octp = ctx.enter_context(tc.tile_pool(name="octp", bufs=4))
        cdp = ctx.enter_context(tc.tile_pool(name="cdp", bufs=1))
        htp = ctx.enter_context(tc.tile_pool(name="htp", bufs=8))
        bstg = ctx.enter_context(tc.tile_pool(name="bstage", bufs=3))
        dstg = ctx.enter_context(tc.tile_pool(name="dstage", bufs=3))

        fpack = cp.tile([128, 900 + 128], f32)
        nc.sync.dma_start(fpack[:], fpk[:])
        bpack = cp.tile([128, NBF], bf16)
        # split the weight load: small part (warm-up, gram, z) first so the
        # big w2xb block (12KB/partition) doesn't gate the early phases
        nc.sync.dma_start(bpack[:, C_G:NBF], bpk[:, C_G:NBF])
        nc.sync.dma_start(bpack[:, 0:C_P1], bpk[:, 0:C_P1])
        nc.sync.dma_start(bpack[:, C_P1:C_G], bpk[:, C_P1:C_G])

        raw = fpack[:, 0:900]
        idf = fpack[:, 900:1028]
        w1b = [(bpack[:, C_W1B + (2 * b) * 128: C_W1B + (2 * b + 1) * 128],
                bpack[:, C_W1B + (2 * b + 1) * 128: C_W1B + (2 * b + 2) * 128])
               for b in range(4)]
        w2hi = bpack[:, C_W2HI:C_W2HI + 512]
        w2lo = bpack[:, C_W2LO:C_W2LO + 512]
        w2xb = [[bpack[:, C_W2XB + (c * 4 + b) * 512: C_W2XB + (c * 4 + b + 1) * 512]
                 for b in range(4)] for c in range(3)]
        Gblk = bpack[:, C_G:C_G + 128]
        idb = bpack[:, C_ID:C_ID + 128]

        # ---------------- PE pre-warm: junk matmuls on memset tiles -----
        # (independent of the input DMAs so the PE busies from ~0.5us)
        junkw = cp.tile([128, 512], bf16)
        nc.vector.memset(junkw[:], 0.25)
        for _ in range(70):
            wps = opa.tile([128, D], f32, tag="oa", name="oa")
            nc.tensor.matmul(wps[:], junkw[:, 0:128], junkw[:],
                             start=True, stop=True)

        # ---------------- P1: feature planes ----------------
        TFB = cp.tile([128, JBP * 32], bf16)
        TFD = cp.tile([128, JDP * 32], bf16)
        TFb = TFB.rearrange("p (j f) -> p j f", f=32)
        TFd = TFD.rearrange("p (j f) -> p j f", f=32)
        # zeros: box f16,17 + f20..31 + pad slots; dist all but f16,f17
        # zeros via ACT (uint32-bitcast x0 is NaN-safe); tiny pads + the 1.0
        # fill on DVE.  Pad slots' f0..9 must precede the gram prepack.
        nc.vector.memset(TFb[:, JB:JBP, 0:16], 0.0)
        nc.vector.memset(TFb[:, JB:JBP, 18:20], 0.0)
        nc.scalar.memzero(TFb[:, :, 16:18])
        nc.scalar.memzero(TFb[:, :, 20:32])
        nc.scalar.memzero(TFd[:, :, 0:16])
        nc.scalar.memzero(TFd[:, :, 18:32])
        nc.vector.memset(TFd[:, :, 17], 1.0)
        nc.vector.memset(TFd[:, JD:JDP, 16], 0.0)

        TFr = TFb[:, 0:JB, :]
        raw6 = raw.rearrange("p (b s) -> p b s", s=6)
        rawp = raw.rearrange("p (m g s) -> p m g s", g=2, s=6)
        TFbp = TFB[:, 0:JB * 32].rearrange("p (m g f) -> p m g f", g=2, f=32)

        sPres = sc.tile([128, JB], f32)
        sKey = sc.tile([128, JB], f32)
        sSwap = sc.tile([128, JD], f32)
        sD = sc.tile([128, JD], f32)
        sSD = sc.tile([128, JD], f32)
        sT0 = sc.tile([128, JB], f32)
        sT1 = sc.tile([128, JB], f32)

        nc.vector.tensor_tensor(sT0[:], raw6[:, :, 0], raw6[:, :, 1], A.add)
        nc.vector.tensor_tensor(sT1[:], raw6[:, :, 2], raw6[:, :, 3], A.add)
        nc.vector.tensor_tensor(sT0[:], sT0[:], sT1[:], A.add)
        nc.vector.tensor_scalar(sPres[:], sT0[:], 0.0, None, A.not_equal)
        nc.vector.scalar_tensor_tensor(sKey[:], sPres[:], -1000.0,
                                       raw6[:, :, 4], A.mult, A.add)
        sKeyp = sKey.rearrange("p (m g) -> p m g", g=2)
        nc.vector.tensor_tensor(sSwap[:], sKeyp[:, :, 1], sKeyp[:, :, 0], A.is_lt)

        # block compare-and-swap: all 6 raw components in 4 DVE ops
        sRaw = sc.tile([128, JD, 2, 6], f32)
        sD6 = sc.tile([128, JD, 6], f32)
        swb = sSwap[:].unsqueeze(-1).broadcast_to([128, JD, 6])
        nc.vector.tensor_tensor(sD6[:], rawp[:, :, 1, :], rawp[:, :, 0, :],
                                A.subtract)
        nc.vector.tensor_tensor(sD6[:], sD6[:], swb, A.mult)
        nc.vector.tensor_tensor(sRaw[:, :, 0, :], rawp[:, :, 0, :], sD6[:], A.add)
        nc.vector.tensor_tensor(sRaw[:, :, 1, :], rawp[:, :, 1, :], sD6[:],
                                A.subtract)
        sPresP = sPres.rearrange("p (m g) -> p m g", g=2)
        nc.vector.tensor_tensor(sD[:], sPresP[:, :, 1], sPresP[:, :, 0], A.subtract)
        nc.vector.tensor_tensor(sSD[:], sD[:], sSwap[:], A.mult)
        nc.vector.tensor_tensor(TFbp[:, :, 0, 14], sPresP[:, :, 0], sSD[:], A.add)
        nc.vector.tensor_tensor(TFbp[:, :, 1, 14], sPresP[:, :, 1], sSD[:], A.subtract)

        sRw = sRaw.rearrange("p m g s -> p (m g) s")
        sX1, sY1, sX2, sY2 = (sRw[:, :, i] for i in range(4))
        sCat, sConf = sRw[:, :, 4], sRw[:, :, 5]
        # all derived geometry in f32 scratch (bf16-rounded coords would
        # catastrophically cancel in w/h near zero -> aspect blows up)
        sW32 = sc.tile([128, JB], f32)
        sH32 = sc.tile([128, JB], f32)
        sCx = sc.tile([128, JB], f32)
        sCy = sc.tile([128, JB], f32)
        nc.scalar.mul(TFr[:, :, 0], sX1, 1.0 / IW)
        nc.scalar.mul(TFr[:, :, 1], sY1, 1.0 / IH)
        nc.scalar.mul(TFr[:, :, 2], sX2, 1.0 / IW)
        nc.scalar.mul(TFr[:, :, 3], sY2, 1.0 / IH)
        nc.vector.tensor_tensor(sW32[:], sX2, sX1, A.subtract)
        nc.vector.tensor_tensor(sH32[:], sY2, sY1, A.subtract)
        nc.vector.tensor_tensor(sCx[:], sX1, sX2, A.add)
        nc.vector.tensor_tensor(sCy[:], sY1, sY2, A.add)
        nc.scalar.mul(TFr[:, :, 4], sW32[:], 1.0 / IW)
        nc.scalar.mul(TFr[:, :, 5], sH32[:], 1.0 / IH)
        nc.scalar.mul(TFr[:, :, 6], sCx[:], 1.0 / IW)
        nc.scalar.mul(TFr[:, :, 7], sCy[:], 1.0 / IH)
        nc.scalar.mul(TFr[:, :, 18], sCx[:], 1.0 / IW)
        nc.scalar.mul(TFr[:, :, 19], sCy[:], 1.0 / IH)
        sT2 = sc.tile([128, JB], f32)
        nc.vector.tensor_tensor(sT2[:], sW32[:], sH32[:], A.mult)
        nc.scalar.mul(TFr[:, :, 8], sT2[:], 1.0 / (IW * IH))
        sHp = sT0
        nc.vector.tensor_scalar(sHp[:], sH32[:], 1e-6 * IH, 1.0 / IH, A.add, A.mult)
        sR = sT1
        nc.vector.reciprocal(sR[:], sHp[:])
        nc.vector.tensor_scalar(sT2[:], sW32[:], 1.0 / IW, None, A.mult)
        nc.vector.tensor_tensor(TFr[:, :, 9], sT2[:], sR[:], A.mult)
        for k in range(3):
            nc.vector.scalar_tensor_tensor(TFr[:, :, 10 + k], sCat, float(k),
                                           TFr[:, :, 14], A.is_equal, A.mult)
        nc.vector.tensor_tensor(TFr[:, :, 13], sConf, TFr[:, :, 14], A.mult)
        nc.scalar.activation(TFr[:, :, 15], TFr[:, :, 14],
                             AF.Identity, bias=1.0, scale=-1.0)
        # dist features: f16 = 0.5*sqrt(dx2^2+dy2^2)/IW-scaled, f17 = 1
        sDx = sc.tile([128, JD], f32)
        sDy = sc.tile([128, JD], f32)
        sCxp = sCx.rearrange("p (m g) -> p m g", g=2)
        sCyp = sCy.rearrange("p (m g) -> p m g", g=2)
        nc.vector.tensor_tensor(sDx[:], sCxp[:, :, 0], sCxp[:, :, 1], A.subtract)
        nc.vector.tensor_tensor(sDy[:], sCyp[:, :, 0], sCyp[:, :, 1], A.subtract)
        nc.vector.tensor_scalar(sDx[:], sDx[:], 1.0 / IW, None, A.mult)
        nc.vector.tensor_scalar(sDy[:], sDy[:], 1.0 / IH, None, A.mult)
        nc.vector.tensor_tensor(sDx[:], sDx[:], sDx[:], A.mult)
        nc.vector.tensor_tensor(sDy[:], sDy[:], sDy[:], A.mult)
        nc.vector.tensor_tensor(sDx[:], sDx[:], sDy[:], A.add)
        nc.scalar.activation(TFd[:, 0:JD, 16], sDx[:], AF.Sqrt, scale=0.25)

        # ---------------- P2: gram variance ----------------
        v = sc.tile([128, 160], f32)
        copy_rr = [nc.vector.tensor_copy, nc.scalar.copy]

        cta_d = cp.tile([128, NDC * 128], bf16)
        # prepack geom features f0..9 of all 156 slots contiguously; each
        # 128-col transpose window overlaps 8 cols into the next pack, which
        # land on zero rows of Gblk (harmless).
        gprep = sc.tile([128, 13 * 120 + 8], bf16)
        nc.vector.memset(gprep[:, 13 * 120:], 0.0)
        nc.vector.tensor_copy(
            gprep[:, 0:1560].rearrange("p (j f) -> p j f", f=10),
            TFb[:, 0:156, 0:10])

        for gi in range(NGP):
            s0 = 12 * gi
            src = TFb[:, s0:s0 + 12, 0:10]
            pst = zp.tile([128, 1024], bf16, tag="z", name="z")
            ps = pst[:, 0:128]
            nc.tensor.transpose(ps[:], gprep[:, 120 * gi:120 * gi + 128], idb)
            pk = gpck.tile([128, 128], bf16, tag="gp", name="gp")
            nc.scalar.copy(pk[:], ps[:])
            yt = zp.tile([128, 512], f32, tag="z", name="z")
            y = yt[:, 0:128]
            nc.tensor.matmul(y, pk[:], Gblk, start=True, stop=True)
            tmp = sc.tile([128, 120], f32, tag="gtmp", name="gtmp")
            nc.vector.tensor_tensor(tmp[:], src, y[:, 0:120], A.mult)
            nc.vector.tensor_reduce(v[:, s0:s0 + 12],
                                    tmp.rearrange("p (j f) -> p j f", f=10),
                                    mybir.AxisListType.X, A.add)
            for dc in range(int(gi * 19 / 13), int((gi + 1) * 19 / 13)):
                pst2 = zp.tile([128, 1024], bf16, tag="z", name="z")
                psd = pst2[:, 0:128]
                nc.tensor.transpose(psd[:], TFd[:, 4 * dc:4 * dc + 4, :], idb)
                copy_rr[dc % 2](cta_d[:, dc * 128:(dc + 1) * 128], psd[:])

        # ---------------- P3: rstd + feature scale + gelu preload --------
        eps = sc.tile([128, 1], f32)
        nc.vector.memset(eps[:], 1e-5)
        sd = sc.tile([128, 156], f32)
        rstd = sc.tile([128, 156], f32)
        nc.scalar.activation(sd[:], v[:, 0:156], AF.Sqrt,
                             bias=eps[:], scale=1.0 / DH)
        nc.vector.reciprocal(rstd[:], sd[:])
        # preload the Gelu ACT table off the critical path
        gjunk = sc.tile([128, 8], bf16)
        nc.scalar.activation(gjunk[:], sd[:, 0:8], AF.Gelu)
        # scale geometry features f0..9 by rstd (per token)
        for f in range(10):
            nc.vector.tensor_tensor(TFb[:, :, f], TFb[:, :, f], rstd[:], A.mult)

        # ---------------- P2c: dist W2 + staging ----------------
        dist_stage = {"tile": None, "fill": 0, "base": 0}
        vd = out_d[:, 0:600, :].rearrange("b (q r) d -> b q r d", q=8)

        def stage_dist(kd, o):
            if dist_stage["tile"] is None:
                dist_stage["tile"] = dstg.tile([128, 6 * D], bf16, tag="dstage",
                                               name="dstage")
                dist_stage["fill"] = 0
                dist_stage["base"] = kd
            fill = dist_stage["fill"]
            copy_rr[kd % 2](dist_stage["tile"][:, fill * D:(fill + 1) * D], o[:])
            dist_stage["fill"] = fill + 1
            if dist_stage["fill"] == 6 or kd == JD - 1:
                b0 = dist_stage["base"]
                gsz = dist_stage["fill"]
                nc.sync.dma_start(vd[:, :, b0:b0 + gsz, :],
                                  dist_stage["tile"][:, 0:gsz * D])
                dist_stage["tile"] = None


        # mid fillers: keep the PE busy across the rstd chain window
        for _ in range(28):
            wps = opa.tile([128, D], f32, tag="oa", name="oa")
            nc.tensor.matmul(wps[:], junkw[:, 0:128], junkw[:],
                             start=True, stop=True)

        # ---------------- P4/P5: box pipeline ----------------
        vb = out_d[:, 600:1800, :].rearrange("b (q r) d -> b q r d", q=8)
        oct_tiles = {}     # o -> sbuf [128, 256] bf16 (chunks 2o | 2o+1)
        ht_tiles = {}      # o -> list of 4 sbuf [128, 512] bf16 (per band)
        box_stage = {"tile": None, "fill": 0, "base": 0}
        ccnt = {"i": 0}

        def emit_pass2_half(o, ci):
            if ci == 0:
                oct_tiles[o] = octp.tile([128, 256], bf16, tag="oct", name="oct")
            t = oct_tiles[o]
            c = 2 * o + ci
            pst = zp.tile([128, 1024], bf16, tag="z", name="z")
            ps = pst[:, 0:128]
            nc.tensor.transpose(ps[:], TFb[:, 4 * c:4 * c + 4, :], idb)
            copy_rr[c % 2](t[:, ci * 128:(ci + 1) * 128], ps[:])

        z_banks = {}

        def emit_z_mm(o):
            rhs = oct_tiles[o]
            zbs = []
            for b in range(4):
                zb = zp.tile([128, 512], f32, tag="z", name="z")
                nc.tensor.matmul(zb[:, 0:256], w1b[b][0], rhs[:],
                                 start=True, stop=True)
                nc.tensor.matmul(zb[:, 256:512], w1b[b][1], rhs[:],
                                 start=True, stop=True)
                zbs.append(zb)
            z_banks[o] = zbs

        def emit_gelu(o):
            zbs = z_banks.pop(o)
            hts = []
            for b in range(4):
                ht = htp.tile([128, 512], bf16, tag="ht", name="ht")
                nc.scalar.activation(ht[:], zbs[b][:], AF.Gelu)
                hts.append(ht)
            ht_tiles[o] = hts

        def flush_box(last_k):
            b0 = box_stage["base"]
            gsz = box_stage["fill"]
            nc.sync.dma_start(vb[:, :, b0:b0 + gsz, :],
                              box_stage["tile"][:, 0:gsz * D])
            box_stage["tile"] = None

        def emit_w2(o):
            hts = ht_tiles.pop(o)
            cchunk = oct_tiles[o]
            for ci in range(2):
                c = 2 * o + ci
                for b in range(4):
                    s = 4 * c + b
                    if s >= JB:
                        continue
                    ht = hts[b]
                    cam = (s % 6) // 2
                    ot = opa.tile([128, D], f32, tag="oa", name="oa")
                    nc.tensor.matmul(ot[:], ht[:, ci * 128:ci * 128 + 128],
                                     w2hi, start=True, stop=False)
                    nc.tensor.matmul(ot[:], ht[:, 256 + ci * 128:256 + ci * 128 + 128],
                                     w2lo, start=False, stop=False)
                    nc.tensor.matmul(ot[:], cchunk[:, ci * 128:(ci + 1) * 128],
                                     w2xb[cam][b], start=False, stop=True)
                    if box_stage["tile"] is None:
                        box_stage["tile"] = bstg.tile([128, 8 * D], bf16,
                                                      tag="bstage", name="bstage")
                        box_stage["fill"] = 0
                        box_stage["base"] = s - s % 8
                    csel = 0 if (ccnt["i"] % 8) in (0, 2, 3, 5, 6) else 1
                    copy_rr[csel](
                        box_stage["tile"][:, (s % 8) * D:(s % 8 + 1) * D], ot[:])
                    ccnt["i"] += 1
                    box_stage["fill"] += 1
                    if box_stage["fill"] == 8 or s == JB - 1:
                        flush_box(s)
            oct_tiles.pop(o)
            dc = o
            for b in range(4):
                kd = 4 * dc + b
                if kd >= JD:
                    continue
                od = opa.tile([128, D], f32, tag="oa", name="oa")
                nc.tensor.matmul(od[:], cta_d[:, dc * 128:(dc + 1) * 128],
                                 w2xb[0][b], start=True, stop=True)
                stage_dist(kd, od)

        for step in range(NOCT + 2):
            if step < NOCT:
                emit_pass2_half(step, 0)
            if 1 <= step <= NOCT:
                emit_z_mm(step - 1)
            if step < NOCT:
                emit_pass2_half(step, 1)
            if step >= 2:
                emit_w2(step - 2)
            if 1 <= step <= NOCT:
                emit_gelu(step - 1)

    nc.compile()
    return nc


def _prep_inputs(inputs):
    f32 = np.float32
    bf = ml_dtypes.bfloat16
    scale = float(np.asarray(inputs["scale"]))

    W1p = np.zeros((32, DH), f32)
    W1p[0:10] = np.asarray(inputs["geom_w1"], f32)
    W1p[6] *= 0.5
    W1p[7] *= 0.5
    W1p -= W1p.mean(axis=1, keepdims=True)

    w1b_cols = []
    for b in range(4):
        hi = np.zeros((128, 128), f32)
        hi[32 * b:32 * b + 32] = W1p[:, :128]
        lo = np.zeros((128, 128), f32)
        lo[32 * b:32 * b + 32] = W1p[:, 128:]
        w1b_cols += [hi, lo]

    W2s = scale * np.asarray(inputs["geom_w2"], f32)
    w2hi, w2lo = W2s[:128], W2s[128:]

    cat_t = np.asarray(inputs["cat_table"], f32)
    cam_t = np.asarray(inputs["cam_table"], f32)
    bias_row = (np.asarray(inputs["geom_b2"], f32)
                + np.asarray(inputs["conf_b"], f32)
                + np.asarray(inputs["center_b"], f32))
    w2xb_cols = []
    for c in range(3):
        W2X = np.zeros((32, D), f32)
        W2X[10:13] = scale * cat_t
        W2X[13] = scale * np.asarray(inputs["conf_w"], f32)[0]
        W2X[14] = scale * (bias_row + cam_t[c])
        W2X[15] = np.asarray(inputs["missing_emb"], f32)[0]
        W2X[16] = np.asarray(inputs["dist_w"], f32)[0]
        W2X[17] = np.asarray(inputs["dist_b"], f32)
        W2X[18] = scale * np.asarray(inputs["center_w"], f32)[0] * 0.5
        W2X[19] = scale * np.asarray(inputs["center_w"], f32)[1] * 0.5
        for b in range(4):
            t = np.zeros((128, D), f32)
            t[32 * b:32 * b + 32] = W2X
            w2xb_cols.append(t)

    G10 = (W1p[0:10] @ W1p[0:10].T).astype(f32)
    Gblk = np.zeros((128, 128), f32)
    for s in range(12):
        Gblk[10 * s:10 * s + 10, 10 * s:10 * s + 10] = G10

    idf32 = np.eye(128, dtype=f32)
    bpk = np.concatenate(w1b_cols + [w2hi, w2lo] + w2xb_cols + [Gblk, idf32],
                         axis=1).astype(bf)

    box = np.asarray(inputs["box_data"], f32)
    fpks = []
    for c in range(NCORES):
        rawc = box[c * BPC:(c + 1) * BPC].reshape(BPC, T * 6, 6)
        rawc = rawc.reshape(BPC, 8, JB, 6).reshape(128, 900)
        fpks.append(np.ascontiguousarray(
            np.concatenate([rawc, idf32], axis=1), dtype=f32))
    return fpks, bpk


def _fast_path_ok(inputs):
    try:
        shapes = {
            "box_data": (B, T, 6, 6), "cat_table": (3, D), "geom_w1": (10, DH),
            "geom_b1": (DH,), "ln_g": (DH,), "ln_b": (DH,), "geom_w2": (DH, D),
            "geom_b2": (D,), "conf_w": (1, D), "conf_b": (D,),
            "center_w": (2, D), "center_b": (D,), "missing_emb": (1, D),
            "dist_w": (1, D), "dist_b": (D,), "cam_table": (NCAM, D),
        }
        for k, s in shapes.items():
            if tuple(np.asarray(inputs[k]).shape) != s:
                return False
        if not np.all(np.asarray(inputs["geom_b1"]) == 0):
            return False
        if not np.all(np.asarray(inputs["ln_g"]) == 1):
            return False
        if not np.all(np.asarray(inputs["ln_b"]) == 0):
            return False
        return True
    except Exception:
        return False


def _numpy_fallback(inputs):
    import math
    f32 = np.float32
    inp = {k: np.asarray(v) for k, v in inputs.items()}
    coords = inp["box_data"][..., :4].astype(f32)
    category = inp["box_data"][..., 4].astype(np.int32)
    conf = inp["box_data"][..., 5].astype(f32)
    norm = np.array([IW, IH, IW, IH], f32)
    cn = (coords / norm).reshape(B, T, NCAM, NB, 4)
    category = category.reshape(B, T, NCAM, NB)
    conf = conf.reshape(B, T, NCAM, NB, 1)
    presence = (cn.sum(-1) != 0).astype(f32)
    sort_key = category.astype(f32) + (1.0 - presence) * 1000.0
    idx = np.argsort(sort_key, axis=-1, kind="stable")
    cn = np.take_along_axis(cn, idx[..., None], axis=-2)
    category = np.take_along_axis(category, idx, axis=-1)
    conf = np.take_along_axis(conf, idx[..., None], axis=-2)
    presence = (cn.sum(-1) != 0).astype(f32)[..., None]
    x1, y1, x2, y2 = cn[..., 0], cn[..., 1], cn[..., 2], cn[..., 3]
    w, h = x2 - x1, y2 - y1
    cx, cy = (x1 + x2) * 0.5, (y1 + y2) * 0.5
    area, aspect = w * h, w / (h + 1e-6)
    dx, dy = cx[..., 0] - cx[..., 1], cy[..., 0] - cy[..., 1]
    dist = np.sqrt(dx * dx + dy * dy)[..., None]
    dist_tok = dist @ inp["dist_w"].astype(f32) + inp["dist_b"].astype(f32)
    geom = np.stack([x1, y1, x2, y2, w, h, cx, cy, area, aspect], axis=-1)
    z = geom @ inp["geom_w1"].astype(f32) + inp["geom_b1"].astype(f32)
    mu = z.mean(-1, keepdims=True)
    var = ((z - mu) ** 2).mean(-1, keepdims=True)
    xh = (z - mu) / np.sqrt(var + 1e-5) * inp["ln_g"].astype(f32) + inp["ln_b"].astype(f32)
    try:
        from scipy.special import erf as _erf
        g = xh * 0.5 * (1.0 + _erf(xh / np.sqrt(2.0)))
    except Exception:
        verf = np.vectorize(math.erf)
        g = xh * 0.5 * (1.0 + verf(xh / np.sqrt(2.0)))
    geom_p = g @ inp["geom_w2"].astype(f32) + inp["geom_b2"].astype(f32)
    cat_emb = inp["cat_table"].astype(f32)[category]
    conf_p = conf @ inp["conf_w"].astype(f32) + inp["conf_b"].astype(f32)
    center_p = np.stack([cx, cy], axis=-1) @ inp["center_w"].astype(f32) + inp["center_b"].astype(f32)
    cam_emb = inp["cam_table"].astype(f32).reshape(1, 1, NCAM, 1, D)
    tok = (geom_p + cat_emb + conf_p + center_p + cam_emb) * float(inp["scale"])
    tok = np.where(presence == 0, inp["missing_emb"].astype(f32)[0], tok)
    out = np.concatenate([dist_tok.reshape(B, T * NCAM, D),
                          tok.reshape(B, T * NCAM * NB, D)], axis=1)
    return out.astype(np.float32)


def _run(inputs, trace=False, tmpdir=None):
    from concourse.bass_utils import run_bass_kernel_spmd

    if "nc" not in _CACHE:
        _CACHE["nc"] = _build_nc()
    nc = _CACHE["nc"]

    fpks, bpk = _prep_inputs(inputs)
    in_maps = [{"fpk": fpks[c], "bpk": bpk} for c in range(NCORES)]
    res = run_bass_kernel_spmd(nc, in_maps, list(range(NCORES)),
                               trace=trace, tmpdir=tmpdir)
    out = np.concatenate([np.asarray(res.results[c]["out"])
                          for c in range(NCORES)], axis=0)
    return out.astype(np.float32), res


def kernel(**inputs):
    if not _fast_path_ok(inputs):
        return _numpy_fallback(inputs)
    out, _ = _run(inputs)
    return out


if __name__ == "__main__":
    import reference as ref
    inputs = {k: np.asarray(v) for k, v in ref.setup_inputs().items()}
    got = kernel(**inputs)
    exp = np.load("/tmp/expected.npy")
    d = got - exp
    print("rel fro:", np.linalg.norm(d) / np.linalg.norm(exp))
    print("absmax rel:", np.abs(d).max() / np.abs(exp).max())


# revision 14
# speedup vs baseline: 1.1954x; 1.0044x over previous
"""Trainium2 Bass kernel v3 for nn_BoxEncoder (B=128, T=200, NC=3, NB=2, D=512, DH=256).

Data-parallel over batch: 16 batch items per core x 8 cores; partition
p = bt*8 + q.  Per partition: 75 dist tokens (+1 pad) and 150 box tokens
(+2 pad), processed as 4-slot transpose chunks.

v3 vs v2 (~2x): the whole kernel is built to keep the PE p-state warm
(cost model: matmuls run 2x faster once the PE has been ~continuously
busy for 3us; any long stall drops it back):
 - z is computed TRANSPOSED (weights-stationary): zT = W1bandedT @ cta_s,
   so gelu(zT) directly yields hT = the lhsT of the W2 matmuls.  The
   dma_start_transpose of h (SP-queue serial 1.24us each + 900ns sem) is
   gone - that chain caused the recurring 3-5us PE stalls in v2.
 - LN rstd is folded into the geometry features BEFORE the z matmul
   (f0..f9 *= rstd per token), so gelu needs no per-partition scale and
   batches [128,512] over 4 slots per call.  cx,cy are duplicated into
   f18,f19 (unscaled) for the center_w rows of the extras matmul.
 - all matmuls are full-K (no tile_position): banded *weights* (zero rows
   outside the slot's 32-band) instead of banded matmuls; the extras and
   z matmuls share one LDWEIGHTS of the feature chunk.
 - variance via a 12-slot x 10-feature gram pack (13 transposes instead
   of 38) and a single batched sqrt+reciprocal.
 - PE pre-warm: a dozen junk matmuls issued at t~1us keep the PE busy
   during the DVE feature-prep phase so the clock is warm when real
   matmuls start.
 - staging copies round-robin DVE/ACT/GPSIMD; out DMAs on the otherwise
   idle SP queue; one ACT table preload for Gelu right after the rstd
   sqrt so no table load lands inside the steady-state loop.
"""

import numpy as np
import ml_dtypes

B, T, NCAM, NB, D, DH = 128, 200, 3, 2, 512, 256
IW, IH = 640.0, 400.0
NCORES = 8
BPC = B // NCORES
JB = 150                  # real box slots per partition
JBP = 156                 # padded (38 chunks use 152; gram packs use 156)
JD = 75                   # real dist slots
JDP = 76                  # padded (19 chunks)
NCH = 38                  # box chunks
NDC = 19                  # dist chunks
NOCT = 19                 # box octs (2 chunks = 8 slots each; last has 6)
NGP = 13                  # gram packs (12 slots x 10 feats)

_CACHE = {}


def _build_nc():
    from contextlib import ExitStack
    import concourse.bacc as bacc
    import concourse.mybir as mybir
    import concourse.tile as tile

    f32 = mybir.dt.float32
    bf16 = mybir.dt.bfloat16
    A = mybir.AluOpType
    AF = mybir.ActivationFunctionType

    # bpk bf16 column offsets
    C_W1B = 0                       # 8 x 128 (band b: hi, lo)
    C_W2HI = C_W1B + 8 * 128
    C_W2LO = C_W2HI + 512
    C_W2XB = C_W2LO + 512           # 12 x 512 (cam c, band b)
    C_G = C_W2XB + 12 * 512
    C_ID = C_G + 128
    NBF = C_ID + 128
    C_P1 = C_W2XB            # part 1 = w1b, w2hi, w2lo (cols 0..C_W2XB)


    nc = bacc.Bacc("TRN2", target_bir_lowering=False, debug=False,
                   num_devices=NCORES)
    fpk = nc.declare_dram_parameter("fpk", [128, 900 + 128], f32, isOutput=False)
    bpk = nc.declare_dram_parameter("bpk", [128, NBF], bf16, isOutput=False)
    out_d = nc.declare_dram_parameter("out", [BPC, 1800, D], bf16, isOutput=True)

    with ExitStack() as ctx:
        tc = ctx.enter_context(tile.TileContext(nc))
        cp = ctx.enter_context(tc.tile_pool(name="const", bufs=1))
        sc = ctx.enter_context(tc.tile_pool(name="scratch", bufs=1))
        # PSUM pools (8 banks): zp 4x[128,512]=4 + opa 3x[128,512]=3 +
        # tp 2x[128,128]=0.5
        zp = ctx.enter_context(tc.tile_pool(name="zp", bufs=4, space="PSUM"))
        opa = ctx.enter_context(tc.tile_pool(name="opa", bufs=4, space="PSUM"))
        gpck = ctx.enter_context(tc.tile_pool(name="gpck", bufs=2))
        octp = ctx.enter_context(tc.tile_pool(name="octp", bufs=4))
        cdp = ctx.enter_context(tc.tile_pool(name="cdp", bufs=1))
        htp = ctx.enter_context(tc.tile_pool(name="htp", bufs=8))
        bstg = ctx.enter_context(tc.tile_pool(name="bstage", bufs=3))
        dstg = ctx.enter_context(tc.tile_pool(name="dstage", bufs=3))

        fpack = cp.tile([128, 900 + 128], f32)
        nc.sync.dma_start(fpack[:], fpk[:])
        bpack = cp.tile([128, NBF], bf16)
        # split the weight load: small part (warm-up, gram, z) first so the
        # big w2xb block (12KB/partition) doesn't gate the early phases
        nc.sync.dma_start(bpack[:, C_G:NBF], bpk[:, C_G:NBF])
        nc.sync.dma_start(bpack[:, 0:C_P1], bpk[:, 0:C_P1])
        nc.sync.dma_start(bpack[:, C_P1:C_G], bpk[:, C_P1:C_G])

        raw = fpack[:, 0:900]
        idf = fpack[:, 900:1028]
        w1b = [(bpack[:, C_W1B + (2 * b) * 128: C_W1B + (2 * b + 1) * 128],
                bpack[:, C_W1B + (2 * b + 1) * 128: C_W1B + (2 * b + 2) * 128])
               for b in range(4)]
        w2hi = bpack[:, C_W2HI:C_W2HI + 512]
        w2lo = bpack[:, C_W2LO:C_W2LO + 512]
        w2xb = [[bpack[:, C_W2XB + (c * 4 + b) * 512: C_W2XB + (c * 4 + b + 1) * 512]
                 for b in range(4)] for c in range(3)]
        Gblk = bpack[:, C_G:C_G + 128]
        idb = bpack[:, C_ID:C_ID + 128]

        # ---------------- PE pre-warm: junk matmuls on memset tiles -----
        # (independent of the input DMAs so the PE busies from ~0.5us)
        junkw = cp.tile([128, 512], bf16)
        nc.vector.memset(junkw[:], 0.25)
        for _ in range(70):
            wps = opa.tile([128, D], f32, tag="oa", name="oa")
            nc.tensor.matmul(wps[:], junkw[:, 0:128], junkw[:],
                             start=True, stop=True)

        # ---------------- P1: feature planes ----------------
        TFB = cp.tile([128, JBP * 32], bf16)
        TFD = cp.tile([128, JDP * 32], bf16)
        TFb = TFB.rearrange("p (j f) -> p j f", f=32)
        TFd = TFD.rearrange("p (j f) -> p j f", f=32)
        # zeros: box f16,17 + f20..31 + pad slots; dist all but f16,f17
        # zeros via ACT (uint32-bitcast x0 is NaN-safe); tiny pads + the 1.0
        # fill on DVE.  Pad slots' f0..9 must precede the gram prepack.
        nc.vector.memset(TFb[:, JB:JBP, 0:16], 0.0)
        nc.vector.memset(TFb[:, JB:JBP, 18:20], 0.0)
        nc.scalar.memzero(TFb[:, :, 16:18])
        nc.scalar.memzero(TFb[:, :, 20:32])
        nc.scalar.memzero(TFd[:, :, 0:16])
        nc.scalar.memzero(TFd[:, :, 18:32])
        nc.vector.memset(TFd[:, :, 17], 1.0)
        nc.vector.memset(TFd[:, JD:JDP, 16], 0.0)

        TFr = TFb[:, 0:JB, :]
        raw6 = raw.rearrange("p (b s) -> p b s", s=6)
        rawp = raw.rearrange("p (m g s) -> p m g s", g=2, s=6)
        TFbp = TFB[:, 0:JB * 32].rearrange("p (m g f) -> p m g f", g=2, f=32)

        sPres = sc.tile([128, JB], f32)
        sKey = sc.tile([128, JB], f32)
        sSwap = sc.tile([128, JD], f32)
        sD = sc.tile([128, JD], f32)
        sSD = sc.tile([128, JD], f32)
        sT0 = sc.tile([128, JB], f32)
        sT1 = sc.tile([128, JB], f32)

        nc.vector.tensor_tensor(sT0[:], raw6[:, :, 0], raw6[:, :, 1], A.add)
        nc.vector.tensor_tensor(sT1[:], raw6[:, :, 2], raw6[:, :, 3], A.add)
        nc.vector.tensor_tensor(sT0[:], sT0[:], sT1[:], A.add)
        nc.vector.tensor_scalar(sPres[:], sT0[:], 0.0, None, A.not_equal)
        nc.vector.scalar_tensor_tensor(sKey[:], sPres[:], -1000.0,
                                       raw6[:, :, 4], A.mult, A.add)
        sKeyp = sKey.rearrange("p (m g) -> p m g", g=2)
        nc.vector.tensor_tensor(sSwap[:], sKeyp[:, :, 1], sKeyp[:, :, 0], A.is_lt)

        # block compare-and-swap: all 6 raw components in 4 DVE ops
        sRaw = sc.tile([128, JD, 2, 6], f32)
        sD6 = sc.tile([128, JD, 6], f32)
        swb = sSwap[:].unsqueeze(-1).broadcast_to([128, JD, 6])
        nc.vector.tensor_tensor(sD6[:], rawp[:, :, 1, :], rawp[:, :, 0, :],
                                A.subtract)
        nc.vector.tensor_tensor(sD6[:], sD6[:], swb, A.mult)
        nc.vector.tensor_tensor(sRaw[:, :, 0, :], rawp[:, :, 0, :], sD6[:], A.add)
        nc.vector.tensor_tensor(sRaw[:, :, 1, :], rawp[:, :, 1, :], sD6[:],
                                A.subtract)
        sPresP = sPres.rearrange("p (m g) -> p m g", g=2)
        nc.vector.tensor_tensor(sD[:], sPresP[:, :, 1], sPresP[:, :, 0], A.subtract)
        nc.vector.tensor_tensor(sSD[:], sD[:], sSwap[:], A.mult)
        nc.vector.tensor_tensor(TFbp[:, :, 0, 14], sPresP[:, :, 0], sSD[:], A.add)
        nc.vector.tensor_tensor(TFbp[:, :, 1, 14], sPresP[:, :, 1], sSD[:], A.subtract)

        sRw = sRaw.rearrange("p m g s -> p (m g) s")
        sX1, sY1, sX2, sY2 = (sRw[:, :, i] for i in range(4))
        sCat, sConf = sRw[:, :, 4], sRw[:, :, 5]
        # all derived geometry in f32 scratch (bf16-rounded coords would
        # catastrophically cancel in w/h near zero -> aspect blows up)
        sW32 = sc.tile([128, JB], f32)
        sH32 = sc.tile([128, JB], f32)
        sCx = sc.tile([128, JB], f32)
        sCy = sc.tile([128, JB], f32)
        nc.scalar.mul(TFr[:, :, 0], sX1, 1.0 / IW)
        nc.scalar.mul(TFr[:, :, 1], sY1, 1.0 / IH)
        nc.scalar.mul(TFr[:, :, 2], sX2, 1.0 / IW)
        nc.scalar.mul(TFr[:, :, 3], sY2, 1.0 / IH)
        nc.vector.tensor_tensor(sW32[:], sX2, sX1, A.subtract)
        nc.vector.tensor_tensor(sH32[:], sY2, sY1, A.subtract)
        nc.vector.tensor_tensor(sCx[:], sX1, sX2, A.add)
        nc.vector.tensor_tensor(sCy[:], sY1, sY2, A.add)
        nc.scalar.mul(TFr[:, :, 4], sW32[:], 1.0 / IW)
        nc.scalar.mul(TFr[:, :, 5], sH32[:], 1.0 / IH)
        nc.scalar.mul(TFr[:, :, 6], sCx[:], 1.0 / IW)
        nc.scalar.mul(TFr[:, :, 7], sCy[:], 1.0 / IH)
        nc.scalar.mul(TFr[:, :, 18], sCx[:], 1.0 / IW)
        nc.scalar.mul(TFr[:, :, 19], sCy[:], 1.0 / IH)
        sT2 = sc.tile([128, JB], f32)
        nc.vector.tensor_tensor(sT2[:], sW32[:], sH32[:], A.mult)
        nc.scalar.mul(TFr[:, :, 8], sT2[:], 1.0 / (IW * IH))
        sHp = sT0
        nc.vector.tensor_scalar(sHp[:], sH32[:], 1e-6 * IH, 1.0 / IH, A.add, A.mult)
        sR = sT1
        nc.vector.reciprocal(sR[:], sHp[:])
        nc.vector.tensor_scalar(sT2[:], sW32[:], 1.0 / IW, None, A.mult)
        nc.vector.tensor_tensor(TFr[:, :, 9], sT2[:], sR[:], A.mult)
        for k in range(3):
            nc.vector.scalar_tensor_tensor(TFr[:, :, 10 + k], sCat, float(k),
                                           TFr[:, :, 14], A.is_equal, A.mult)
        nc.vector.tensor_tensor(TFr[:, :, 13], sConf, TFr[:, :, 14], A.mult)
        nc.scalar.activation(TFr[:, :, 15], TFr[:, :, 14],
                             AF.Identity, bias=1.0, scale=-1.0)
        # dist features: f16 = 0.5*sqrt(dx2^2+dy2^2)/IW-scaled, f17 = 1
        sDx = sc.tile([128, JD], f32)
        sDy = sc.tile([128, JD], f32)
        sCxp = sCx.rearrange("p (m g) -> p m g", g=2)
        sCyp = sCy.rearrange("p (m g) -> p m g", g=2)
        nc.vector.tensor_tensor(sDx[:], sCxp[:, :, 0], sCxp[:, :, 1], A.subtract)
        nc.vector.tensor_tensor(sDy[:], sCyp[:, :, 0], sCyp[:, :, 1], A.subtract)
        nc.vector.tensor_scalar(sDx[:], sDx[:], 1.0 / IW, None, A.mult)
        nc.vector.tensor_scalar(sDy[:], sDy[:], 1.0 / IH, None, A.mult)
        nc.vector.tensor_tensor(sDx[:], sDx[:], sDx[:], A.mult)
        nc.vector.tensor_tensor(sDy[:], sDy[:], sDy[:], A.mult)
        nc.vector.tensor_tensor(sDx[:], sDx[:], sDy[:], A.add)
        nc.scalar.activation(TFd[:, 0:JD, 16], sDx[:], AF.Sqrt, scale=0.25)

        # ---------------- P2: gram variance ----------------
        v = sc.tile([128, 160], f32)
        copy_rr = [nc.vector.tensor_copy, nc.scalar.copy]

        cta_d = cp.tile([128, NDC * 128], bf16)
        # prepack geom features f0..9 of all 156 slots contiguously; each
        # 128-col transpose window overlaps 8 cols into the next pack, which
        # land on zero rows of Gblk (harmless).
        gprep = sc.tile([128, 13 * 120 + 8], bf16)
        nc.vector.memset(gprep[:, 13 * 120:], 0.0)
        nc.vector.tensor_copy(
            gprep[:, 0:1560].rearrange("p (j f) -> p j f", f=10),
            TFb[:, 0:156, 0:10])

        for gi in range(NGP):
            s0 = 12 * gi
            src = TFb[:, s0:s0 + 12, 0:10]
            pst = zp.tile([128, 1024], bf16, tag="z", name="z")
            ps = pst[:, 0:128]
            nc.tensor.transpose(ps[:], gprep[:, 120 * gi:120 * gi + 128], idb)
            pk = gpck.tile([128, 128], bf16, tag="gp", name="gp")
            nc.scalar.copy(pk[:], ps[:])
            yt = zp.tile([128, 512], f32, tag="z", name="z")
            y = yt[:, 0:128]
            nc.tensor.matmul(y, pk[:], Gblk, start=True, stop=True)
            tmp = sc.tile([128, 120], f32, tag="gtmp", name="gtmp")
            nc.vector.tensor_tensor(tmp[:], src, y[:, 0:120], A.mult)
            nc.vector.tensor_reduce(v[:, s0:s0 + 12],
                                    tmp.rearrange("p (j f) -> p j f", f=10),
                                    mybir.AxisListType.X, A.add)
            for dc in range(int(gi * 19 / 13), int((gi + 1) * 19 / 13)):
                pst2 = zp.tile([128, 1024], bf16, tag="z", name="z")
                psd = pst2[:, 0:128]
                nc.tensor.transpose(psd[:], TFd[:, 4 * dc:4 * dc + 4, :], idb)
                copy_rr[dc % 2](cta_d[:, dc * 128:(dc + 1) * 128], psd[:])

        # ---------------- P3: rstd + feature scale + gelu preload --------
        eps = sc.tile([128, 1], f32)
        nc.vector.memset(eps[:], 1e-5)
        sd = sc.tile([128, 156], f32)
        rstd = sc.tile([128, 156], f32)
        nc.scalar.activation(sd[:], v[:, 0:156], AF.Sqrt,
                             bias=eps[:], scale=1.0 / DH)
        nc.vector.reciprocal(rstd[:], sd[:])
        # preload the Gelu ACT table off the critical path
        gjunk = sc.tile([128, 8], bf16)
        nc.scalar.activation(gjunk[:], sd[:, 0:8], AF.Gelu)
        # scale geometry features f0..9 by rstd (per token)
        for f in range(10):
            nc.vector.tensor_tensor(TFb[:, :, f], TFb[:, :, f], rstd[:], A.mult)

        # ---------------- P2c: dist W2 + staging ----------------
        dist_stage = {"tile": None, "fill": 0, "base": 0}
        vd = out_d[:, 0:600, :].rearrange("b (q r) d -> b q r d", q=8)

        def stage_dist(kd, o):
            if dist_stage["tile"] is None:
                dist_stage["tile"] = dstg.tile([128, 6 * D], bf16, tag="dstage",
                                               name="dstage")
                dist_stage["fill"] = 0
                dist_stage["base"] = kd
            fill = dist_stage["fill"]
            copy_rr[kd % 2](dist_stage["tile"][:, fill * D:(fill + 1) * D], o[:])
            dist_stage["fill"] = fill + 1
            if dist_stage["fill"] == 6 or kd == JD - 1:
                b0 = dist_stage["base"]
                gsz = dist_stage["fill"]
                nc.sync.dma_start(vd[:, :, b0:b0 + gsz, :],
                                  dist_stage["tile"][:, 0:gsz * D])
                dist_stage["tile"] = None


        # mid fillers: keep the PE busy across the rstd chain window.
        # lhsT reads the last dist chunk so the scheduler cannot hoist them
        # before the gram/dist phase.
        for _ in range(30):
            wps = opa.tile([128, D], f32, tag="oa", name="oa")
            nc.tensor.matmul(wps[:], cta_d[:, (NDC - 1) * 128:NDC * 128],
                             junkw[:], start=True, stop=True)

        # ---------------- P4/P5: box pipeline ----------------
        vb = out_d[:, 600:1800, :].rearrange("b (q r) d -> b q r d", q=8)
        oct_tiles = {}     # o -> sbuf [128, 256] bf16 (chunks 2o | 2o+1)
        ht_tiles = {}      # o -> list of 4 sbuf [128, 512] bf16 (per band)
        box_stage = {"tile": None, "fill": 0, "base": 0}
        ccnt = {"i": 0}

        def emit_pass2_half(o, ci):
            if ci == 0:
                oct_tiles[o] = octp.tile([128, 256], bf16, tag="oct", name="oct")
            t = oct_tiles[o]
            c = 2 * o + ci
            pst = zp.tile([128, 1024], bf16, tag="z", name="z")
            ps = pst[:, 0:128]
            nc.tensor.transpose(ps[:], TFb[:, 4 * c:4 * c + 4, :], idb)
            copy_rr[c % 2](t[:, ci * 128:(ci + 1) * 128], ps[:])

        z_banks = {}

        def emit_z_mm(o):
            rhs = oct_tiles[o]
            zbs = []
            for b in range(4):
                zb = zp.tile([128, 512], f32, tag="z", name="z")
                nc.tensor.matmul(zb[:, 0:256], w1b[b][0], rhs[:],
                                 start=True, stop=True)
                nc.tensor.matmul(zb[:, 256:512], w1b[b][1], rhs[:],
                                 start=True, stop=True)
                zbs.append(zb)
            z_banks[o] = zbs

        def emit_gelu(o):
            zbs = z_banks.pop(o)
            hts = []
            for b in range(4):
                ht = htp.tile([128, 512], bf16, tag="ht", name="ht")
                nc.scalar.activation(ht[:], zbs[b][:], AF.Gelu)
                hts.append(ht)
            ht_tiles[o] = hts

        def flush_box(last_k):
            b0 = box_stage["base"]
            gsz = box_stage["fill"]
            nc.sync.dma_start(vb[:, :, b0:b0 + gsz, :],
                              box_stage["tile"][:, 0:gsz * D])
            box_stage["tile"] = None

        def emit_w2(o):
            hts = ht_tiles.pop(o)
            cchunk = oct_tiles[o]
            for ci in range(2):
                c = 2 * o + ci
                for b in range(4):
                    s = 4 * c + b
                    if s >= JB:
                        continue
                    ht = hts[b]
                    cam = (s % 6) // 2
                    ot = opa.tile([128, D], f32, tag="oa", name="oa")
                    nc.tensor.matmul(ot[:], ht[:, ci * 128:ci * 128 + 128],
                                     w2hi, start=True, stop=False)
                    nc.tensor.matmul(ot[:], ht[:, 256 + ci * 128:256 + ci * 128 + 128],
                                     w2lo, start=False, stop=False)
                    nc.tensor.matmul(ot[:], cchunk[:, ci * 128:(ci + 1) * 128],
                                     w2xb[cam][b], start=False, stop=True)
                    if box_stage["tile"] is None:
                        box_stage["tile"] = bstg.tile([128, 8 * D], bf16,
                                                      tag="bstage", name="bstage")
                        box_stage["fill"] = 0
                        box_stage["base"] = s
                    csel = 0 if (ccnt["i"] % 8) in (0, 2, 3, 5, 6) else 1
                    off = s - box_stage["base"]
                    copy_rr[csel](
                        box_stage["tile"][:, off * D:(off + 1) * D], ot[:])
                    ccnt["i"] += 1
                    box_stage["fill"] += 1
                    if (box_stage["fill"] == 8 or s == JB - 1
                            or (s >= 144 and box_stage["fill"] == 4)):
                        flush_box(s)
            oct_tiles.pop(o)
            dc = o
            for b in range(4):
                kd = 4 * dc + b
                if kd >= JD:
                    continue
                od = opa.tile([128, D], f32, tag="oa", name="oa")
                nc.tensor.matmul(od[:], cta_d[:, dc * 128:(dc + 1) * 128],
                                 w2xb[0][b], start=True, stop=True)
                stage_dist(kd, od)

        for step in range(NOCT + 2):
            if step < NOCT:
                emit_pass2_half(step, 0)
            if 1 <= step <= NOCT:
                emit_z_mm(step - 1)
            if step < NOCT:
                emit_pass2_half(step, 1)
            if step >= 2:
                emit_w2(step - 2)
            if 1 <= step <= NOCT:
                emit_gelu(step - 1)

    nc.compile()
    return nc


def _prep_inputs(inputs):
    f32 = np.float32
    bf = ml_dtypes.bfloat16
    scale = float(np.asarray(inputs["scale"]))

    W1p = np.zeros((32, DH), f32)
    W1p[0:10] = np.asarray(inputs["geom_w1"], f32)
    W1p[6] *= 0.5
    W1p[7] *= 0.5
    W1p -= W1p.mean(axis=1, keepdims=True)

    w1b_cols = []
    for b in range(4):
        hi = np.zeros((128, 128), f32)
        hi[32 * b:32 * b + 32] = W1p[:, :128]
        lo = np.zeros((128, 128), f32)
        lo[32 * b:32 * b + 32] = W1p[:, 128:]
        w1b_cols += [hi, lo]

    W2s = scale * np.asarray(inputs["geom_w2"], f32)
    w2hi, w2lo = W2s[:128], W2s[128:]

    cat_t = np.asarray(inputs["cat_table"], f32)
    cam_t = np.asarray(inputs["cam_table"], f32)
    bias_row = (np.asarray(inputs["geom_b2"], f32)
                + np.asarray(inputs["conf_b"], f32)
                + np.asarray(inputs["center_b"], f32))
    w2xb_cols = []
    for c in range(3):
        W2X = np.zeros((32, D), f32)
        W2X[10:13] = scale * cat_t
        W2X[13] = scale * np.asarray(inputs["conf_w"], f32)[0]
        W2X[14] = scale * (bias_row + cam_t[c])
        W2X[15] = np.asarray(inputs["missing_emb"], f32)[0]
        W2X[16] = np.asarray(inputs["dist_w"], f32)[0]
        W2X[17] = np.asarray(inputs["dist_b"], f32)
        W2X[18] = scale * np.asarray(inputs["center_w"], f32)[0] * 0.5
        W2X[19] = scale * np.asarray(inputs["center_w"], f32)[1] * 0.5
        for b in range(4):
            t = np.zeros((128, D), f32)
            t[32 * b:32 * b + 32] = W2X
            w2xb_cols.append(t)

    G10 = (W1p[0:10] @ W1p[0:10].T).astype(f32)
    Gblk = np.zeros((128, 128), f32)
    for s in range(12):
        Gblk[10 * s:10 * s + 10, 10 * s:10 * s + 10] = G10

    idf32 = np.eye(128, dtype=f32)
    bpk = np.concatenate(w1b_cols + [w2hi, w2lo] + w2xb_cols + [Gblk, idf32],
                         axis=1).astype(bf)

    box = np.asarray(inputs["box_data"], f32)
    fpks = []
    for c in range(NCORES):
        rawc = box[c * BPC:(c + 1) * BPC].reshape(BPC, T * 6, 6)
        rawc = rawc.reshape(BPC, 8, JB, 6).reshape(128, 900)
        fpks.append(np.ascontiguousarray(
            np.concatenate([rawc, idf32], axis=1), dtype=f32))
    return fpks, bpk


def _fast_path_ok(inputs):
    try:
        shapes = {
            "box_data": (B, T, 6, 6), "cat_table": (3, D), "geom_w1": (10, DH),
            "geom_b1": (DH,), "ln_g": (DH,), "ln_b": (DH,), "geom_w2": (DH, D),
            "geom_b2": (D,), "conf_w": (1, D), "conf_b": (D,),
            "center_w": (2, D), "center_b": (D,), "missing_emb": (1, D),
            "dist_w": (1, D), "dist_b": (D,), "cam_table": (NCAM, D),
        }
        for k, s in shapes.items():
            if tuple(np.asarray(inputs[k]).shape) != s:
                return False
        if not np.all(np.asarray(inputs["geom_b1"]) == 0):
            return False
        if not np.all(np.asarray(inputs["ln_g"]) == 1):
            return False
        if not np.all(np.asarray(inputs["ln_b"]) == 0):
            return False
        return True
    except Exception:
        return False


def _numpy_fallback(inputs):
    import math
    f32 = np.float32
    inp = {k: np.asarray(v) for k, v in inputs.items()}
    coords = inp["box_data"][..., :4].astype(f32)
    category = inp["box_data"][..., 4].astype(np.int32)
    conf = inp["box_data"][..., 5].astype(f32)
    norm = np.array([IW, IH, IW, IH], f32)
    cn = (coords / norm).reshape(B, T, NCAM, NB, 4)
    category = category.reshape(B, T, NCAM, NB)
    conf = conf.reshape(B, T, NCAM, NB, 1)
    presence = (cn.sum(-1) != 0).astype(f32)
    sort_key = category.astype(f32) + (1.0 - presence) * 1000.0
    idx = np.argsort(sort_key, axis=-1, kind="stable")
    cn = np.take_along_axis(cn, idx[..., None], axis=-2)
    category = np.take_along_axis(category, idx, axis=-1)
    conf = np.take_along_axis(conf, idx[..., None], axis=-2)
    presence = (cn.sum(-1) != 0).astype(f32)[..., None]
    x1, y1, x2, y2 = cn[..., 0], cn[..., 1], cn[..., 2], cn[..., 3]
    w, h = x2 - x1, y2 - y1
    cx, cy = (x1 + x2) * 0.5, (y1 + y2) * 0.5
    area, aspect = w * h, w / (h + 1e-6)
    dx, dy = cx[..., 0] - cx[..., 1], cy[..., 0] - cy[..., 1]
    dist = np.sqrt(dx * dx + dy * dy)[..., None]
    dist_tok = dist @ inp["dist_w"].astype(f32) + inp["dist_b"].astype(f32)
    geom = np.stack([x1, y1, x2, y2, w, h, cx, cy, area, aspect], axis=-1)
    z = geom @ inp["geom_w1"].astype(f32) + inp["geom_b1"].astype(f32)
    mu = z.mean(-1, keepdims=True)
    var = ((z - mu) ** 2).mean(-1, keepdims=True)
    xh = (z - mu) / np.sqrt(var + 1e-5) * inp["ln_g"].astype(f32) + inp["ln_b"].astype(f32)
    try:
        from scipy.special import erf as _erf
        g = xh * 0.5 * (1.0 + _erf(xh / np.sqrt(2.0)))
    except Exception:
        verf = np.vectorize(math.erf)
        g = xh * 0.5 * (1.0 + verf(xh / np.sqrt(2.0)))
    geom_p = g @ inp["geom_w2"].astype(f32) + inp["geom_b2"].astype(f32)
    cat_emb = inp["cat_table"].astype(f32)[category]
    conf_p = conf @ inp["conf_w"].astype(f32) + inp["conf_b"].astype(f32)
    center_p = np.stack([cx, cy], axis=-1) @ inp["center_w"].astype(f32) + inp["center_b"].astype(f32)
    cam_emb = inp["cam_table"].astype(f32).reshape(1, 1, NCAM, 1, D)
    tok = (geom_p + cat_emb + conf_p + center_p + cam_emb) * float(inp["scale"])
    tok = np.where(presence == 0, inp["missing_emb"].astype(f32)[0], tok)
    out = np.concatenate([dist_tok.reshape(B, T * NCAM, D),
                          tok.reshape(B, T * NCAM * NB, D)], axis=1)
    return out.astype(np.float32)


def _run(inputs, trace=False, tmpdir=None):
    from concourse.bass_utils import run_bass_kernel_spmd

    if "nc" not in _CACHE:
        _CACHE["nc"] = _build_nc()
    nc = _CACHE["nc"]

    fpks, bpk = _prep_inputs(inputs)
    in_maps = [{"fpk": fpks[c], "bpk": bpk} for c in range(NCORES)]
    res = run_bass_kernel_spmd(nc, in_maps, list(range(NCORES)),
                               trace=trace, tmpdir=tmpdir)
    out = np.concatenate([np.asarray(res.results[c]["out"])
                          for c in range(NCORES)], axis=0)
    return out.astype(np.float32), res


def kernel(**inputs):
    if not _fast_path_ok(inputs):
        return _numpy_fallback(inputs)
    out, _ = _run(inputs)
    return out


if __name__ == "__main__":
    import reference as ref
    inputs = {k: np.asarray(v) for k, v in ref.setup_inputs().items()}
    got = kernel(**inputs)
    exp = np.load("/tmp/expected.npy")
    d = got - exp
    print("rel fro:", np.linalg.norm(d) / np.linalg.norm(exp))
    print("absmax rel:", np.abs(d).max() / np.abs(exp).max())


# revision 15
# speedup vs baseline: 1.2138x; 1.0154x over previous
"""Trainium2 Bass kernel v3 for nn_BoxEncoder (B=128, T=200, NC=3, NB=2, D=512, DH=256).

Data-parallel over batch: 16 batch items per core x 8 cores; partition
p = bt*8 + q.  Per partition: 75 dist tokens (+1 pad) and 150 box tokens
(+2 pad), processed as 4-slot transpose chunks.

v3 vs v2 (~2x): the whole kernel is built to keep the PE p-state warm
(cost model: matmuls run 2x faster once the PE has been ~continuously
busy for 3us; any long stall drops it back):
 - z is computed TRANSPOSED (weights-stationary): zT = W1bandedT @ cta_s,
   so gelu(zT) directly yields hT = the lhsT of the W2 matmuls.  The
   dma_start_transpose of h (SP-queue serial 1.24us each + 900ns sem) is
   gone - that chain caused the recurring 3-5us PE stalls in v2.
 - LN rstd is folded into the geometry features BEFORE the z matmul
   (f0..f9 *= rstd per token), so gelu needs no per-partition scale and
   batches [128,512] over 4 slots per call.  cx,cy are duplicated into
   f18,f19 (unscaled) for the center_w rows of the extras matmul.
 - all matmuls are full-K (no tile_position): banded *weights* (zero rows
   outside the slot's 32-band) instead of banded matmuls; the extras and
   z matmuls share one LDWEIGHTS of the feature chunk.
 - variance via a 12-slot x 10-feature gram pack (13 transposes instead
   of 38) and a single batched sqrt+reciprocal.
 - PE pre-warm: a dozen junk matmuls issued at t~1us keep the PE busy
   during the DVE feature-prep phase so the clock is warm when real
   matmuls start.
 - staging copies round-robin DVE/ACT/GPSIMD; out DMAs on the otherwise
   idle SP queue; one ACT table preload for Gelu right after the rstd
   sqrt so no table load lands inside the steady-state loop.
"""

import numpy as np
import ml_dtypes

B, T, NCAM, NB, D, DH = 128, 200, 3, 2, 512, 256
IW, IH = 640.0, 400.0
NCORES = 8
BPC = B // NCORES
JB = 150                  # real box slots per partition
JBP = 156                 # padded (38 chunks use 152; gram packs use 156)
JD = 75                   # real dist slots
JDP = 76                  # padded (19 chunks)
NCH = 38                  # box chunks
NDC = 19                  # dist chunks
NOCT = 19                 # box octs (2 chunks = 8 slots each; last has 6)
NGP = 13                  # gram packs (12 slots x 10 feats)

_CACHE = {}


def _build_nc():
    from contextlib import ExitStack
    import concourse.bacc as bacc
    import concourse.mybir as mybir
    import concourse.tile as tile

    f32 = mybir.dt.float32
    bf16 = mybir.dt.bfloat16
    A = mybir.AluOpType
    AF = mybir.ActivationFunctionType

    # bpk bf16 column offsets
    C_W1B = 0                       # 8 x 128 (band b: hi, lo)
    C_W2HI = C_W1B + 8 * 128
    C_W2LO = C_W2HI + 512
    C_W2XB = C_W2LO + 512           # 12 x 512 (cam c, band b)
    C_G = C_W2XB + 12 * 512
    C_ID = C_G + 128
    NBF = C_ID + 128
    C_P1 = C_W2XB            # part 1 = w1b, w2hi, w2lo (cols 0..C_W2XB)


    nc = bacc.Bacc("TRN2", target_bir_lowering=False, debug=False,
                   num_devices=NCORES)
    fpk = nc.declare_dram_parameter("fpk", [128, 900 + 128], f32, isOutput=False)
    bpk = nc.declare_dram_parameter("bpk", [128, NBF], bf16, isOutput=False)
    out_d = nc.declare_dram_parameter("out", [BPC, 1800, D], bf16, isOutput=True)

    with ExitStack() as ctx:
        tc = ctx.enter_context(tile.TileContext(nc))
        cp = ctx.enter_context(tc.tile_pool(name="const", bufs=1))
        sc = ctx.enter_context(tc.tile_pool(name="scratch", bufs=1))
        # PSUM pools (8 banks): zp 4x[128,512]=4 + opa 3x[128,512]=3 +
        # tp 2x[128,128]=0.5
        zp = ctx.enter_context(tc.tile_pool(name="zp", bufs=4, space="PSUM"))
        opa = ctx.enter_context(tc.tile_pool(name="opa", bufs=4, space="PSUM"))
        gpck = ctx.enter_context(tc.tile_pool(name="gpck", bufs=2))
        octp = ctx.enter_context(tc.tile_pool(name="octp", bufs=4))
        cdp = ctx.enter_context(tc.tile_pool(name="cdp", bufs=1))
        htp = ctx.enter_context(tc.tile_pool(name="htp", bufs=8))
        bstg = ctx.enter_context(tc.tile_pool(name="bstage", bufs=3))
        dstg = ctx.enter_context(tc.tile_pool(name="dstage", bufs=3))

        fpack = cp.tile([128, 900 + 128], f32)
        nc.sync.dma_start(fpack[:], fpk[:])
        bpack = cp.tile([128, NBF], bf16)
        # split the weight load: small part (warm-up, gram, z) first so the
        # big w2xb block (12KB/partition) doesn't gate the early phases
        nc.sync.dma_start(bpack[:, C_G:NBF], bpk[:, C_G:NBF])
        nc.sync.dma_start(bpack[:, 0:C_P1], bpk[:, 0:C_P1])
        nc.sync.dma_start(bpack[:, C_P1:C_G], bpk[:, C_P1:C_G])

        raw = fpack[:, 0:900]
        idf = fpack[:, 900:1028]
        w1b = [(bpack[:, C_W1B + (2 * b) * 128: C_W1B + (2 * b + 1) * 128],
                bpack[:, C_W1B + (2 * b + 1) * 128: C_W1B + (2 * b + 2) * 128])
               for b in range(4)]
        w2hi = bpack[:, C_W2HI:C_W2HI + 512]
        w2lo = bpack[:, C_W2LO:C_W2LO + 512]
        w2xb = [[bpack[:, C_W2XB + (c * 4 + b) * 512: C_W2XB + (c * 4 + b + 1) * 512]
                 for b in range(4)] for c in range(3)]
        Gblk = bpack[:, C_G:C_G + 128]
        idb = bpack[:, C_ID:C_ID + 128]

        # ---------------- PE pre-warm: junk matmuls on memset tiles -----
        # (independent of the input DMAs so the PE busies from ~0.5us)
        junkw = cp.tile([128, 512], bf16)
        nc.vector.memset(junkw[:], 0.25)
        for _ in range(70):
            wps = opa.tile([128, D], f32, tag="oa", name="oa")
            nc.tensor.matmul(wps[:], junkw[:, 0:128], junkw[:],
                             start=True, stop=True)

        # ---------------- P1: feature planes ----------------
        TFB = cp.tile([128, JBP * 32], bf16)
        TFD = cp.tile([128, JDP * 32], bf16)
        TFb = TFB.rearrange("p (j f) -> p j f", f=32)
        TFd = TFD.rearrange("p (j f) -> p j f", f=32)
        # zeros: box f16,17 + f20..31 + pad slots; dist all but f16,f17
        # zeros via ACT (uint32-bitcast x0 is NaN-safe); tiny pads + the 1.0
        # fill on DVE.  Pad slots' f0..9 must precede the gram prepack.
        nc.vector.memset(TFb[:, JB:JBP, 0:16], 0.0)
        nc.vector.memset(TFb[:, JB:JBP, 18:20], 0.0)
        nc.scalar.memzero(TFb[:, :, 16:18])
        nc.scalar.memzero(TFb[:, :, 20:32])
        nc.scalar.memzero(TFd[:, :, 0:16])
        nc.scalar.memzero(TFd[:, :, 18:32])
        nc.vector.memset(TFd[:, :, 17], 1.0)
        nc.vector.memset(TFd[:, JD:JDP, 16], 0.0)

        TFr = TFb[:, 0:JB, :]
        raw6 = raw.rearrange("p (b s) -> p b s", s=6)
        rawp = raw.rearrange("p (m g s) -> p m g s", g=2, s=6)
        TFbp = TFB[:, 0:JB * 32].rearrange("p (m g f) -> p m g f", g=2, f=32)

        sPres = sc.tile([128, JB], f32)
        sKey = sc.tile([128, JB], f32)
        sSwap = sc.tile([128, JD], f32)
        sD = sc.tile([128, JD], f32)
        sSD = sc.tile([128, JD], f32)
        sT0 = sc.tile([128, JB], f32)
        sT1 = sc.tile([128, JB], f32)

        nc.vector.tensor_tensor(sT0[:], raw6[:, :, 0], raw6[:, :, 1], A.add)
        nc.vector.tensor_tensor(sT1[:], raw6[:, :, 2], raw6[:, :, 3], A.add)
        nc.vector.tensor_tensor(sT0[:], sT0[:], sT1[:], A.add)
        nc.vector.tensor_scalar(sPres[:], sT0[:], 0.0, None, A.not_equal)
        nc.vector.scalar_tensor_tensor(sKey[:], sPres[:], -1000.0,
                                       raw6[:, :, 4], A.mult, A.add)
        sKeyp = sKey.rearrange("p (m g) -> p m g", g=2)
        nc.vector.tensor_tensor(sSwap[:], sKeyp[:, :, 1], sKeyp[:, :, 0], A.is_lt)

        # block compare-and-swap: all 6 raw components in 4 DVE ops
        sRaw = sc.tile([128, JD, 2, 6], f32)
        sD6 = sc.tile([128, JD, 6], f32)
        swb = sSwap[:].unsqueeze(-1).broadcast_to([128, JD, 6])
        nc.vector.tensor_tensor(sD6[:], rawp[:, :, 1, :], rawp[:, :, 0, :],
                                A.subtract)
        nc.vector.tensor_tensor(sD6[:], sD6[:], swb, A.mult)
        nc.vector.tensor_tensor(sRaw[:, :, 0, :], rawp[:, :, 0, :], sD6[:], A.add)
        nc.vector.tensor_tensor(sRaw[:, :, 1, :], rawp[:, :, 1, :], sD6[:],
                                A.subtract)
        sPresP = sPres.rearrange("p (m g) -> p m g", g=2)
        nc.vector.tensor_tensor(sD[:], sPresP[:, :, 1], sPresP[:, :, 0], A.subtract)
        nc.vector.tensor_tensor(sSD[:], sD[:], sSwap[:], A.mult)
        nc.vector.tensor_tensor(TFbp[:, :, 0, 14], sPresP[:, :, 0], sSD[:], A.add)
        nc.vector.tensor_tensor(TFbp[:, :, 1, 14], sPresP[:, :, 1], sSD[:], A.subtract)

        sRw = sRaw.rearrange("p m g s -> p (m g) s")
        sX1, sY1, sX2, sY2 = (sRw[:, :, i] for i in range(4))
        sCat, sConf = sRw[:, :, 4], sRw[:, :, 5]
        # all derived geometry in f32 scratch (bf16-rounded coords would
        # catastrophically cancel in w/h near zero -> aspect blows up)
        sW32 = sc.tile([128, JB], f32)
        sH32 = sc.tile([128, JB], f32)
        sCx = sc.tile([128, JB], f32)
        sCy = sc.tile([128, JB], f32)
        nc.scalar.mul(TFr[:, :, 0], sX1, 1.0 / IW)
        nc.scalar.mul(TFr[:, :, 1], sY1, 1.0 / IH)
        nc.scalar.mul(TFr[:, :, 2], sX2, 1.0 / IW)
        nc.scalar.mul(TFr[:, :, 3], sY2, 1.0 / IH)
        nc.vector.tensor_tensor(sW32[:], sX2, sX1, A.subtract)
        nc.vector.tensor_tensor(sH32[:], sY2, sY1, A.subtract)
        nc.vector.tensor_tensor(sCx[:], sX1, sX2, A.add)
        nc.vector.tensor_tensor(sCy[:], sY1, sY2, A.add)
        nc.scalar.mul(TFr[:, :, 4], sW32[:], 1.0 / IW)
        nc.scalar.mul(TFr[:, :, 5], sH32[:], 1.0 / IH)
        nc.scalar.mul(TFr[:, :, 6], sCx[:], 1.0 / IW)
        nc.scalar.mul(TFr[:, :, 7], sCy[:], 1.0 / IH)
        nc.scalar.mul(TFr[:, :, 18], sCx[:], 1.0 / IW)
        nc.scalar.mul(TFr[:, :, 19], sCy[:], 1.0 / IH)
        sT2 = sc.tile([128, JB], f32)
        nc.vector.tensor_tensor(sT2[:], sW32[:], sH32[:], A.mult)
        nc.scalar.mul(TFr[:, :, 8], sT2[:], 1.0 / (IW * IH))
        sHp = sT0
        nc.vector.tensor_scalar(sHp[:], sH32[:], 1e-6 * IH, 1.0 / IH, A.add, A.mult)
        sR = sT1
        nc.vector.reciprocal(sR[:], sHp[:])
        nc.vector.tensor_scalar(sT2[:], sW32[:], 1.0 / IW, None, A.mult)
        nc.vector.tensor_tensor(TFr[:, :, 9], sT2[:], sR[:], A.mult)
        for k in range(3):
            nc.vector.scalar_tensor_tensor(TFr[:, :, 10 + k], sCat, float(k),
                                           TFr[:, :, 14], A.is_equal, A.mult)
        nc.vector.tensor_tensor(TFr[:, :, 13], sConf, TFr[:, :, 14], A.mult)
        nc.scalar.activation(TFr[:, :, 15], TFr[:, :, 14],
                             AF.Identity, bias=1.0, scale=-1.0)
        # dist features: f16 = 0.5*sqrt(dx2^2+dy2^2)/IW-scaled, f17 = 1
        sDx = sc.tile([128, JD], f32)
        sDy = sc.tile([128, JD], f32)
        sCxp = sCx.rearrange("p (m g) -> p m g", g=2)
        sCyp = sCy.rearrange("p (m g) -> p m g", g=2)
        nc.vector.tensor_tensor(sDx[:], sCxp[:, :, 0], sCxp[:, :, 1], A.subtract)
        nc.vector.tensor_tensor(sDy[:], sCyp[:, :, 0], sCyp[:, :, 1], A.subtract)
        nc.vector.tensor_scalar(sDx[:], sDx[:], 1.0 / IW, None, A.mult)
        nc.vector.tensor_scalar(sDy[:], sDy[:], 1.0 / IH, None, A.mult)
        nc.vector.tensor_tensor(sDx[:], sDx[:], sDx[:], A.mult)
        nc.vector.tensor_tensor(sDy[:], sDy[:], sDy[:], A.mult)
        nc.vector.tensor_tensor(sDx[:], sDx[:], sDy[:], A.add)
        nc.scalar.activation(TFd[:, 0:JD, 16], sDx[:], AF.Sqrt, scale=0.25)

        # ---------------- P2: gram variance ----------------
        v = sc.tile([128, 160], f32)
        copy_rr = [nc.vector.tensor_copy, nc.scalar.copy]

        cta_d = cp.tile([128, NDC * 128], bf16)
        # prepack geom features f0..9 of all 156 slots contiguously; each
        # 128-col transpose window overlaps 8 cols into the next pack, which
        # land on zero rows of Gblk (harmless).
        gprep = sc.tile([128, 13 * 120 + 8], bf16)
        nc.vector.memset(gprep[:, 13 * 120:], 0.0)
        nc.vector.tensor_copy(
            gprep[:, 0:1560].rearrange("p (j f) -> p j f", f=10),
            TFb[:, 0:156, 0:10])

        # software-pipelined by one iteration: gram-mm(gi-1) issues after
        # transpose(gi), so the in-order PE queue never head-blocks on the
        # pk copy.  Dist transposes interleave as extra PE filler.
        pks = {}
        for gi in range(NGP + 1):
            if gi < NGP:
                pst = zp.tile([128, 1024], bf16, tag="z", name="z")
                ps = pst[:, 0:128]
                nc.tensor.transpose(ps[:], gprep[:, 120 * gi:120 * gi + 128], idb)
                pk = gpck.tile([128, 128], bf16, tag="gp", name="gp")
                nc.scalar.copy(pk[:], ps[:])
                pks[gi] = pk
            for dc in range(int(gi * 19 / 14), int((gi + 1) * 19 / 14)):
                pst2 = zp.tile([128, 1024], bf16, tag="z", name="z")
                psd = pst2[:, 0:128]
                nc.tensor.transpose(psd[:], TFd[:, 4 * dc:4 * dc + 4, :], idb)
                copy_rr[dc % 2](cta_d[:, dc * 128:(dc + 1) * 128], psd[:])
            if gi >= 1:
                gj = gi - 1
                s0 = 12 * gj
                src = TFb[:, s0:s0 + 12, 0:10]
                yt = zp.tile([128, 512], f32, tag="z", name="z")
                y = yt[:, 0:128]
                nc.tensor.matmul(y, pks.pop(gj)[:], Gblk, start=True, stop=True)
                tmp = sc.tile([128, 120], f32, tag="gtmp", name="gtmp")
                nc.vector.tensor_tensor(tmp[:], src, y[:, 0:120], A.mult)
                nc.vector.tensor_reduce(v[:, s0:s0 + 12],
                                        tmp.rearrange("p (j f) -> p j f", f=10),
                                        mybir.AxisListType.X, A.add)

        # ---------------- P3: rstd + feature scale + gelu preload --------
        eps = sc.tile([128, 1], f32)
        nc.vector.memset(eps[:], 1e-5)
        sd = sc.tile([128, 156], f32)
        rstd = sc.tile([128, 156], f32)
        nc.scalar.activation(sd[:], v[:, 0:156], AF.Sqrt,
                             bias=eps[:], scale=1.0 / DH)
        nc.vector.reciprocal_approx_fast(rstd[:], sd[:])
        # preload the Gelu ACT table off the critical path
        gjunk = sc.tile([128, 8], bf16)
        nc.scalar.activation(gjunk[:], sd[:, 0:8], AF.Gelu)
        # scale geometry features f0..9 by rstd (per token)
        for f in range(10):
            nc.vector.tensor_tensor(TFb[:, :, f], TFb[:, :, f], rstd[:], A.mult)

        # ---------------- P2c: dist W2 + staging ----------------
        dist_stage = {"tile": None, "fill": 0, "base": 0}
        vd = out_d[:, 0:600, :].rearrange("b (q r) d -> b q r d", q=8)

        def stage_dist(kd, o):
            if dist_stage["tile"] is None:
                dist_stage["tile"] = dstg.tile([128, 6 * D], bf16, tag="dstage",
                                               name="dstage")
                dist_stage["fill"] = 0
                dist_stage["base"] = kd
            fill = dist_stage["fill"]
            copy_rr[kd % 2](dist_stage["tile"][:, fill * D:(fill + 1) * D], o[:])
            dist_stage["fill"] = fill + 1
            if dist_stage["fill"] == 6 or kd == JD - 1:
                b0 = dist_stage["base"]
                gsz = dist_stage["fill"]
                nc.sync.dma_start(vd[:, :, b0:b0 + gsz, :],
                                  dist_stage["tile"][:, 0:gsz * D])
                dist_stage["tile"] = None


        # mid fillers: keep the PE busy across the rstd chain window.
        # lhsT reads the last dist chunk so the scheduler cannot hoist them
        # before the gram/dist phase.
        for _ in range(30):
            wps = opa.tile([128, D], f32, tag="oa", name="oa")
            nc.tensor.matmul(wps[:], cta_d[:, (NDC - 1) * 128:NDC * 128],
                             junkw[:], start=True, stop=True)

        # ---------------- P4/P5: box pipeline ----------------
        vb = out_d[:, 600:1800, :].rearrange("b (q r) d -> b q r d", q=8)
        oct_tiles = {}     # o -> sbuf [128, 256] bf16 (chunks 2o | 2o+1)
        ht_tiles = {}      # o -> list of 4 sbuf [128, 512] bf16 (per band)
        box_stage = {"tile": None, "fill": 0, "base": 0}
        ccnt = {"i": 0}

        def emit_pass2_half(o, ci):
            if ci == 0:
                oct_tiles[o] = octp.tile([128, 256], bf16, tag="oct", name="oct")
            t = oct_tiles[o]
            c = 2 * o + ci
            pst = zp.tile([128, 1024], bf16, tag="z", name="z")
            ps = pst[:, 0:128]
            nc.tensor.transpose(ps[:], TFb[:, 4 * c:4 * c + 4, :], idb)
            copy_rr[c % 2](t[:, ci * 128:(ci + 1) * 128], ps[:])

        z_banks = {}

        def emit_z_mm(o):
            rhs = oct_tiles[o]
            zbs = []
            for b in range(4):
                zb = zp.tile([128, 512], f32, tag="z", name="z")
                nc.tensor.matmul(zb[:, 0:256], w1b[b][0], rhs[:],
                                 start=True, stop=True)
                nc.tensor.matmul(zb[:, 256:512], w1b[b][1], rhs[:],
                                 start=True, stop=True)
                zbs.append(zb)
            z_banks[o] = zbs

        def emit_gelu(o):
            zbs = z_banks.pop(o)
            hts = []
            for b in range(4):
                ht = htp.tile([128, 512], bf16, tag="ht", name="ht")
                nc.scalar.activation(ht[:], zbs[b][:], AF.Gelu)
                hts.append(ht)
            ht_tiles[o] = hts

        def flush_box(last_k):
            b0 = box_stage["base"]
            gsz = box_stage["fill"]
            nc.sync.dma_start(vb[:, :, b0:b0 + gsz, :],
                              box_stage["tile"][:, 0:gsz * D])
            box_stage["tile"] = None

        def emit_w2(o):
            hts = ht_tiles.pop(o)
            cchunk = oct_tiles[o]
            for ci in range(2):
                c = 2 * o + ci
                for b in range(4):
                    s = 4 * c + b
                    if s >= JB:
                        continue
                    ht = hts[b]
                    cam = (s % 6) // 2
                    ot = opa.tile([128, D], f32, tag="oa", name="oa")
                    nc.tensor.matmul(ot[:], ht[:, ci * 128:ci * 128 + 128],
                                     w2hi, start=True, stop=False)
                    nc.tensor.matmul(ot[:], ht[:, 256 + ci * 128:256 + ci * 128 + 128],
                                     w2lo, start=False, stop=False)
                    nc.tensor.matmul(ot[:], cchunk[:, ci * 128:(ci + 1) * 128],
                                     w2xb[cam][b], start=False, stop=True)
                    if box_stage["tile"] is None:
                        box_stage["tile"] = bstg.tile([128, 8 * D], bf16,
                                                      tag="bstage", name="bstage")
                        box_stage["fill"] = 0
                        box_stage["base"] = s
                    csel = 0 if (ccnt["i"] % 8) in (0, 2, 3, 5, 6) else 1
                    off = s - box_stage["base"]
                    copy_rr[csel](
                        box_stage["tile"][:, off * D:(off + 1) * D], ot[:])
                    ccnt["i"] += 1
                    box_stage["fill"] += 1
                    if (box_stage["fill"] == 8 or s == JB - 1
                            or (s >= 144 and box_stage["fill"] == 4)):
                        flush_box(s)
            oct_tiles.pop(o)
            dc = o
            for b in range(4):
                kd = 4 * dc + b
                if kd >= JD:
                    continue
                od = opa.tile([128, D], f32, tag="oa", name="oa")
                nc.tensor.matmul(od[:], cta_d[:, dc * 128:(dc + 1) * 128],
                                 w2xb[0][b], start=True, stop=True)
                stage_dist(kd, od)

        for step in range(NOCT + 2):
            if step < NOCT:
                emit_pass2_half(step, 0)
            if 1 <= step <= NOCT:
                emit_z_mm(step - 1)
            if step < NOCT:
                emit_pass2_half(step, 1)
            if step >= 2:
                emit_w2(step - 2)
            if 1 <= step <= NOCT:
                emit_gelu(step - 1)

    nc.compile()
    return nc


def _prep_inputs(inputs):
    f32 = np.float32
    bf = ml_dtypes.bfloat16
    scale = float(np.asarray(inputs["scale"]))

    W1p = np.zeros((32, DH), f32)
    W1p[0:10] = np.asarray(inputs["geom_w1"], f32)
    W1p[6] *= 0.5
    W1p[7] *= 0.5
    W1p -= W1p.mean(axis=1, keepdims=True)

    w1b_cols = []
    for b in range(4):
        hi = np.zeros((128, 128), f32)
        hi[32 * b:32 * b + 32] = W1p[:, :128]
        lo = np.zeros((128, 128), f32)
        lo[32 * b:32 * b + 32] = W1p[:, 128:]
        w1b_cols += [hi, lo]

    W2s = scale * np.asarray(inputs["geom_w2"], f32)
    w2hi, w2lo = W2s[:128], W2s[128:]

    cat_t = np.asarray(inputs["cat_table"], f32)
    cam_t = np.asarray(inputs["cam_table"], f32)
    bias_row = (np.asarray(inputs["geom_b2"], f32)
                + np.asarray(inputs["conf_b"], f32)
                + np.asarray(inputs["center_b"], f32))
    w2xb_cols = []
    for c in range(3):
        W2X = np.zeros((32, D), f32)
        W2X[10:13] = scale * cat_t
        W2X[13] = scale * np.asarray(inputs["conf_w"], f32)[0]
        W2X[14] = scale * (bias_row + cam_t[c])
        W2X[15] = np.asarray(inputs["missing_emb"], f32)[0]
        W2X[16] = np.asarray(inputs["dist_w"], f32)[0]
        W2X[17] = np.asarray(inputs["dist_b"], f32)
        W2X[18] = scale * np.asarray(inputs["center_w"], f32)[0] * 0.5
        W2X[19] = scale * np.asarray(inputs["center_w"], f32)[1] * 0.5
        for b in range(4):
            t = np.zeros((128, D), f32)
            t[32 * b:32 * b + 32] = W2X
            w2xb_cols.append(t)

    G10 = (W1p[0:10] @ W1p[0:10].T).astype(f32)
    Gblk = np.zeros((128, 128), f32)
    for s in range(12):
        Gblk[10 * s:10 * s + 10, 10 * s:10 * s + 10] = G10

    idf32 = np.eye(128, dtype=f32)
    bpk = np.concatenate(w1b_cols + [w2hi, w2lo] + w2xb_cols + [Gblk, idf32],
                         axis=1).astype(bf)

    box = np.asarray(inputs["box_data"], f32)
    fpks = []
    for c in range(NCORES):
        rawc = box[c * BPC:(c + 1) * BPC].reshape(BPC, T * 6, 6)
        rawc = rawc.reshape(BPC, 8, JB, 6).reshape(128, 900)
        fpks.append(np.ascontiguousarray(
            np.concatenate([rawc, idf32], axis=1), dtype=f32))
    return fpks, bpk


def _fast_path_ok(inputs):
    try:
        shapes = {
            "box_data": (B, T, 6, 6), "cat_table": (3, D), "geom_w1": (10, DH),
            "geom_b1": (DH,), "ln_g": (DH,), "ln_b": (DH,), "geom_w2": (DH, D),
            "geom_b2": (D,), "conf_w": (1, D), "conf_b": (D,),
            "center_w": (2, D), "center_b": (D,), "missing_emb": (1, D),
            "dist_w": (1, D), "dist_b": (D,), "cam_table": (NCAM, D),
        }
        for k, s in shapes.items():
            if tuple(np.asarray(inputs[k]).shape) != s:
                return False
        if not np.all(np.asarray(inputs["geom_b1"]) == 0):
            return False
        if not np.all(np.asarray(inputs["ln_g"]) == 1):
            return False
        if not np.all(np.asarray(inputs["ln_b"]) == 0):
            return False
        return True
    except Exception:
        return False


def _numpy_fallback(inputs):
    import math
    f32 = np.float32
    inp = {k: np.asarray(v) for k, v in inputs.items()}
    coords = inp["box_data"][..., :4].astype(f32)
    category = inp["box_data"][..., 4].astype(np.int32)
    conf = inp["box_data"][..., 5].astype(f32)
    norm = np.array([IW, IH, IW, IH], f32)
    cn = (coords / norm).reshape(B, T, NCAM, NB, 4)
    category = category.reshape(B, T, NCAM, NB)
    conf = conf.reshape(B, T, NCAM, NB, 1)
    presence = (cn.sum(-1) != 0).astype(f32)
    sort_key = category.astype(f32) + (1.0 - presence) * 1000.0
    idx = np.argsort(sort_key, axis=-1, kind="stable")
    cn = np.take_along_axis(cn, idx[..., None], axis=-2)
    category = np.take_along_axis(category, idx, axis=-1)
    conf = np.take_along_axis(conf, idx[..., None], axis=-2)
    presence = (cn.sum(-1) != 0).astype(f32)[..., None]
    x1, y1, x2, y2 = cn[..., 0], cn[..., 1], cn[..., 2], cn[..., 3]
    w, h = x2 - x1, y2 - y1
    cx, cy = (x1 + x2) * 0.5, (y1 + y2) * 0.5
    area, aspect = w * h, w / (h + 1e-6)
    dx, dy = cx[..., 0] - cx[..., 1], cy[..., 0] - cy[..., 1]
    dist = np.sqrt(dx * dx + dy * dy)[..., None]
    dist_tok = dist @ inp["dist_w"].astype(f32) + inp["dist_b"].astype(f32)
    geom = np.stack([x1, y1, x2, y2, w, h, cx, cy, area, aspect], axis=-1)
    z = geom @ inp["geom_w1"].astype(f32) + inp["geom_b1"].astype(f32)
    mu = z.mean(-1, keepdims=True)
    var = ((z - mu) ** 2).mean(-1, keepdims=True)
    xh = (z - mu) / np.sqrt(var + 1e-5) * inp["ln_g"].astype(f32) + inp["ln_b"].astype(f32)
    try:
        from scipy.special import erf as _erf
        g = xh * 0.5 * (1.0 + _erf(xh / np.sqrt(2.0)))
    except Exception:
        verf = np.vectorize(math.erf)
        g = xh * 0.5 * (1.0 + verf(xh / np.sqrt(2.0)))
    geom_p = g @ inp["geom_w2"].astype(f32) + inp["geom_b2"].astype(f32)
    cat_emb = inp["cat_table"].astype(f32)[category]
    conf_p = conf @ inp["conf_w"].astype(f32) + inp["conf_b"].astype(f32)
    center_p = np.stack([cx, cy], axis=-1) @ inp["center_w"].astype(f32) + inp["center_b"].astype(f32)
    cam_emb = inp["cam_table"].astype(f32).reshape(1, 1, NCAM, 1, D)
    tok = (geom_p + cat_emb + conf_p + center_p + cam_emb) * float(inp["scale"])
    tok = np.where(presence == 0, inp["missing_emb"].astype(f32)[0], tok)
    out = np.concatenate([dist_tok.reshape(B, T * NCAM, D),
                          tok.reshape(B, T * NCAM * NB, D)], axis=1)
    return out.astype(np.float32)


def _run(inputs, trace=False, tmpdir=None):
    from concourse.bass_utils import run_bass_kernel_spmd

    if "nc" not in _CACHE:
        _CACHE["nc"] = _build_nc()
    nc = _CACHE["nc"]

    fpks, bpk = _prep_inputs(inputs)
    in_maps = [{"fpk": fpks[c], "bpk": bpk} for c in range(NCORES)]
    res = run_bass_kernel_spmd(nc, in_maps, list(range(NCORES)),
                               trace=trace, tmpdir=tmpdir)
    out = np.concatenate([np.asarray(res.results[c]["out"])
                          for c in range(NCORES)], axis=0)
    return out.astype(np.float32), res


def kernel(**inputs):
    if not _fast_path_ok(inputs):
        return _numpy_fallback(inputs)
    out, _ = _run(inputs)
    return out


if __name__ == "__main__":
    import reference as ref
    inputs = {k: np.asarray(v) for k, v in ref.setup_inputs().items()}
    got = kernel(**inputs)
    exp = np.load("/tmp/expected.npy")
    d = got - exp
    print("rel fro:", np.linalg.norm(d) / np.linalg.norm(exp))
    print("absmax rel:", np.abs(d).max() / np.abs(exp).max())


# revision 16
# speedup vs baseline: 1.2170x; 1.0027x over previous
"""Trainium2 Bass kernel (v3.5) for nn_BoxEncoder (B=128, T=200, NC=3, NB=2, D=512, DH=256).

Data-parallel over batch: 16 batch items per core x 8 cores; partition
p = bt*8 + q owns 150 box tokens (+2 pad) and 75 dist tokens (+1 pad).
HW exec ~200us vs the 319us v2 baseline (cost-model sim: matmuls run 2x
faster once the PE has been ~continuously busy; every structure below
exists to keep the PE p-state warm and the copy engines off the
critical path).

 - z is computed TRANSPOSED (weights-stationary, banded W1 with zero
   rows outside each slot's 32-band): gelu(zT) directly yields hT = the
   lhsT of the W2 matmuls.  No h transpose exists at all (v2 spent 62us
   of SP-queue time + the stalls on dma_start_transpose).
 - LN rstd is folded into the geometry features BEFORE the z matmul, so
   gelu is a plain [128,512] call; cx,cy are duplicated unscaled into
   f18,f19 for the extras matmul (W2X rows 18,19).
 - all matmuls full-K, no tile_position: banded weights instead.  The
   extras matmul shares the feature-chunk stationary with z.
 - derived geometry (w,h,cx,cy,area,aspect) is computed in f32 scratch
   and only then written to the bf16 feature tile: bf16-first cancels
   catastrophically in aspect = w/(h+1e-6) near h=0.
 - variance via a 12-slot x 10-feature gram pack (13 bf16 transposes),
   software-pipelined by one iteration so the in-order PE queue never
   head-blocks on the pk copy; block compare-and-swap does all 6 box
   components in 4 DVE ops (stride-0 broadcast of the swap flag).
 - dist tokens are folded into the main loop (one 4-slot dist chunk per
   oct) because a standalone dist phase is copy-bound (75 PSUM->bf16
   copies vs only 16us of PE work).
 - PE pre-warm: ~70 junk matmuls on memset tiles cover the DVE-only P1
   phase; 30 more (anchored on the last dist transpose so the scheduler
   cannot hoist them) cover the sqrt->reciprocal->scale window.
 - gelus are issued AFTER the previous oct's staging copies on the ACT
   queue (copies otherwise wait ~2.7us behind 4 gelus and stall the PE
   through the 4-deep PSUM out-tile rotation); staging copies split 5:3
   DVE:ACT; GPSIMD is avoided entirely (it shares the SBUF write port
   with DVE in the cost model).
 - memsets via ACT uint32 memzero (NaN-safe), fpk/bpk loads split so
   the small weights (warm-up, gram, z) land first, out staged bf16
   with the host converting to f32 (~0.2% extra rel err; budget 2e-2).

Measured: HW exec 199840 ns, rel fro err 0.0042 (budget 2e-2).
"""

import numpy as np
import ml_dtypes

B, T, NCAM, NB, D, DH = 128, 200, 3, 2, 512, 256
IW, IH = 640.0, 400.0
NCORES = 8
BPC = B // NCORES
JB = 150                  # real box slots per partition
JBP = 156                 # padded (38 chunks use 152; gram packs use 156)
JD = 75                   # real dist slots
JDP = 76                  # padded (19 chunks)
NCH = 38                  # box chunks
NDC = 19                  # dist chunks
NOCT = 19                 # box octs (2 chunks = 8 slots each; last has 6)
NGP = 13                  # gram packs (12 slots x 10 feats)

_CACHE = {}


def _build_nc():
    from contextlib import ExitStack
    import concourse.bacc as bacc
    import concourse.mybir as mybir
    import concourse.tile as tile

    f32 = mybir.dt.float32
    bf16 = mybir.dt.bfloat16
    A = mybir.AluOpType
    AF = mybir.ActivationFunctionType

    # bpk bf16 column offsets
    C_W1B = 0                       # 8 x 128 (band b: hi, lo)
    C_W2HI = C_W1B + 8 * 128
    C_W2LO = C_W2HI + 512
    C_W2XB = C_W2LO + 512           # 12 x 512 (cam c, band b)
    C_G = C_W2XB + 12 * 512
    C_ID = C_G + 128
    NBF = C_ID + 128
    C_P1 = C_W2XB            # part 1 = w1b, w2hi, w2lo (cols 0..C_W2XB)


    nc = bacc.Bacc("TRN2", target_bir_lowering=False, debug=False,
                   num_devices=NCORES)
    fpk = nc.declare_dram_parameter("fpk", [128, 900 + 128], f32, isOutput=False)
    bpk = nc.declare_dram_parameter("bpk", [128, NBF], bf16, isOutput=False)
    out_d = nc.declare_dram_parameter("out", [BPC, 1800, D], bf16, isOutput=True)

    with ExitStack() as ctx:
        tc = ctx.enter_context(tile.TileContext(nc))
        cp = ctx.enter_context(tc.tile_pool(name="const", bufs=1))
        sc = ctx.enter_context(tc.tile_pool(name="scratch", bufs=1))
        # PSUM pools (8 banks): zp 4x[128,512]=4 + opa 3x[128,512]=3 +
        # tp 2x[128,128]=0.5
        zp = ctx.enter_context(tc.tile_pool(name="zp", bufs=4, space="PSUM"))
        opa = ctx.enter_context(tc.tile_pool(name="opa", bufs=4, space="PSUM"))
        gpck = ctx.enter_context(tc.tile_pool(name="gpck", bufs=2))
        octp = ctx.enter_context(tc.tile_pool(name="octp", bufs=4))
        cdp = ctx.enter_context(tc.tile_pool(name="cdp", bufs=1))
        htp = ctx.enter_context(tc.tile_pool(name="htp", bufs=8))
        bstg = ctx.enter_context(tc.tile_pool(name="bstage", bufs=3))
        dstg = ctx.enter_context(tc.tile_pool(name="dstage", bufs=3))

        fpack = cp.tile([128, 900 + 128], f32)
        nc.sync.dma_start(fpack[:], fpk[:])
        bpack = cp.tile([128, NBF], bf16)
        # split the weight load: small part (warm-up, gram, z) first so the
        # big w2xb block (12KB/partition) doesn't gate the early phases
        nc.sync.dma_start(bpack[:, C_G:NBF], bpk[:, C_G:NBF])
        nc.sync.dma_start(bpack[:, 0:C_P1], bpk[:, 0:C_P1])
        nc.sync.dma_start(bpack[:, C_P1:C_G], bpk[:, C_P1:C_G])

        raw = fpack[:, 0:900]
        idf = fpack[:, 900:1028]
        w1b = [(bpack[:, C_W1B + (2 * b) * 128: C_W1B + (2 * b + 1) * 128],
                bpack[:, C_W1B + (2 * b + 1) * 128: C_W1B + (2 * b + 2) * 128])
               for b in range(4)]
        w2hi = bpack[:, C_W2HI:C_W2HI + 512]
        w2lo = bpack[:, C_W2LO:C_W2LO + 512]
        w2xb = [[bpack[:, C_W2XB + (c * 4 + b) * 512: C_W2XB + (c * 4 + b + 1) * 512]
                 for b in range(4)] for c in range(3)]
        Gblk = bpack[:, C_G:C_G + 128]
        idb = bpack[:, C_ID:C_ID + 128]

        # ---------------- PE pre-warm: junk matmuls on memset tiles -----
        # (independent of the input DMAs so the PE busies from ~0.5us)
        junkw = cp.tile([128, 512], bf16)
        nc.vector.memset(junkw[:], 0.25)
        for _ in range(70):
            wps = opa.tile([128, D], f32, tag="oa", name="oa")
            nc.tensor.matmul(wps[:], junkw[:, 0:128], junkw[:],
                             start=True, stop=True)

        # ---------------- P1: feature planes ----------------
        TFB = cp.tile([128, JBP * 32], bf16)
        TFD = cp.tile([128, JDP * 32], bf16)
        TFb = TFB.rearrange("p (j f) -> p j f", f=32)
        TFd = TFD.rearrange("p (j f) -> p j f", f=32)
        # zeros: box f16,17 + f20..31 + pad slots; dist all but f16,f17
        # zeros via ACT (uint32-bitcast x0 is NaN-safe); tiny pads + the 1.0
        # fill on DVE.  Pad slots' f0..9 must precede the gram prepack.
        nc.vector.memset(TFb[:, JB:JBP, 0:16], 0.0)
        nc.vector.memset(TFb[:, JB:JBP, 18:20], 0.0)
        nc.scalar.memzero(TFb[:, :, 16:18])
        nc.scalar.memzero(TFb[:, :, 20:32])
        nc.scalar.memzero(TFd[:, :, 0:16])
        nc.scalar.memzero(TFd[:, :, 18:32])
        nc.vector.memset(TFd[:, :, 17], 1.0)
        nc.vector.memset(TFd[:, JD:JDP, 16], 0.0)

        TFr = TFb[:, 0:JB, :]
        raw6 = raw.rearrange("p (b s) -> p b s", s=6)
        rawp = raw.rearrange("p (m g s) -> p m g s", g=2, s=6)
        TFbp = TFB[:, 0:JB * 32].rearrange("p (m g f) -> p m g f", g=2, f=32)

        sPres = sc.tile([128, JB], f32)
        sKey = sc.tile([128, JB], f32)
        sSwap = sc.tile([128, JD], f32)
        sD = sc.tile([128, JD], f32)
        sSD = sc.tile([128, JD], f32)
        sT0 = sc.tile([128, JB], f32)
        sT1 = sc.tile([128, JB], f32)

        nc.vector.tensor_tensor(sT0[:], raw6[:, :, 0], raw6[:, :, 1], A.add)
        nc.vector.tensor_tensor(sT1[:], raw6[:, :, 2], raw6[:, :, 3], A.add)
        nc.vector.tensor_tensor(sT0[:], sT0[:], sT1[:], A.add)
        nc.vector.tensor_scalar(sPres[:], sT0[:], 0.0, None, A.not_equal)
        nc.vector.scalar_tensor_tensor(sKey[:], sPres[:], -1000.0,
                                       raw6[:, :, 4], A.mult, A.add)
        sKeyp = sKey.rearrange("p (m g) -> p m g", g=2)
        nc.vector.tensor_tensor(sSwap[:], sKeyp[:, :, 1], sKeyp[:, :, 0], A.is_lt)

        # block compare-and-swap: all 6 raw components in 4 DVE ops
        sRaw = sc.tile([128, JD, 2, 6], f32)
        sD6 = sc.tile([128, JD, 6], f32)
        swb = sSwap[:].unsqueeze(-1).broadcast_to([128, JD, 6])
        nc.vector.tensor_tensor(sD6[:], rawp[:, :, 1, :], rawp[:, :, 0, :],
                                A.subtract)
        nc.vector.tensor_tensor(sD6[:], sD6[:], swb, A.mult)
        nc.vector.tensor_tensor(sRaw[:, :, 0, :], rawp[:, :, 0, :], sD6[:], A.add)
        nc.vector.tensor_tensor(sRaw[:, :, 1, :], rawp[:, :, 1, :], sD6[:],
                                A.subtract)
        sPresP = sPres.rearrange("p (m g) -> p m g", g=2)
        nc.vector.tensor_tensor(sD[:], sPresP[:, :, 1], sPresP[:, :, 0], A.subtract)
        nc.vector.tensor_tensor(sSD[:], sD[:], sSwap[:], A.mult)
        nc.vector.tensor_tensor(TFbp[:, :, 0, 14], sPresP[:, :, 0], sSD[:], A.add)
        nc.vector.tensor_tensor(TFbp[:, :, 1, 14], sPresP[:, :, 1], sSD[:], A.subtract)

        sRw = sRaw.rearrange("p m g s -> p (m g) s")
        sX1, sY1, sX2, sY2 = (sRw[:, :, i] for i in range(4))
        sCat, sConf = sRw[:, :, 4], sRw[:, :, 5]
        # all derived geometry in f32 scratch (bf16-rounded coords would
        # catastrophically cancel in w/h near zero -> aspect blows up)
        sW32 = sc.tile([128, JB], f32)
        sH32 = sc.tile([128, JB], f32)
        sCx = sc.tile([128, JB], f32)
        sCy = sc.tile([128, JB], f32)
        nc.scalar.mul(TFr[:, :, 0], sX1, 1.0 / IW)
        nc.scalar.mul(TFr[:, :, 1], sY1, 1.0 / IH)
        nc.scalar.mul(TFr[:, :, 2], sX2, 1.0 / IW)
        nc.scalar.mul(TFr[:, :, 3], sY2, 1.0 / IH)
        nc.vector.tensor_tensor(sW32[:], sX2, sX1, A.subtract)
        nc.vector.tensor_tensor(sH32[:], sY2, sY1, A.subtract)
        nc.vector.tensor_tensor(sCx[:], sX1, sX2, A.add)
        nc.vector.tensor_tensor(sCy[:], sY1, sY2, A.add)
        nc.scalar.mul(TFr[:, :, 4], sW32[:], 1.0 / IW)
        nc.scalar.mul(TFr[:, :, 5], sH32[:], 1.0 / IH)
        nc.scalar.mul(TFr[:, :, 6], sCx[:], 1.0 / IW)
        nc.scalar.mul(TFr[:, :, 7], sCy[:], 1.0 / IH)
        nc.scalar.mul(TFr[:, :, 18], sCx[:], 1.0 / IW)
        nc.scalar.mul(TFr[:, :, 19], sCy[:], 1.0 / IH)
        sT2 = sc.tile([128, JB], f32)
        nc.vector.tensor_tensor(sT2[:], sW32[:], sH32[:], A.mult)
        nc.scalar.mul(TFr[:, :, 8], sT2[:], 1.0 / (IW * IH))
        sHp = sT0
        nc.vector.tensor_scalar(sHp[:], sH32[:], 1e-6 * IH, 1.0 / IH, A.add, A.mult)
        sR = sT1
        nc.vector.reciprocal(sR[:], sHp[:])
        nc.vector.tensor_scalar(sT2[:], sW32[:], 1.0 / IW, None, A.mult)
        nc.vector.tensor_tensor(TFr[:, :, 9], sT2[:], sR[:], A.mult)
        for k in range(3):
            nc.vector.scalar_tensor_tensor(TFr[:, :, 10 + k], sCat, float(k),
                                           TFr[:, :, 14], A.is_equal, A.mult)
        nc.vector.tensor_tensor(TFr[:, :, 13], sConf, TFr[:, :, 14], A.mult)
        nc.scalar.activation(TFr[:, :, 15], TFr[:, :, 14],
                             AF.Identity, bias=1.0, scale=-1.0)
        # dist features: f16 = 0.5*sqrt(dx2^2+dy2^2)/IW-scaled, f17 = 1
        sDx = sc.tile([128, JD], f32)
        sDy = sc.tile([128, JD], f32)
        sCxp = sCx.rearrange("p (m g) -> p m g", g=2)
        sCyp = sCy.rearrange("p (m g) -> p m g", g=2)
        nc.vector.tensor_tensor(sDx[:], sCxp[:, :, 0], sCxp[:, :, 1], A.subtract)
        nc.vector.tensor_tensor(sDy[:], sCyp[:, :, 0], sCyp[:, :, 1], A.subtract)
        nc.vector.tensor_scalar(sDx[:], sDx[:], 1.0 / IW, None, A.mult)
        nc.vector.tensor_scalar(sDy[:], sDy[:], 1.0 / IH, None, A.mult)
        nc.vector.tensor_tensor(sDx[:], sDx[:], sDx[:], A.mult)
        nc.vector.tensor_tensor(sDy[:], sDy[:], sDy[:], A.mult)
        nc.vector.tensor_tensor(sDx[:], sDx[:], sDy[:], A.add)
        nc.scalar.activation(TFd[:, 0:JD, 16], sDx[:], AF.Sqrt, scale=0.25)

        # ---------------- P2: gram variance ----------------
        v = sc.tile([128, 160], f32)
        copy_rr = [nc.vector.tensor_copy, nc.scalar.copy]

        cta_d = cp.tile([128, NDC * 128], bf16)
        # prepack geom features f0..9 of all 156 slots contiguously; each
        # 128-col transpose window overlaps 8 cols into the next pack, which
        # land on zero rows of Gblk (harmless).
        gprep = sc.tile([128, 13 * 120 + 8], bf16)
        nc.vector.memset(gprep[:, 13 * 120:], 0.0)
        nc.vector.tensor_copy(
            gprep[:, 0:1560].rearrange("p (j f) -> p j f", f=10),
            TFb[:, 0:156, 0:10])

        # software-pipelined by one iteration: gram-mm(gi-1) issues after
        # transpose(gi), so the in-order PE queue never head-blocks on the
        # pk copy.  Dist transposes interleave as extra PE filler.
        pks = {}
        for gi in range(NGP + 1):
            if gi < NGP:
                pst = zp.tile([128, 1024], bf16, tag="z", name="z")
                ps = pst[:, 0:128]
                nc.tensor.transpose(ps[:], gprep[:, 120 * gi:120 * gi + 128], idb)
                pk = gpck.tile([128, 128], bf16, tag="gp", name="gp")
                nc.scalar.copy(pk[:], ps[:])
                pks[gi] = pk
            for dc in range(int(gi * 19 / 14), int((gi + 1) * 19 / 14)):
                pst2 = zp.tile([128, 1024], bf16, tag="z", name="z")
                psd = pst2[:, 0:128]
                nc.tensor.transpose(psd[:], TFd[:, 4 * dc:4 * dc + 4, :], idb)
                copy_rr[dc % 2](cta_d[:, dc * 128:(dc + 1) * 128], psd[:])
            if gi >= 1:
                gj = gi - 1
                s0 = 12 * gj
                src = TFb[:, s0:s0 + 12, 0:10]
                yt = zp.tile([128, 512], f32, tag="z", name="z")
                y = yt[:, 0:128]
                nc.tensor.matmul(y, pks.pop(gj)[:], Gblk, start=True, stop=True)
                tmp = sc.tile([128, 120], f32, tag="gtmp", name="gtmp")
                nc.vector.tensor_tensor(tmp[:], src, y[:, 0:120], A.mult)
                nc.vector.tensor_reduce(v[:, s0:s0 + 12],
                                        tmp.rearrange("p (j f) -> p j f", f=10),
                                        mybir.AxisListType.X, A.add)

        # ---------------- P3: rstd + feature scale + gelu preload --------
        eps = sc.tile([128, 1], f32)
        nc.vector.memset(eps[:], 1e-5)
        sd = sc.tile([128, 156], f32)
        rstd = sc.tile([128, 156], f32)
        nc.scalar.activation(sd[:], v[:, 0:156], AF.Sqrt,
                             bias=eps[:], scale=1.0 / DH)
        nc.vector.reciprocal_approx_fast(rstd[:], sd[:])
        # preload the Gelu ACT table off the critical path
        gjunk = sc.tile([128, 8], bf16)
        nc.scalar.activation(gjunk[:], sd[:, 0:8], AF.Gelu)
        # scale geometry features f0..9 by rstd (per token)
        for f in range(10):
            nc.vector.tensor_tensor(TFb[:, :, f], TFb[:, :, f], rstd[:], A.mult)

        # ---------------- P2c: dist W2 + staging ----------------
        dist_stage = {"tile": None, "fill": 0, "base": 0}
        vd = out_d[:, 0:600, :].rearrange("b (q r) d -> b q r d", q=8)

        def stage_dist(kd, o):
            if dist_stage["tile"] is None:
                dist_stage["tile"] = dstg.tile([128, 6 * D], bf16, tag="dstage",
                                               name="dstage")
                dist_stage["fill"] = 0
                dist_stage["base"] = kd
            fill = dist_stage["fill"]
            copy_rr[kd % 2](dist_stage["tile"][:, fill * D:(fill + 1) * D], o[:])
            dist_stage["fill"] = fill + 1
            if dist_stage["fill"] == 6 or kd == JD - 1:
                b0 = dist_stage["base"]
                gsz = dist_stage["fill"]
                nc.sync.dma_start(vd[:, :, b0:b0 + gsz, :],
                                  dist_stage["tile"][:, 0:gsz * D])
                dist_stage["tile"] = None


        # mid fillers: keep the PE busy across the rstd chain window.
        # lhsT reads the last dist chunk so the scheduler cannot hoist them
        # before the gram/dist phase.
        for _ in range(30):
            wps = opa.tile([128, D], f32, tag="oa", name="oa")
            nc.tensor.matmul(wps[:], cta_d[:, (NDC - 1) * 128:NDC * 128],
                             junkw[:], start=True, stop=True)

        # ---------------- P4/P5: box pipeline ----------------
        vb = out_d[:, 600:1800, :].rearrange("b (q r) d -> b q r d", q=8)
        oct_tiles = {}     # o -> sbuf [128, 256] bf16 (chunks 2o | 2o+1)
        ht_tiles = {}      # o -> list of 4 sbuf [128, 512] bf16 (per band)
        box_stage = {"tile": None, "fill": 0, "base": 0}
        ccnt = {"i": 0}

        def emit_pass2_half(o, ci):
            if ci == 0:
                oct_tiles[o] = octp.tile([128, 256], bf16, tag="oct", name="oct")
            t = oct_tiles[o]
            c = 2 * o + ci
            pst = zp.tile([128, 1024], bf16, tag="z", name="z")
            ps = pst[:, 0:128]
            nc.tensor.transpose(ps[:], TFb[:, 4 * c:4 * c + 4, :], idb)
            copy_rr[c % 2](t[:, ci * 128:(ci + 1) * 128], ps[:])

        z_banks = {}

        def emit_z_mm(o):
            rhs = oct_tiles[o]
            zbs = []
            for b in range(4):
                zb = zp.tile([128, 512], f32, tag="z", name="z")
                nc.tensor.matmul(zb[:, 0:256], w1b[b][0], rhs[:],
                                 start=True, stop=True)
                nc.tensor.matmul(zb[:, 256:512], w1b[b][1], rhs[:],
                                 start=True, stop=True)
                zbs.append(zb)
            z_banks[o] = zbs

        def emit_gelu(o):
            zbs = z_banks.pop(o)
            hts = []
            for b in range(4):
                ht = htp.tile([128, 512], bf16, tag="ht", name="ht")
                nc.scalar.activation(ht[:], zbs[b][:], AF.Gelu)
                hts.append(ht)
            ht_tiles[o] = hts

        def flush_box(last_k):
            b0 = box_stage["base"]
            gsz = box_stage["fill"]
            nc.sync.dma_start(vb[:, :, b0:b0 + gsz, :],
                              box_stage["tile"][:, 0:gsz * D])
            box_stage["tile"] = None

        def emit_w2(o):
            hts = ht_tiles.pop(o)
            cchunk = oct_tiles[o]
            for ci in range(2):
                c = 2 * o + ci
                for b in range(4):
                    s = 4 * c + b
                    if s >= JB:
                        continue
                    ht = hts[b]
                    cam = (s % 6) // 2
                    ot = opa.tile([128, D], f32, tag="oa", name="oa")
                    nc.tensor.matmul(ot[:], ht[:, ci * 128:ci * 128 + 128],
                                     w2hi, start=True, stop=False)
                    nc.tensor.matmul(ot[:], ht[:, 256 + ci * 128:256 + ci * 128 + 128],
                                     w2lo, start=False, stop=False)
                    nc.tensor.matmul(ot[:], cchunk[:, ci * 128:(ci + 1) * 128],
                                     w2xb[cam][b], start=False, stop=True)
                    if box_stage["tile"] is None:
                        box_stage["tile"] = bstg.tile([128, 8 * D], bf16,
                                                      tag="bstage", name="bstage")
                        box_stage["fill"] = 0
                        box_stage["base"] = s
                    csel = 0 if (ccnt["i"] % 8) in (0, 2, 3, 5, 6) else 1
                    off = s - box_stage["base"]
                    copy_rr[csel](
                        box_stage["tile"][:, off * D:(off + 1) * D], ot[:])
                    ccnt["i"] += 1
                    box_stage["fill"] += 1
                    if (box_stage["fill"] == 8 or s == JB - 1
                            or (s >= 144 and box_stage["fill"] == 4)):
                        flush_box(s)
            oct_tiles.pop(o)
            dc = o
            for b in range(4):
                kd = 4 * dc + b
                if kd >= JD:
                    continue
                od = opa.tile([128, D], f32, tag="oa", name="oa")
                nc.tensor.matmul(od[:], cta_d[:, dc * 128:(dc + 1) * 128],
                                 w2xb[0][b], start=True, stop=True)
                stage_dist(kd, od)

        for step in range(NOCT + 2):
            if step < NOCT:
                emit_pass2_half(step, 0)
            if 1 <= step <= NOCT:
                emit_z_mm(step - 1)
            if step < NOCT:
                emit_pass2_half(step, 1)
            if step >= 2:
                emit_w2(step - 2)
            if 1 <= step <= NOCT:
                emit_gelu(step - 1)

    nc.compile()
    return nc


def _prep_inputs(inputs):
    f32 = np.float32
    bf = ml_dtypes.bfloat16
    scale = float(np.asarray(inputs["scale"]))

    W1p = np.zeros((32, DH), f32)
    W1p[0:10] = np.asarray(inputs["geom_w1"], f32)
    W1p[6] *= 0.5
    W1p[7] *= 0.5
    W1p -= W1p.mean(axis=1, keepdims=True)

    w1b_cols = []
    for b in range(4):
        hi = np.zeros((128, 128), f32)
        hi[32 * b:32 * b + 32] = W1p[:, :128]
        lo = np.zeros((128, 128), f32)
        lo[32 * b:32 * b + 32] = W1p[:, 128:]
        w1b_cols += [hi, lo]

    W2s = scale * np.asarray(inputs["geom_w2"], f32)
    w2hi, w2lo = W2s[:128], W2s[128:]

    cat_t = np.asarray(inputs["cat_table"], f32)
    cam_t = np.asarray(inputs["cam_table"], f32)
    bias_row = (np.asarray(inputs["geom_b2"], f32)
                + np.asarray(inputs["conf_b"], f32)
                + np.asarray(inputs["center_b"], f32))
    w2xb_cols = []
    for c in range(3):
        W2X = np.zeros((32, D), f32)
        W2X[10:13] = scale * cat_t
        W2X[13] = scale * np.asarray(inputs["conf_w"], f32)[0]
        W2X[14] = scale * (bias_row + cam_t[c])
        W2X[15] = np.asarray(inputs["missing_emb"], f32)[0]
        W2X[16] = np.asarray(inputs["dist_w"], f32)[0]
        W2X[17] = np.asarray(inputs["dist_b"], f32)
        W2X[18] = scale * np.asarray(inputs["center_w"], f32)[0] * 0.5
        W2X[19] = scale * np.asarray(inputs["center_w"], f32)[1] * 0.5
        for b in range(4):
            t = np.zeros((128, D), f32)
            t[32 * b:32 * b + 32] = W2X
            w2xb_cols.append(t)

    G10 = (W1p[0:10] @ W1p[0:10].T).astype(f32)
    Gblk = np.zeros((128, 128), f32)
    for s in range(12):
        Gblk[10 * s:10 * s + 10, 10 * s:10 * s + 10] = G10

    idf32 = np.eye(128, dtype=f32)
    bpk = np.concatenate(w1b_cols + [w2hi, w2lo] + w2xb_cols + [Gblk, idf32],
                         axis=1).astype(bf)

    box = np.asarray(inputs["box_data"], f32)
    fpks = []
    for c in range(NCORES):
        rawc = box[c * BPC:(c + 1) * BPC].reshape(BPC, T * 6, 6)
        rawc = rawc.reshape(BPC, 8, JB, 6).reshape(128, 900)
        fpks.append(np.ascontiguousarray(
            np.concatenate([rawc, idf32], axis=1), dtype=f32))
    return fpks, bpk


def _fast_path_ok(inputs):
    try:
        shapes = {
            "box_data": (B, T, 6, 6), "cat_table": (3, D), "geom_w1": (10, DH),
            "geom_b1": (DH,), "ln_g": (DH,), "ln_b": (DH,), "geom_w2": (DH, D),
            "geom_b2": (D,), "conf_w": (1, D), "conf_b": (D,),
            "center_w": (2, D), "center_b": (D,), "missing_emb": (1, D),
            "dist_w": (1, D), "dist_b": (D,), "cam_table": (NCAM, D),
        }
        for k, s in shapes.items():
            if tuple(np.asarray(inputs[k]).shape) != s:
                return False
        if not np.all(np.asarray(inputs["geom_b1"]) == 0):
            return False
        if not np.all(np.asarray(inputs["ln_g"]) == 1):
            return False
        if not np.all(np.asarray(inputs["ln_b"]) == 0):
            return False
        return True
    except Exception:
        return False


def _numpy_fallback(inputs):
    import math
    f32 = np.float32
    inp = {k: np.asarray(v) for k, v in inputs.items()}
    coords = inp["box_data"][..., :4].astype(f32)
    category = inp["box_data"][..., 4].astype(np.int32)
    conf = inp["box_data"][..., 5].astype(f32)
    norm = np.array([IW, IH, IW, IH], f32)
    cn = (coords / norm).reshape(B, T, NCAM, NB, 4)
    category = category.reshape(B, T, NCAM, NB)
    conf = conf.reshape(B, T, NCAM, NB, 1)
    presence = (cn.sum(-1) != 0).astype(f32)
    sort_key = category.astype(f32) + (1.0 - presence) * 1000.0
    idx = np.argsort(sort_key, axis=-1, kind="stable")
    cn = np.take_along_axis(cn, idx[..., None], axis=-2)
    category = np.take_along_axis(category, idx, axis=-1)
    conf = np.take_along_axis(conf, idx[..., None], axis=-2)
    presence = (cn.sum(-1) != 0).astype(f32)[..., None]
    x1, y1, x2, y2 = cn[..., 0], cn[..., 1], cn[..., 2], cn[..., 3]
    w, h = x2 - x1, y2 - y1
    cx, cy = (x1 + x2) * 0.5, (y1 + y2) * 0.5
    area, aspect = w * h, w / (h + 1e-6)
    dx, dy = cx[..., 0] - cx[..., 1], cy[..., 0] - cy[..., 1]
    dist = np.sqrt(dx * dx + dy * dy)[..., None]
    dist_tok = dist @ inp["dist_w"].astype(f32) + inp["dist_b"].astype(f32)
    geom = np.stack([x1, y1, x2, y2, w, h, cx, cy, area, aspect], axis=-1)
    z = geom @ inp["geom_w1"].astype(f32) + inp["geom_b1"].astype(f32)
    mu = z.mean(-1, keepdims=True)
    var = ((z - mu) ** 2).mean(-1, keepdims=True)
    xh = (z - mu) / np.sqrt(var + 1e-5) * inp["ln_g"].astype(f32) + inp["ln_b"].astype(f32)
    try:
        from scipy.special import erf as _erf
        g = xh * 0.5 * (1.0 + _erf(xh / np.sqrt(2.0)))
    except Exception:
        verf = np.vectorize(math.erf)
        g = xh * 0.5 * (1.0 + verf(xh / np.sqrt(2.0)))
    geom_p = g @ inp["geom_w2"].astype(f32) + inp["geom_b2"].astype(f32)
    cat_emb = inp["cat_table"].astype(f32)[category]
    conf_p = conf @ inp["conf_w"].astype(f32) + inp["conf_b"].astype(f32)
    center_p = np.stack([cx, cy], axis=-1) @ inp["center_w"].astype(f32) + inp["center_b"].astype(f32)
    cam_emb = inp["cam_table"].astype(f32).reshape(1, 1, NCAM, 1, D)
    tok = (geom_p + cat_emb + conf_p + center_p + cam_emb) * float(inp["scale"])
    tok = np.where(presence == 0, inp["missing_emb"].astype(f32)[0], tok)
    out = np.concatenate([dist_tok.reshape(B, T * NCAM, D),
                          tok.reshape(B, T * NCAM * NB, D)], axis=1)
    return out.astype(np.float32)


def _run(inputs, trace=False, tmpdir=None):
    from concourse.bass_utils import run_bass_kernel_spmd

    if "nc" not in _CACHE:
        _CACHE["nc"] = _build_nc()
    nc = _CACHE["nc"]

    fpks, bpk = _prep_inputs(inputs)
    in_maps = [{"fpk": fpks[c], "bpk": bpk} for c in range(NCORES)]
    res = run_bass_kernel_spmd(nc, in_maps, list(range(NCORES)),
                               trace=trace, tmpdir=tmpdir)
    out = np.concatenate([np.asarray(res.results[c]["out"])
                          for c in range(NCORES)], axis=0)
    return out.astype(np.float32), res


def kernel(**inputs):
    if not _fast_path_ok(inputs):
        return _numpy_fallback(inputs)
    out, _ = _run(inputs)
    return out


if __name__ == "__main__":
    import reference as ref
    inputs = {k: np.asarray(v) for k, v in ref.setup_inputs().items()}
    got = kernel(**inputs)
    exp = np.load("/tmp/expected.npy")
    d = got - exp
    print("rel fro:", np.linalg.norm(d) / np.linalg.norm(exp))
    print("absmax rel:", np.abs(d).max() / np.abs(exp).max())


# revision 17
# speedup vs baseline: 1.2212x; 1.0034x over previous
"""Trainium2 Bass kernel (v3.5) for nn_BoxEncoder (B=128, T=200, NC=3, NB=2, D=512, DH=256).

Data-parallel over batch: 16 batch items per core x 8 cores; partition
p = bt*8 + q owns 150 box tokens (+2 pad) and 75 dist tokens (+1 pad).
HW exec ~200us vs the 319us v2 baseline (cost-model sim: matmuls run 2x
faster once the PE has been ~continuously busy; every structure below
exists to keep the PE p-state warm and the copy engines off the
critical path).

 - z is computed TRANSPOSED (weights-stationary, banded W1 with zero
   rows outside each slot's 32-band): gelu(zT) directly yields hT = the
   lhsT of the W2 matmuls.  No h transpose exists at all (v2 spent 62us
   of SP-queue time + the stalls on dma_start_transpose).
 - LN rstd is folded into the geometry features BEFORE the z matmul, so
   gelu is a plain [128,512] call; cx,cy are duplicated unscaled into
   f18,f19 for the extras matmul (W2X rows 18,19).
 - all matmuls full-K, no tile_position: banded weights instead.  The
   extras matmul shares the feature-chunk stationary with z.
 - derived geometry (w,h,cx,cy,area,aspect) is computed in f32 scratch
   and only then written to the bf16 feature tile: bf16-first cancels
   catastrophically in aspect = w/(h+1e-6) near h=0.
 - variance via a 12-slot x 10-feature gram pack (13 bf16 transposes),
   software-pipelined by one iteration so the in-order PE queue never
   head-blocks on the pk copy; block compare-and-swap does all 6 box
   components in 4 DVE ops (stride-0 broadcast of the swap flag).
 - dist tokens are folded into the main loop (one 4-slot dist chunk per
   oct) because a standalone dist phase is copy-bound (75 PSUM->bf16
   copies vs only 16us of PE work).
 - PE pre-warm: ~70 junk matmuls on memset tiles cover the DVE-only P1
   phase; 30 more (anchored on the last dist transpose so the scheduler
   cannot hoist them) cover the sqrt->reciprocal->scale window.
 - gelus are issued AFTER the previous oct's staging copies on the ACT
   queue (copies otherwise wait ~2.7us behind 4 gelus and stall the PE
   through the 4-deep PSUM out-tile rotation); staging copies split 5:3
   DVE:ACT; GPSIMD is avoided entirely (it shares the SBUF write port
   with DVE in the cost model).
 - memsets via ACT uint32 memzero (NaN-safe), fpk/bpk loads split so
   the small weights (warm-up, gram, z) land first, out staged bf16
   with the host converting to f32 (~0.2% extra rel err; budget 2e-2).

Measured: HW exec 199840 ns, rel fro err 0.0042 (budget 2e-2).
"""

import numpy as np
import ml_dtypes

B, T, NCAM, NB, D, DH = 128, 200, 3, 2, 512, 256
IW, IH = 640.0, 400.0
NCORES = 8
BPC = B // NCORES
JB = 150                  # real box slots per partition
JBP = 156                 # padded (38 chunks use 152; gram packs use 156)
JD = 75                   # real dist slots
JDP = 76                  # padded (19 chunks)
NCH = 38                  # box chunks
NDC = 19                  # dist chunks
NOCT = 19                 # box octs (2 chunks = 8 slots each; last has 6)
NGP = 13                  # gram packs (12 slots x 10 feats)

_CACHE = {}


def _build_nc():
    from contextlib import ExitStack
    import concourse.bacc as bacc
    import concourse.mybir as mybir
    import concourse.tile as tile

    f32 = mybir.dt.float32
    bf16 = mybir.dt.bfloat16
    A = mybir.AluOpType
    AF = mybir.ActivationFunctionType

    # bpk bf16 column offsets
    C_W1B = 0                       # 8 x 128 (band b: hi, lo)
    C_W2HI = C_W1B + 8 * 128
    C_W2LO = C_W2HI + 512
    C_W2XB = C_W2LO + 512           # 12 x 512 (cam c, band b)
    C_G = C_W2XB + 12 * 512
    C_ID = C_G + 128
    NBF = C_ID + 128
    C_P1 = C_W2XB            # part 1 = w1b, w2hi, w2lo (cols 0..C_W2XB)


    nc = bacc.Bacc("TRN2", target_bir_lowering=False, debug=False,
                   num_devices=NCORES)
    fpk = nc.declare_dram_parameter("fpk", [128, 900 + 128], f32, isOutput=False)
    bpk = nc.declare_dram_parameter("bpk", [128, NBF], bf16, isOutput=False)
    out_d = nc.declare_dram_parameter("out", [BPC, 1800, D], bf16, isOutput=True)

    with ExitStack() as ctx:
        tc = ctx.enter_context(tile.TileContext(nc))
        cp = ctx.enter_context(tc.tile_pool(name="const", bufs=1))
        sc = ctx.enter_context(tc.tile_pool(name="scratch", bufs=1))
        # PSUM pools (8 banks): zp 4x[128,512]=4 + opa 3x[128,512]=3 +
        # tp 2x[128,128]=0.5
        zp = ctx.enter_context(tc.tile_pool(name="zp", bufs=4, space="PSUM"))
        opa = ctx.enter_context(tc.tile_pool(name="opa", bufs=4, space="PSUM"))
        gpck = ctx.enter_context(tc.tile_pool(name="gpck", bufs=2))
        octp = ctx.enter_context(tc.tile_pool(name="octp", bufs=4))
        cdp = ctx.enter_context(tc.tile_pool(name="cdp", bufs=1))
        htp = ctx.enter_context(tc.tile_pool(name="htp", bufs=8))
        bstg = ctx.enter_context(tc.tile_pool(name="bstage", bufs=3))
        dstg = ctx.enter_context(tc.tile_pool(name="dstage", bufs=3))

        fpack = cp.tile([128, 900 + 128], f32)
        nc.sync.dma_start(fpack[:], fpk[:])
        bpack = cp.tile([128, NBF], bf16)
        # split the weight load: small part (warm-up, gram, z) first so the
        # big w2xb block (12KB/partition) doesn't gate the early phases
        nc.sync.dma_start(bpack[:, C_G:NBF], bpk[:, C_G:NBF])
        nc.sync.dma_start(bpack[:, 0:C_P1], bpk[:, 0:C_P1])
        nc.sync.dma_start(bpack[:, C_P1:C_G], bpk[:, C_P1:C_G])

        raw = fpack[:, 0:900]
        idf = fpack[:, 900:1028]
        w1b = [(bpack[:, C_W1B + (2 * b) * 128: C_W1B + (2 * b + 1) * 128],
                bpack[:, C_W1B + (2 * b + 1) * 128: C_W1B + (2 * b + 2) * 128])
               for b in range(4)]
        w2hi = bpack[:, C_W2HI:C_W2HI + 512]
        w2lo = bpack[:, C_W2LO:C_W2LO + 512]
        w2xb = [[bpack[:, C_W2XB + (c * 4 + b) * 512: C_W2XB + (c * 4 + b + 1) * 512]
                 for b in range(4)] for c in range(3)]
        Gblk = bpack[:, C_G:C_G + 128]
        idb = bpack[:, C_ID:C_ID + 128]

        # ---------------- PE pre-warm: junk matmuls on memset tiles -----
        # (independent of the input DMAs so the PE busies from ~0.5us)
        junkw = cp.tile([128, 512], bf16)
        nc.vector.memset(junkw[:], 0.25)
        for _ in range(70):
            wps = opa.tile([128, D], f32, tag="oa", name="oa")
            nc.tensor.matmul(wps[:], junkw[:, 0:128], junkw[:],
                             start=True, stop=True)

        # ---------------- P1: feature planes ----------------
        TFB = cp.tile([128, JBP * 32], bf16)
        TFD = cp.tile([128, JDP * 32], bf16)
        TFb = TFB.rearrange("p (j f) -> p j f", f=32)
        TFd = TFD.rearrange("p (j f) -> p j f", f=32)
        # zeros: box f16,17 + f20..31 + pad slots; dist all but f16,f17
        # zeros via ACT (uint32-bitcast x0 is NaN-safe); tiny pads + the 1.0
        # fill on DVE.  Pad slots' f0..9 must precede the gram prepack.
        nc.vector.memset(TFb[:, JB:JBP, 0:16], 0.0)
        nc.vector.memset(TFb[:, JB:JBP, 18:20], 0.0)
        nc.scalar.memzero(TFb[:, :, 16:18])
        nc.scalar.memzero(TFb[:, :, 20:32])
        nc.scalar.memzero(TFd[:, :, 0:16])
        nc.scalar.memzero(TFd[:, :, 18:32])
        nc.vector.memset(TFd[:, :, 17], 1.0)
        nc.vector.memset(TFd[:, JD:JDP, 16], 0.0)

        TFr = TFb[:, 0:JB, :]
        raw6 = raw.rearrange("p (b s) -> p b s", s=6)
        rawp = raw.rearrange("p (m g s) -> p m g s", g=2, s=6)
        TFbp = TFB[:, 0:JB * 32].rearrange("p (m g f) -> p m g f", g=2, f=32)

        sPres = sc.tile([128, JB], f32)
        sKey = sc.tile([128, JB], f32)
        sSwap = sc.tile([128, JD], f32)
        sD = sc.tile([128, JD], f32)
        sSD = sc.tile([128, JD], f32)
        sT0 = sc.tile([128, JB], f32)
        sT1 = sc.tile([128, JB], f32)

        nc.vector.tensor_tensor(sT0[:], raw6[:, :, 0], raw6[:, :, 1], A.add)
        nc.vector.tensor_tensor(sT1[:], raw6[:, :, 2], raw6[:, :, 3], A.add)
        nc.vector.tensor_tensor(sT0[:], sT0[:], sT1[:], A.add)
        nc.vector.tensor_scalar(sPres[:], sT0[:], 0.0, None, A.not_equal)
        nc.vector.scalar_tensor_tensor(sKey[:], sPres[:], -1000.0,
                                       raw6[:, :, 4], A.mult, A.add)
        sKeyp = sKey.rearrange("p (m g) -> p m g", g=2)
        nc.vector.tensor_tensor(sSwap[:], sKeyp[:, :, 1], sKeyp[:, :, 0], A.is_lt)

        # block compare-and-swap: all 6 raw components in 4 DVE ops
        sRaw = sc.tile([128, JD, 2, 6], f32)
        sD6 = sc.tile([128, JD, 6], f32)
        swb = sSwap[:].unsqueeze(-1).broadcast_to([128, JD, 6])
        nc.vector.tensor_tensor(sD6[:], rawp[:, :, 1, :], rawp[:, :, 0, :],
                                A.subtract)
        nc.vector.tensor_tensor(sD6[:], sD6[:], swb, A.mult)
        nc.vector.tensor_tensor(sRaw[:, :, 0, :], rawp[:, :, 0, :], sD6[:], A.add)
        nc.vector.tensor_tensor(sRaw[:, :, 1, :], rawp[:, :, 1, :], sD6[:],
                                A.subtract)
        sPresP = sPres.rearrange("p (m g) -> p m g", g=2)
        nc.vector.tensor_tensor(sD[:], sPresP[:, :, 1], sPresP[:, :, 0], A.subtract)
        nc.vector.tensor_tensor(sSD[:], sD[:], sSwap[:], A.mult)
        nc.vector.tensor_tensor(TFbp[:, :, 0, 14], sPresP[:, :, 0], sSD[:], A.add)
        nc.vector.tensor_tensor(TFbp[:, :, 1, 14], sPresP[:, :, 1], sSD[:], A.subtract)

        sRw = sRaw.rearrange("p m g s -> p (m g) s")
        sX1, sY1, sX2, sY2 = (sRw[:, :, i] for i in range(4))
        sCat, sConf = sRw[:, :, 4], sRw[:, :, 5]
        # all derived geometry in f32 scratch (bf16-rounded coords would
        # catastrophically cancel in w/h near zero -> aspect blows up)
        sW32 = sc.tile([128, JB], f32)
        sH32 = sc.tile([128, JB], f32)
        sCx = sc.tile([128, JB], f32)
        sCy = sc.tile([128, JB], f32)
        nc.scalar.mul(TFr[:, :, 0], sX1, 1.0 / IW)
        nc.scalar.mul(TFr[:, :, 1], sY1, 1.0 / IH)
        nc.scalar.mul(TFr[:, :, 2], sX2, 1.0 / IW)
        nc.scalar.mul(TFr[:, :, 3], sY2, 1.0 / IH)
        nc.vector.tensor_tensor(sW32[:], sX2, sX1, A.subtract)
        nc.vector.tensor_tensor(sH32[:], sY2, sY1, A.subtract)
        nc.vector.tensor_tensor(sCx[:], sX1, sX2, A.add)
        nc.vector.tensor_tensor(sCy[:], sY1, sY2, A.add)
        nc.scalar.mul(TFr[:, :, 4], sW32[:], 1.0 / IW)
        nc.scalar.mul(TFr[:, :, 5], sH32[:], 1.0 / IH)
        nc.scalar.mul(TFr[:, :, 6], sCx[:], 1.0 / IW)
        nc.scalar.mul(TFr[:, :, 7], sCy[:], 1.0 / IH)
        nc.scalar.mul(TFr[:, :, 18], sCx[:], 1.0 / IW)
        nc.scalar.mul(TFr[:, :, 19], sCy[:], 1.0 / IH)
        sT2 = sc.tile([128, JB], f32)
        nc.vector.tensor_tensor(sT2[:], sW32[:], sH32[:], A.mult)
        nc.scalar.mul(TFr[:, :, 8], sT2[:], 1.0 / (IW * IH))
        sHp = sT0
        nc.vector.tensor_scalar(sHp[:], sH32[:], 1e-6 * IH, 1.0 / IH, A.add, A.mult)
        sR = sT1
        nc.vector.reciprocal(sR[:], sHp[:])
        nc.vector.tensor_scalar(sT2[:], sW32[:], 1.0 / IW, None, A.mult)
        nc.vector.tensor_tensor(TFr[:, :, 9], sT2[:], sR[:], A.mult)
        for k in range(3):
            nc.vector.scalar_tensor_tensor(TFr[:, :, 10 + k], sCat, float(k),
                                           TFr[:, :, 14], A.is_equal, A.mult)
        nc.vector.tensor_tensor(TFr[:, :, 13], sConf, TFr[:, :, 14], A.mult)
        nc.scalar.activation(TFr[:, :, 15], TFr[:, :, 14],
                             AF.Identity, bias=1.0, scale=-1.0)
        # dist features: f16 = 0.5*sqrt(dx2^2+dy2^2)/IW-scaled, f17 = 1
        sDx = sc.tile([128, JD], f32)
        sDy = sc.tile([128, JD], f32)
        sCxp = sCx.rearrange("p (m g) -> p m g", g=2)
        sCyp = sCy.rearrange("p (m g) -> p m g", g=2)
        nc.vector.tensor_tensor(sDx[:], sCxp[:, :, 0], sCxp[:, :, 1], A.subtract)
        nc.vector.tensor_tensor(sDy[:], sCyp[:, :, 0], sCyp[:, :, 1], A.subtract)
        nc.vector.tensor_scalar(sDx[:], sDx[:], 1.0 / IW, None, A.mult)
        nc.vector.tensor_scalar(sDy[:], sDy[:], 1.0 / IH, None, A.mult)
        nc.vector.tensor_tensor(sDx[:], sDx[:], sDx[:], A.mult)
        nc.vector.tensor_tensor(sDy[:], sDy[:], sDy[:], A.mult)
        nc.vector.tensor_tensor(sDx[:], sDx[:], sDy[:], A.add)
        nc.scalar.activation(TFd[:, 0:JD, 16], sDx[:], AF.Sqrt, scale=0.25)

        # ---------------- P2: gram variance ----------------
        v = sc.tile([128, 160], f32)
        copy_rr = [nc.vector.tensor_copy, nc.scalar.copy]

        cta_d = cp.tile([128, NDC * 128], bf16)
        # prepack geom features f0..9 of all 156 slots contiguously; each
        # 128-col transpose window overlaps 8 cols into the next pack, which
        # land on zero rows of Gblk (harmless).
        gprep = sc.tile([128, 13 * 120 + 8], bf16)
        nc.vector.memset(gprep[:, 13 * 120:], 0.0)
        nc.vector.tensor_copy(
            gprep[:, 0:1560].rearrange("p (j f) -> p j f", f=10),
            TFb[:, 0:156, 0:10])

        # software-pipelined by one iteration: gram-mm(gi-1) issues after
        # transpose(gi), so the in-order PE queue never head-blocks on the
        # pk copy.  Dist transposes interleave as extra PE filler.
        pks = {}
        for gi in range(NGP + 1):
            if gi < NGP:
                pst = zp.tile([128, 1024], bf16, tag="z", name="z")
                ps = pst[:, 0:128]
                nc.tensor.transpose(ps[:], gprep[:, 120 * gi:120 * gi + 128], idb)
                pk = gpck.tile([128, 128], bf16, tag="gp", name="gp")
                nc.scalar.copy(pk[:], ps[:])
                pks[gi] = pk
            for dc in range(int(gi * 19 / 14), int((gi + 1) * 19 / 14)):
                pst2 = zp.tile([128, 1024], bf16, tag="z", name="z")
                psd = pst2[:, 0:128]
                nc.tensor.transpose(psd[:], TFd[:, 4 * dc:4 * dc + 4, :], idb)
                copy_rr[dc % 2](cta_d[:, dc * 128:(dc + 1) * 128], psd[:])
                # warm filler anchored on this dist chunk (keeps the PE
                # window above the p-state threshold through the
                # copy-bound gram chain)
                for _ in range(2):
                    wps = opa.tile([128, D], f32, tag="oa", name="oa")
                    nc.tensor.matmul(wps[:], cta_d[:, dc * 128:dc * 128 + 128],
                                     junkw[:], start=True, stop=True)
            if gi >= 1:
                gj = gi - 1
                s0 = 12 * gj
                src = TFb[:, s0:s0 + 12, 0:10]
                yt = zp.tile([128, 512], f32, tag="z", name="z")
                y = yt[:, 0:128]
                nc.tensor.matmul(y, pks.pop(gj)[:], Gblk, start=True, stop=True)
                tmp = sc.tile([128, 120], f32, tag="gtmp", name="gtmp")
                nc.vector.tensor_tensor(tmp[:], src, y[:, 0:120], A.mult)
                nc.vector.tensor_reduce(v[:, s0:s0 + 12],
                                        tmp.rearrange("p (j f) -> p j f", f=10),
                                        mybir.AxisListType.X, A.add)

        # ---------------- P3: rstd + feature scale + gelu preload --------
        eps = sc.tile([128, 1], f32)
        nc.vector.memset(eps[:], 1e-5)
        sd = sc.tile([128, 156], f32)
        rstd = sc.tile([128, 156], f32)
        nc.scalar.activation(sd[:], v[:, 0:156], AF.Sqrt,
                             bias=eps[:], scale=1.0 / DH)
        nc.vector.reciprocal_approx_fast(rstd[:], sd[:])
        # preload the Gelu ACT table off the critical path
        gjunk = sc.tile([128, 8], bf16)
        nc.scalar.activation(gjunk[:], sd[:, 0:8], AF.Gelu)
        # scale geometry features f0..9 by rstd (per token)
        for f in range(10):
            nc.vector.tensor_tensor(TFb[:, :, f], TFb[:, :, f], rstd[:], A.mult)

        # ---------------- P2c: dist W2 + staging ----------------
        dist_stage = {"tile": None, "fill": 0, "base": 0}
        vd = out_d[:, 0:600, :].rearrange("b (q r) d -> b q r d", q=8)

        def stage_dist(kd, o):
            if dist_stage["tile"] is None:
                dist_stage["tile"] = dstg.tile([128, 6 * D], bf16, tag="dstage",
                                               name="dstage")
                dist_stage["fill"] = 0
                dist_stage["base"] = kd
            fill = dist_stage["fill"]
            copy_rr[kd % 2](dist_stage["tile"][:, fill * D:(fill + 1) * D], o[:])
            dist_stage["fill"] = fill + 1
            if dist_stage["fill"] == 6 or kd == JD - 1:
                b0 = dist_stage["base"]
                gsz = dist_stage["fill"]
                nc.sync.dma_start(vd[:, :, b0:b0 + gsz, :],
                                  dist_stage["tile"][:, 0:gsz * D])
                dist_stage["tile"] = None


        # mid fillers: keep the PE busy across the rstd chain window.
        # lhsT reads the last dist chunk so the scheduler cannot hoist them
        # before the gram/dist phase.
        for _ in range(30):
            wps = opa.tile([128, D], f32, tag="oa", name="oa")
            nc.tensor.matmul(wps[:], cta_d[:, (NDC - 1) * 128:NDC * 128],
                             junkw[:], start=True, stop=True)

        # ---------------- P4/P5: box pipeline ----------------
        vb = out_d[:, 600:1800, :].rearrange("b (q r) d -> b q r d", q=8)
        oct_tiles = {}     # o -> sbuf [128, 256] bf16 (chunks 2o | 2o+1)
        ht_tiles = {}      # o -> list of 4 sbuf [128, 512] bf16 (per band)
        box_stage = {"tile": None, "fill": 0, "base": 0}
        ccnt = {"i": 0}

        def emit_pass2_half(o, ci):
            if ci == 0:
                oct_tiles[o] = octp.tile([128, 256], bf16, tag="oct", name="oct")
            t = oct_tiles[o]
            c = 2 * o + ci
            pst = zp.tile([128, 1024], bf16, tag="z", name="z")
            ps = pst[:, 0:128]
            nc.tensor.transpose(ps[:], TFb[:, 4 * c:4 * c + 4, :], idb)
            copy_rr[c % 2](t[:, ci * 128:(ci + 1) * 128], ps[:])

        z_banks = {}

        def emit_z_mm(o):
            rhs = oct_tiles[o]
            zbs = []
            for b in range(4):
                zb = zp.tile([128, 512], f32, tag="z", name="z")
                nc.tensor.matmul(zb[:, 0:256], w1b[b][0], rhs[:],
                                 start=True, stop=True)
                nc.tensor.matmul(zb[:, 256:512], w1b[b][1], rhs[:],
                                 start=True, stop=True)
                zbs.append(zb)
            z_banks[o] = zbs

        def emit_gelu(o):
            zbs = z_banks.pop(o)
            hts = []
            for b in range(4):
                ht = htp.tile([128, 512], bf16, tag="ht", name="ht")
                nc.scalar.activation(ht[:], zbs[b][:], AF.Gelu)
                hts.append(ht)
            ht_tiles[o] = hts

        def flush_box(last_k):
            b0 = box_stage["base"]
            gsz = box_stage["fill"]
            nc.sync.dma_start(vb[:, :, b0:b0 + gsz, :],
                              box_stage["tile"][:, 0:gsz * D])
            box_stage["tile"] = None

        def emit_dist_one(dc, b):
            kd = 4 * dc + b
            if kd >= JD:
                return
            od = opa.tile([128, D], f32, tag="oa", name="oa")
            nc.tensor.matmul(od[:], cta_d[:, dc * 128:(dc + 1) * 128],
                             w2xb[0][b], start=True, stop=True)
            stage_dist(kd, od)

        def emit_w2(o):
            hts = ht_tiles.pop(o)
            cchunk = oct_tiles[o]
            for ci in range(2):
                c = 2 * o + ci
                for b in range(4):
                    s = 4 * c + b
                    if s >= JB:
                        continue
                    ht = hts[b]
                    cam = (s % 6) // 2
                    ot = opa.tile([128, D], f32, tag="oa", name="oa")
                    nc.tensor.matmul(ot[:], ht[:, ci * 128:ci * 128 + 128],
                                     w2hi, start=True, stop=False)
                    nc.tensor.matmul(ot[:], ht[:, 256 + ci * 128:256 + ci * 128 + 128],
                                     w2lo, start=False, stop=False)
                    nc.tensor.matmul(ot[:], cchunk[:, ci * 128:(ci + 1) * 128],
                                     w2xb[cam][b], start=False, stop=True)
                    if box_stage["tile"] is None:
                        box_stage["tile"] = bstg.tile([128, 8 * D], bf16,
                                                      tag="bstage", name="bstage")
                        box_stage["fill"] = 0
                        box_stage["base"] = s
                    csel = 0 if (ccnt["i"] % 8) in (0, 2, 3, 5, 6) else 1
                    off = s - box_stage["base"]
                    copy_rr[csel](
                        box_stage["tile"][:, off * D:(off + 1) * D], ot[:])
                    ccnt["i"] += 1
                    box_stage["fill"] += 1
                    if (box_stage["fill"] == 8 or s == JB - 1
                            or (s >= 144 and box_stage["fill"] == 4)):
                        flush_box(s)
                    if b % 2 == 1:
                        emit_dist_one(o, 2 * ci + b // 2)
            oct_tiles.pop(o)

        for step in range(NOCT + 2):
            if step < NOCT:
                emit_pass2_half(step, 0)
            if 1 <= step <= NOCT:
                emit_z_mm(step - 1)
            if step < NOCT:
                emit_pass2_half(step, 1)
            if step >= 2:
                emit_w2(step - 2)
            if 1 <= step <= NOCT:
                emit_gelu(step - 1)

    nc.compile()
    return nc


def _prep_inputs(inputs):
    f32 = np.float32
    bf = ml_dtypes.bfloat16
    scale = float(np.asarray(inputs["scale"]))

    W1p = np.zeros((32, DH), f32)
    W1p[0:10] = np.asarray(inputs["geom_w1"], f32)
    W1p[6] *= 0.5
    W1p[7] *= 0.5
    W1p -= W1p.mean(axis=1, keepdims=True)

    w1b_cols = []
    for b in range(4):
        hi = np.zeros((128, 128), f32)
        hi[32 * b:32 * b + 32] = W1p[:, :128]
        lo = np.zeros((128, 128), f32)
        lo[32 * b:32 * b + 32] = W1p[:, 128:]
        w1b_cols += [hi, lo]

    W2s = scale * np.asarray(inputs["geom_w2"], f32)
    w2hi, w2lo = W2s[:128], W2s[128:]

    cat_t = np.asarray(inputs["cat_table"], f32)
    cam_t = np.asarray(inputs["cam_table"], f32)
    bias_row = (np.asarray(inputs["geom_b2"], f32)
                + np.asarray(inputs["conf_b"], f32)
                + np.asarray(inputs["center_b"], f32))
    w2xb_cols = []
    for c in range(3):
        W2X = np.zeros((32, D), f32)
        W2X[10:13] = scale * cat_t
        W2X[13] = scale * np.asarray(inputs["conf_w"], f32)[0]
        W2X[14] = scale * (bias_row + cam_t[c])
        W2X[15] = np.asarray(inputs["missing_emb"], f32)[0]
        W2X[16] = np.asarray(inputs["dist_w"], f32)[0]
        W2X[17] = np.asarray(inputs["dist_b"], f32)
        W2X[18] = scale * np.asarray(inputs["center_w"], f32)[0] * 0.5
        W2X[19] = scale * np.asarray(inputs["center_w"], f32)[1] * 0.5
        for b in range(4):
            t = np.zeros((128, D), f32)
            t[32 * b:32 * b + 32] = W2X
            w2xb_cols.append(t)

    G10 = (W1p[0:10] @ W1p[0:10].T).astype(f32)
    Gblk = np.zeros((128, 128), f32)
    for s in range(12):
        Gblk[10 * s:10 * s + 10, 10 * s:10 * s + 10] = G10

    idf32 = np.eye(128, dtype=f32)
    bpk = np.concatenate(w1b_cols + [w2hi, w2lo] + w2xb_cols + [Gblk, idf32],
                         axis=1).astype(bf)

    box = np.asarray(inputs["box_data"], f32)
    fpks = []
    for c in range(NCORES):
        rawc = box[c * BPC:(c + 1) * BPC].reshape(BPC, T * 6, 6)
        rawc = rawc.reshape(BPC, 8, JB, 6).reshape(128, 900)
        fpks.append(np.ascontiguousarray(
            np.concatenate([rawc, idf32], axis=1), dtype=f32))
    return fpks, bpk


def _fast_path_ok(inputs):
    try:
        shapes = {
            "box_data": (B, T, 6, 6), "cat_table": (3, D), "geom_w1": (10, DH),
            "geom_b1": (DH,), "ln_g": (DH,), "ln_b": (DH,), "geom_w2": (DH, D),
            "geom_b2": (D,), "conf_w": (1, D), "conf_b": (D,),
            "center_w": (2, D), "center_b": (D,), "missing_emb": (1, D),
            "dist_w": (1, D), "dist_b": (D,), "cam_table": (NCAM, D),
        }
        for k, s in shapes.items():
            if tuple(np.asarray(inputs[k]).shape) != s:
                return False
        if not np.all(np.asarray(inputs["geom_b1"]) == 0):
            return False
        if not np.all(np.asarray(inputs["ln_g"]) == 1):
            return False
        if not np.all(np.asarray(inputs["ln_b"]) == 0):
            return False
        return True
    except Exception:
        return False


def _numpy_fallback(inputs):
    import math
    f32 = np.float32
    inp = {k: np.asarray(v) for k, v in inputs.items()}
    coords = inp["box_data"][..., :4].astype(f32)
    category = inp["box_data"][..., 4].astype(np.int32)
    conf = inp["box_data"][..., 5].astype(f32)
    norm = np.array([IW, IH, IW, IH], f32)
    cn = (coords / norm).reshape(B, T, NCAM, NB, 4)
    category = category.reshape(B, T, NCAM, NB)
    conf = conf.reshape(B, T, NCAM, NB, 1)
    presence = (cn.sum(-1) != 0).astype(f32)
    sort_key = category.astype(f32) + (1.0 - presence) * 1000.0
    idx = np.argsort(sort_key, axis=-1, kind="stable")
    cn = np.take_along_axis(cn, idx[..., None], axis=-2)
    category = np.take_along_axis(category, idx, axis=-1)
    conf = np.take_along_axis(conf, idx[..., None], axis=-2)
    presence = (cn.sum(-1) != 0).astype(f32)[..., None]
    x1, y1, x2, y2 = cn[..., 0], cn[..., 1], cn[..., 2], cn[..., 3]
    w, h = x2 - x1, y2 - y1
    cx, cy = (x1 + x2) * 0.5, (y1 + y2) * 0.5
    area, aspect = w * h, w / (h + 1e-6)
    dx, dy = cx[..., 0] - cx[..., 1], cy[..., 0] - cy[..., 1]
    dist = np.sqrt(dx * dx + dy * dy)[..., None]
    dist_tok = dist @ inp["dist_w"].astype(f32) + inp["dist_b"].astype(f32)
    geom = np.stack([x1, y1, x2, y2, w, h, cx, cy, area, aspect], axis=-1)
    z = geom @ inp["geom_w1"].astype(f32) + inp["geom_b1"].astype(f32)
    mu = z.mean(-1, keepdims=True)
    var = ((z - mu) ** 2).mean(-1, keepdims=True)
    xh = (z - mu) / np.sqrt(var + 1e-5) * inp["ln_g"].astype(f32) + inp["ln_b"].astype(f32)
    try:
        from scipy.special import erf as _erf
        g = xh * 0.5 * (1.0 + _erf(xh / np.sqrt(2.0)))
    except Exception:
        verf = np.vectorize(math.erf)
        g = xh * 0.5 * (1.0 + verf(xh / np.sqrt(2.0)))
    geom_p = g @ inp["geom_w2"].astype(f32) + inp["geom_b2"].astype(f32)
    cat_emb = inp["cat_table"].astype(f32)[category]
    conf_p = conf @ inp["conf_w"].astype(f32) + inp["conf_b"].astype(f32)
    center_p = np.stack([cx, cy], axis=-1) @ inp["center_w"].astype(f32) + inp["center_b"].astype(f32)
    cam_emb = inp["cam_table"].astype(f32).reshape(1, 1, NCAM, 1, D)
    tok = (geom_p + cat_emb + conf_p + center_p + cam_emb) * float(inp["scale"])
    tok = np.where(presence == 0, inp["missing_emb"].astype(f32)[0], tok)
    out = np.concatenate([dist_tok.reshape(B, T * NCAM, D),
                          tok.reshape(B, T * NCAM * NB, D)], axis=1)
    return out.astype(np.float32)


def _run(inputs, trace=False, tmpdir=None):
    from concourse.bass_utils import run_bass_kernel_spmd

    if "nc" not in _CACHE:
        _CACHE["nc"] = _build_nc()
    nc = _CACHE["nc"]

    fpks, bpk = _prep_inputs(inputs)
    in_maps = [{"fpk": fpks[c], "bpk": bpk} for c in range(NCORES)]
    res = run_bass_kernel_spmd(nc, in_maps, list(range(NCORES)),
                               trace=trace, tmpdir=tmpdir)
    out = np.concatenate([np.asarray(res.results[c]["out"])
                          for c in range(NCORES)], axis=0)
    return out.astype(np.float32), res


def kernel(**inputs):
    if not _fast_path_ok(inputs):
        return _numpy_fallback(inputs)
    out, _ = _run(inputs)
    return out


if __name__ == "__main__":
    import reference as ref
    inputs = {k: np.asarray(v) for k, v in ref.setup_inputs().items()}
    got = kernel(**inputs)
    exp = np.load("/tmp/expected.npy")
    d = got - exp
    print("rel fro:", np.linalg.norm(d) / np.linalg.norm(exp))
    print("absmax rel:", np.abs(d).max() / np.abs(exp).max())
